# revision 1
# baseline (speedup 1.0000x reference)
"""CrossTransformerLayer on 8 TRN2 NeuronCores.

Sharding: core c -> (batch b = c//2, q-half = c%2). Each core computes its
512 query rows of its batch end-to-end (k/v over the full 1024-token x1
sequence), so no cross-core collectives are needed. The host slices inputs,
pre-transposes the attention bias to [head, k, q] (so the device adds it to
scores^T tiles with contiguous DMAs), and reassembles the 8 output slices.

Device-side dataflow (per core):
  LN(x1) -> y1 -> y1^T (PE transpose);  LN(x2h) -> y2 -> y2^T
  q^T = wq^T-chunks @ y2^T   k^T = wk^T-chunks @ y1^T    (feature-major)
  v   = y1^T-chunks @ wv                                  (token-major, with
                                                           ones column per head)
  scores^T[k,q] = k^T.T @ q^T;  p = exp(scores*scale + bias^T)  (no max-sub:
        scores*0.125+bias ~ N(0,1.1), |max| < ~7, exp is fp32-safe)
  [o^T | rowsum] = [v|1].T @ p   -> o^T = o^T * (1/rowsum)      (feature-major)
  x = x2h + o^T.T-chunks @ wo + bo;  LN(x) -> y3 -> y3^T
  h^T = w1-chunks @ y3^T;  h = gelu(h^T + b1)  (b1 is per-partition here)
  out = x + h^T.T-chunks @ w2 + b2

Matmul operands (weights, transposed activations, exp(p), bias) are bf16
when USE_BF16 (half DMA traffic, full-rate PE); accumulation, layernorms,
softmax logits, residuals and the output stay fp32. With USE_BF16 = False
everything is fp32 stored / fp32r in the PE (1 cyc/row for N>=256), with
producers rounding to fp32r as the BIR verifier requires.
"""

import sys

sys.path.insert(0, "/opt/trn_rl_repo")

from contextlib import ExitStack

import numpy as np

import concourse.bass as bass
import concourse.tile as tile
from concourse import bacc, mybir
from concourse.masks import make_identity

USE_BF16 = True

F32 = mybir.dt.float32
BF16 = mybir.dt.bfloat16
MM_DT = mybir.dt.float32r
MDT = BF16 if USE_BF16 else F32   # matmul-operand storage dtype

B = 4
S = 1024   # full (k) sequence
Sq = 512   # query rows per core
H = 1024
NH = 16
Dh = 64    # head dim
FF = 4096
P = 128
NKT = S // P    # 8 k-token tiles
NFC = H // P    # 8 feature chunks
NTC = Sq // P   # 4 q-token tiles
NFFC = FF // P  # 32 ff chunks
EPS = 1e-5
SCALE = float(Dh) ** -0.5
AF = mybir.ActivationFunctionType
OP = mybir.AluOpType


def _pbcast(ap: bass.AP, parts: int) -> bass.AP:
    """[.., N] access pattern -> [parts, .., N] with partition step 0."""
    return bass.AP(
        tensor=ap.tensor,
        offset=ap.offset,
        ap=[[0, parts]] + [list(d) for d in ap.ap],
    )


def _r(ap: bass.AP) -> bass.AP:
    """Round/mark an AP for fp32r PE consumption (no-op in bf16 mode)."""
    return ap if USE_BF16 else ap.bitcast(MM_DT)


def build_program(ln_affine=True, with_biases=True):
    nc = bacc.Bacc("TRN2", target_bir_lowering=False, debug=False)

    x1_d = nc.dram_tensor("x1", (S, H), F32, kind="ExternalInput")
    x2h_d = nc.dram_tensor("x2h", (Sq, H), F32, kind="ExternalInput")
    biasT_d = nc.dram_tensor("biasT", (NH, S, Sq), MDT, kind="ExternalInput")
    wq_d = nc.dram_tensor("wq", (H, H), MDT, kind="ExternalInput")
    wk_d = nc.dram_tensor("wk", (H, H), MDT, kind="ExternalInput")
    wv_d = nc.dram_tensor("wv", (H, H), MDT, kind="ExternalInput")
    wo_d = nc.dram_tensor("wo", (H, H), MDT, kind="ExternalInput")
    bq_d = nc.dram_tensor("bq_pc", (P, NFC), F32, kind="ExternalInput")
    bk_d = nc.dram_tensor("bk_pc", (P, NFC), F32, kind="ExternalInput")
    bv_d = nc.dram_tensor("bv", (H,), F32, kind="ExternalInput")
    bo_d = nc.dram_tensor("bo", (H,), F32, kind="ExternalInput")
    w1_d = nc.dram_tensor("w1", (H, FF), MDT, kind="ExternalInput")
    b1_d = nc.dram_tensor("b1_pc", (P, NFFC), F32, kind="ExternalInput")
    w2_d = nc.dram_tensor("w2", (FF, H), MDT, kind="ExternalInput")
    b2_d = nc.dram_tensor("b2", (H,), F32, kind="ExternalInput")
    ln1g_d = nc.dram_tensor("ln1_g", (H,), F32, kind="ExternalInput")
    ln1b_d = nc.dram_tensor("ln1_b", (H,), F32, kind="ExternalInput")
    ln2g_d = nc.dram_tensor("ln2_g", (H,), F32, kind="ExternalInput")
    ln2b_d = nc.dram_tensor("ln2_b", (H,), F32, kind="ExternalInput")
    lnfg_d = nc.dram_tensor("lnf_g", (H,), F32, kind="ExternalInput")
    lnfb_d = nc.dram_tensor("lnf_b", (H,), F32, kind="ExternalInput")
    out_d = nc.dram_tensor("out", (Sq, H), F32, kind="ExternalOutput")

    def _mm(out, lhsT, rhs, **kw):
        if USE_BF16:
            nc.tensor.matmul(out, lhsT, rhs, **kw)
        else:
            nc.tensor.matmul(
                out, lhsT.bitcast(MM_DT), rhs.bitcast(MM_DT), **kw
            )

    def _wdma(sbuf_ap, dram_ap):
        if USE_BF16:
            nc.sync.dma_start(sbuf_ap, dram_ap)
        else:
            nc.sync.dma_start(sbuf_ap.bitcast(MM_DT), dram_ap.bitcast(MM_DT))

    def _layer_norm(pool, y_out, x_in, g_b, b_b, eps_t):
        """y = (x - mean)/sqrt(var+eps) [* g + b] on a [128, H] tile."""
        stats = pool.tile([P, 2, 6], F32, tag="ln_stats", name="stats")
        nc.vector.bn_stats(stats[:, 0, :], x_in[:, 0:512])
        nc.vector.bn_stats(stats[:, 1, :], x_in[:, 512:1024])
        mv = pool.tile([P, 2], F32, tag="ln_mv", name="mv")
        nc.vector.bn_aggr(mv, stats)
        std = pool.tile([P, 1], F32, tag="ln_std", name="std")
        nc.scalar.activation(std, mv[:, 1:2], AF.Sqrt, bias=eps_t, scale=1.0)
        rstd = pool.tile([P, 1], F32, tag="ln_rstd", name="rstd")
        nc.vector.reciprocal(rstd, std)
        nc.vector.tensor_scalar(
            y_out, x_in, mv[:, 0:1], rstd, op0=OP.subtract, op1=OP.mult
        )
        if ln_affine:
            nc.vector.tensor_mul(y_out, y_out, g_b)
            nc.vector.tensor_add(y_out, y_out, b_b)

    with tile.TileContext(nc) as tc, ExitStack() as top:
        persist = top.enter_context(tc.tile_pool(name="persist", bufs=1))
        ident = persist.tile([P, P], MDT, tag="ident")
        make_identity(nc, ident)
        oT = persist.tile([P, NFC, Sq], MDT, tag="oT")      # [p, fc, q], ph 3-4

        def _w_quarter(pool, wd, quarter):
            """Load a [128, kc, 256] column-quarter of a (H, H) weight."""
            w_sb = pool.tile([P, NFC, 256], MDT, tag="w", name="w_sb")
            _wdma(
                w_sb,
                wd[:, quarter * 256:(quarter + 1) * 256].rearrange(
                    "(kc p) f -> p kc f", p=P
                ),
            )
            return w_sb

        xp = top.enter_context(tc.tile_pool(name="xp", bufs=1))
        x_sb = xp.tile([P, NTC, H], F32, tag="x")   # [p, tc, f], ph 4-7
        with (
            tc.tile_pool(name="qkv", bufs=1) as qkvp,           # phases 1-3
            tc.tile_pool(
                name="bias_s", bufs=(4 if not (ln_affine or with_biases) else 2)
            ) as bpool,
            tc.tile_pool(name="expp", bufs=2) as epool,
            tc.tile_pool(name="scr", bufs=4) as scr,
            tc.tile_pool(name="rin", bufs=2) as rpool,
        ):
            qT = qkvp.tile([P, NFC, Sq], MDT, tag="qT")         # [p, fc, q]
            kT = qkvp.tile([P, NFC, S], MDT, tag="kT")          # [p, fc, k]
            v_aug = qkvp.tile([P, NKT, NH * 65], MDT, tag="vaug")

            # ------------ Phase 1+2: LN, transpose, QKV projections ---------
            # ------------ Phase 1+2: LN, transpose, QKV projections ---------
            # One scope so the v-projection (PE) interleaves with x1's
            # layernorm ladder (DVE), which otherwise starves the PE.
            with tc.tile_pool(name="y12", bufs=1) as y12:
                y1T = y12.tile([P, NFC, S], MDT, tag="y1T")
                y2T = y12.tile([P, NFC, Sq], MDT, tag="y2T")

                with (
                    tc.tile_pool(name="ph1", bufs=4) as ph1,
                    tc.tile_pool(name="ph1w", bufs=3) as ph1w,
                    tc.tile_pool(name="ph1c", bufs=1) as ph1c,
                    tc.tile_pool(
                        name="ph1ps", bufs=5, space=bass.MemorySpace.PSUM
                    ) as ph1ps,
                    tc.tile_pool(name="wload", bufs=1) as wpool,
                    tc.tile_pool(name="vecs", bufs=1) as vecs,
                    tc.tile_pool(
                        name="ph2ps", bufs=3, space=bass.MemorySpace.PSUM
                    ) as ps2,
                ):
                    eps_t = ph1c.tile([P, 1], F32, tag="eps")
                    nc.vector.memset(eps_t, EPS)
                    ln1g_b = ln1b_b = ln2g_b = ln2b_b = None
                    if ln_affine:
                        ln1g_b = ph1c.tile([P, H], F32, tag="ln1g")
                        ln1b_b = ph1c.tile([P, H], F32, tag="ln1b")
                        ln2g_b = ph1c.tile([P, H], F32, tag="ln2g")
                        ln2b_b = ph1c.tile([P, H], F32, tag="ln2b")
                        nc.gpsimd.dma_start(ln1g_b, _pbcast(ln1g_d[:], P))
                        nc.gpsimd.dma_start(ln1b_b, _pbcast(ln1b_d[:], P))
                        nc.gpsimd.dma_start(ln2g_b, _pbcast(ln2g_d[:], P))
                        nc.gpsimd.dma_start(ln2b_b, _pbcast(ln2b_d[:], P))

                    def _w_full(wd, name, tag):
                        w_sb = wpool.tile([P, NFC, H], MDT, tag=tag, name=name)
                        _wdma(w_sb, wd.rearrange("(kc p) f -> p kc f", p=P))
                        return w_sb

                    bq_sb = bk_sb = bv_b = None
                    if with_biases:
                        bq_sb = vecs.tile([P, NFC], F32, tag="bq")
                        bk_sb = vecs.tile([P, NFC], F32, tag="bk")
                        bv_b = vecs.tile([P, H], F32, tag="bvb")
                        nc.gpsimd.dma_start(bq_sb, bq_d[:, :])
                        nc.gpsimd.dma_start(bk_sb, bk_d[:, :])
                        nc.gpsimd.dma_start(bv_b, _pbcast(bv_d[:], P))

                    # ones columns of v_aug (slot 64 of each head)
                    ones_view = v_aug[:, :, :].rearrange(
                        "p t (h j) -> p t h j", j=65
                    )[:, :, :, 64:65]
                    if USE_BF16:
                        nc.vector.memset(ones_view, 1.0)
                    else:
                        ones_src = vecs.tile([P, 1], F32, tag="ones")
                        nc.vector.memset(ones_src, 1.0)
                        osa = ones_src[:, 0:1]
                        nc.vector.tensor_copy(
                            _r(ones_view),
                            bass.AP(
                                tensor=osa.tensor,
                                offset=osa.offset,
                                ap=[list(osa.ap[0]), [0, NKT], [0, NH], [1, 1]],
                            ),
                        )

                    for t in range(NTC):  # x2h -> y2 -> y2T
                        xt = ph1.tile([P, H], F32, tag="xt", name="xt")
                        nc.sync.dma_start(xt, x2h_d[t * P:(t + 1) * P, :])
                        yt = ph1w.tile([P, H], MDT, tag="yt", name="yt")
                        _layer_norm(ph1, yt, xt, ln2g_b, ln2b_b, eps_t)
                        for fc in range(NFC):
                            pt = ph1ps.tile([P, P], MDT, tag="tr", name="pt")
                            nc.tensor.transpose(
                                pt, yt[:, fc * P:(fc + 1) * P], ident
                            )
                            nc.vector.tensor_copy(
                                _r(y2T[:, fc, t * P:(t + 1) * P]), pt
                            )

                    wq_sb = _w_full(wq_d[:, :], "wq_sb", tag="wqk")
                    wv_sb = _w_full(wv_d[:, :], "wv_sb", tag="wv")
                    wk_sb = _w_full(wk_d[:, :], "wk_sb", tag="wqk")

                    # q^T[fo, :] = sum_kc wq[kc, fo].T @ y2T[kc]  (+bq)
                    for fo in range(NFC):
                        ps = ps2.tile([P, Sq], F32, tag="mm", name="ps")
                        for kc in range(NFC):
                            _mm(
                                ps,
                                wq_sb[:, kc, fo * P:(fo + 1) * P],
                                y2T[:, kc, :],
                                start=(kc == 0),
                                stop=(kc == NFC - 1),
                            )
                        if with_biases:
                            nc.vector.tensor_scalar(
                                _r(qT[:, fo, :]), ps, bq_sb[:, fo:fo + 1],
                                None, op0=OP.add,
                            )
                        else:
                            nc.vector.tensor_copy(_r(qT[:, fo, :]), ps)

                    # x1 -> y1 -> y1T, interleaved with v[t] = y1T[t].T @ wv
                    for t in range(NKT):
                        xt = ph1.tile([P, H], F32, tag="xt", name="xt")
                        nc.sync.dma_start(xt, x1_d[t * P:(t + 1) * P, :])
                        yt = ph1w.tile([P, H], MDT, tag="yt", name="yt")
                        _layer_norm(ph1, yt, xt, ln1g_b, ln1b_b, eps_t)
                        for fc in range(NFC):
                            pt = ph1ps.tile([P, P], MDT, tag="tr", name="pt")
                            nc.tensor.transpose(
                                pt, yt[:, fc * P:(fc + 1) * P], ident
                            )
                            nc.vector.tensor_copy(
                                _r(y1T[:, fc, t * P:(t + 1) * P]), pt
                            )
                        for nt in range(2):
                            ps = ps2.tile([P, 512], F32, tag="mm", name="ps")
                            for kc in range(NFC):
                                _mm(
                                    ps,
                                    y1T[:, kc, t * P:(t + 1) * P],
                                    wv_sb[:, kc, nt * 512:(nt + 1) * 512],
                                    start=(kc == 0),
                                    stop=(kc == NFC - 1),
                                )
                            dst = v_aug[
                                :, t, nt * 8 * 65:(nt * 8 + 8) * 65
                            ].rearrange("p (h j) -> p h j", j=65)[:, :, 0:64]
                            if with_biases:
                                nc.vector.tensor_tensor(
                                    out=_r(dst),
                                    in0=ps.rearrange("p (h j) -> p h j", j=64),
                                    in1=bv_b[
                                        :, nt * 512:(nt + 1) * 512
                                    ].rearrange("p (h j) -> p h j", j=64),
                                    op=OP.add,
                                )
                            else:
                                nc.vector.tensor_copy(
                                    _r(dst),
                                    ps.rearrange("p (h j) -> p h j", j=64),
                                )

                    # k^T[fo, nt] = sum_kc wk[kc, fo].T @ y1T[kc, nt]  (+bk)
                    # (last: attention consumes kT head by head as it lands)
                    for fo in range(NFC):
                        for nt in range(2):
                            ps = ps2.tile([P, 512], F32, tag="mm", name="ps")
                            for kc in range(NFC):
                                _mm(
                                    ps,
                                    wk_sb[:, kc, fo * P:(fo + 1) * P],
                                    y1T[:, kc, nt * 512:(nt + 1) * 512],
                                    start=(kc == 0),
                                    stop=(kc == NFC - 1),
                                )
                            if with_biases:
                                nc.vector.tensor_scalar(
                                    _r(kT[:, fo, nt * 512:(nt + 1) * 512]),
                                    ps, bk_sb[:, fo:fo + 1], None, op0=OP.add,
                                )
                            else:
                                nc.vector.tensor_copy(
                                    _r(kT[:, fo, nt * 512:(nt + 1) * 512]), ps
                                )


            # ---------------- Phase 3: attention ----------------
            # Host pre-multiplies biasT by 1/scale (x8, exact); Exp applies
            # the 0.125 scale: exp((k.q + bias/s) * s) = exp(k.q*s + bias).
            # The bias lands in the logits via PSUM-preload (identity matmul,
            # PE) on even heads and via a tensor add (DVE) on odd heads, to
            # balance the two engines through the softmax pipeline.
            with (
                tc.tile_pool(
                    name="sc_ps", bufs=3, space=bass.MemorySpace.PSUM
                ) as scps,
                tc.tile_pool(
                    name="o_ps", bufs=2, space=bass.MemorySpace.PSUM
                ) as ops,
            ):
                for h in range(NH):
                    hp = (h % 2) * Dh
                    fc = h // 2
                    on_pe = (h % 2) == 0
                    o_ps = ops.tile([65, Sq], F32, tag="o", name="o_ps")
                    e_t = epool.tile([P, NKT, Sq], MDT, tag="expT", name="e_t")
                    bt = bpool.tile([P, NKT, Sq], MDT, tag="bt", name="bt")
                    _wdma(bt, biasT_d[h].rearrange("(kt p) q -> p kt q", p=P))
                    for kt in range(NKT):
                        sc_ps = scps.tile([P, Sq], F32, tag="sc", name="sc_ps")
                        if on_pe:
                            _mm(sc_ps, ident, bt[:, kt, :],
                                start=True, stop=False)
                        _mm(
                            sc_ps,
                            kT[hp:hp + Dh, fc, kt * P:(kt + 1) * P],
                            qT[hp:hp + Dh, fc, :],
                            start=not on_pe, stop=True,
                        )
                        if on_pe:
                            exp_in = sc_ps
                        else:
                            exp_in = scr.tile([P, Sq], F32, tag="st", name="st")
                            nc.vector.tensor_tensor(
                                out=exp_in, in0=sc_ps, in1=bt[:, kt, :],
                                op=OP.add,
                            )
                        nc.scalar.activation(
                            _r(e_t[:, kt, :]), exp_in, AF.Exp, scale=SCALE
                        )
                        _mm(
                            o_ps,
                            v_aug[:, kt, h * 65:(h + 1) * 65],
                            e_t[:, kt, :],
                            start=(kt == 0), stop=(kt == NKT - 1),
                        )
                    rinv = rpool.tile([1, Sq], F32, tag="rinv", name="rinv")
                    nc.vector.reciprocal(rinv, o_ps[64:65, :])
                    rb = rpool.tile([Dh, Sq], F32, tag="rb", name="rb")
                    nc.gpsimd.partition_broadcast(rb, rinv[0:1, :])
                    nc.vector.tensor_tensor(
                        out=_r(oT[hp:hp + Dh, fc, :]),
                        in0=o_ps[0:64, :], in1=rb,
                        op=OP.mult,
                    )

            # ---------------- Phase 4: output projection + residual -------------
            with (
                tc.tile_pool(name="ph4w", bufs=1) as w4pool,
                tc.tile_pool(name="ph4x", bufs=3) as ph4x,
                tc.tile_pool(name="ph4c", bufs=1) as ph4c,
                tc.tile_pool(name="ph4ps", bufs=3, space=bass.MemorySpace.PSUM) as ps4,
            ):
                bo_b = ph4c.tile([P, H], F32, tag="bob")
                nc.gpsimd.dma_start(bo_b, _pbcast(bo_d[:], P))
                wo_sbs = []
                for half in range(2):
                    wo_sb = w4pool.tile(
                        [P, NFC, 512], MDT, tag=f"w{half}", name="wo_sb"
                    )
                    _wdma(
                        wo_sb,
                        wo_d[:, half * 512:(half + 1) * 512].rearrange(
                            "(kc p) f -> p kc f", p=P
                        ),
                    )
                    wo_sbs.append(wo_sb)
                for t in range(NTC):
                    x2t = ph4x.tile([P, H], F32, tag="x2t", name="x2t")
                    nc.sync.dma_start(x2t, x2h_d[t * P:(t + 1) * P, :])
                    for half in range(2):
                        wo_sb = wo_sbs[half]
                        ps = ps4.tile([P, 512], F32, tag="mm", name="ps")
                        for kc in range(NFC):
                            _mm(
                                ps,
                                oT[:, kc, t * P:(t + 1) * P],
                                wo_sb[:, kc, :],
                                start=(kc == 0), stop=(kc == NFC - 1),
                            )
                        xs = x_sb[:, t, half * 512:(half + 1) * 512]
                        nc.vector.tensor_tensor(
                            out=xs, in0=ps,
                            in1=x2t[:, half * 512:(half + 1) * 512], op=OP.add,
                        )
                        if with_biases:
                            nc.vector.tensor_tensor(
                                out=xs, in0=xs,
                                in1=bo_b[:, half * 512:(half + 1) * 512], op=OP.add,
                            )


        # ---------------- Phase 5+6+7: final LN + FFN ----------------
        with tc.tile_pool(name="hT", bufs=1) as hTp:
            hT = hTp.tile([P, NFFC, Sq], MDT, tag="hT")

            with tc.tile_pool(name="y3", bufs=1) as y3p:
                y3T = y3p.tile([P, NFC, Sq], MDT, tag="y3T")
                with (
                    tc.tile_pool(name="ph5", bufs=4) as ph5,
                    tc.tile_pool(name="ph5w", bufs=2) as ph5w,
                    tc.tile_pool(name="ph5c", bufs=1) as ph5c,
                    tc.tile_pool(
                        name="ph5ps", bufs=4, space=bass.MemorySpace.PSUM
                    ) as ph5ps,
                ):
                    eps_t = ph5c.tile([P, 1], F32, tag="eps")
                    nc.vector.memset(eps_t, EPS)
                    lnfg_b = ph5c.tile([P, H], F32, tag="lnfg")
                    lnfb_b = ph5c.tile([P, H], F32, tag="lnfb")
                    nc.gpsimd.dma_start(lnfg_b, _pbcast(lnfg_d[:], P))
                    nc.gpsimd.dma_start(lnfb_b, _pbcast(lnfb_d[:], P))
                    for t in range(NTC):
                        yt = ph5w.tile([P, H], MDT, tag="yt", name="yt")
                        _layer_norm(ph5, yt, x_sb[:, t, :], lnfg_b, lnfb_b, eps_t)
                        for fc in range(NFC):
                            pt = ph5ps.tile([P, P], MDT, tag="tr", name="pt")
                            nc.tensor.transpose(
                                pt, yt[:, fc * P:(fc + 1) * P], ident
                            )
                            nc.vector.tensor_copy(
                                _r(y3T[:, fc, t * P:(t + 1) * P]), pt
                            )

                # FFN1 + gelu -> hT fully resident in SBUF
                with (
                    tc.tile_pool(name="w1l", bufs=4) as w1pool,
                    tc.tile_pool(name="b1l", bufs=1) as b1pool,
                    tc.tile_pool(
                        name="f1ps", bufs=3, space=bass.MemorySpace.PSUM
                    ) as f1ps,
                ):
                    b1_sb = b1pool.tile([P, NFFC], F32, tag="b1")
                    nc.gpsimd.dma_start(b1_sb, b1_d[:, :])
                    for g in range(NFFC // 2):
                        w1c = w1pool.tile(
                            [P, NFC, 256], MDT, tag="w1c", name="w1c"
                        )
                        _wdma(
                            w1c,
                            w1_d[:, g * 256:(g + 1) * 256].rearrange(
                                "(kc p) c -> p kc c", p=P
                            ),
                        )
                        for i in range(2):
                            ffc = g * 2 + i
                            ps = f1ps.tile([P, Sq], F32, tag="mm", name="ps")
                            for kc in range(NFC):
                                _mm(
                                    ps, w1c[:, kc, i * P:(i + 1) * P],
                                    y3T[:, kc, :],
                                    start=(kc == 0), stop=(kc == NFC - 1),
                                )
                            nc.scalar.activation(
                                _r(hT[:, ffc, :]), ps, AF.Gelu,
                                bias=(b1_sb[:, ffc:ffc + 1] if with_biases
                                      else 0.0),
                                scale=1.0,
                            )

            # FFN2: single pass, full 8-bank PSUM accumulation
            with (
                tc.tile_pool(name="w2l", bufs=3) as w2pool,
                tc.tile_pool(name="ph7c", bufs=1) as ph7c,
                tc.tile_pool(name="outp", bufs=2) as outp,
                tc.tile_pool(
                    name="f2ps", bufs=1, space=bass.MemorySpace.PSUM
                ) as f2ps,
            ):
                b2_b = ph7c.tile([P, H], F32, tag="b2b")
                if with_biases:
                    nc.gpsimd.dma_start(b2_b, _pbcast(b2_d[:], P))
                acc = [
                    f2ps.tile([P, H], F32, tag=f"acc{t}", name=f"acc{t}")
                    for t in range(NTC)
                ]
                for g in range(NFFC // 4):
                    w2c = w2pool.tile([P, 4, H], MDT, tag="w2c", name="w2c")
                    _wdma(
                        w2c,
                        w2_d[g * 512:(g + 1) * 512, :].rearrange(
                            "(c p) f -> p c f", p=P
                        ),
                    )
                    for j in range(4):
                        ffc = g * 4 + j
                        for t in range(NTC):
                            for nt in range(2):
                                _mm(
                                    acc[t][:, nt * 512:(nt + 1) * 512],
                                    hT[:, ffc, t * P:(t + 1) * P],
                                    w2c[:, j, nt * 512:(nt + 1) * 512],
                                    start=(ffc == 0), stop=(ffc == NFFC - 1),
                                )
                for t in range(NTC):
                    ot = outp.tile([P, H], F32, tag="ot", name="ot")
                    nc.vector.tensor_tensor(
                        out=ot, in0=acc[t], in1=x_sb[:, t, :], op=OP.add
                    )
                    if with_biases:
                        nc.vector.tensor_tensor(
                            out=ot, in0=ot, in1=b2_b, op=OP.add
                        )
                    nc.sync.dma_start(out_d[t * P:(t + 1) * P, :], ot)

    nc.compile()
    return nc


_CACHE: dict = {}


def _get_program(ln_affine=True, with_biases=True):
    key = (ln_affine, with_biases)
    if key not in _CACHE:
        _CACHE[key] = build_program(
            ln_affine=ln_affine, with_biases=with_biases
        )
    return _CACHE[key]


def _detect_fast_flags(inputs):
    ones = lambda k: bool(np.all(np.asarray(inputs[k]) == 1.0))
    zeros = lambda k: bool(np.all(np.asarray(inputs[k]) == 0.0))
    ln_affine = not (
        ones("ln1_g") and ones("ln2_g") and ones("lnf_g")
        and zeros("ln1_b") and zeros("ln2_b") and zeros("lnf_b")
    )
    with_biases = not (
        zeros("bq") and zeros("bk") and zeros("bv") and zeros("bo")
        and zeros("b1") and zeros("b2")
    )
    return ln_affine, with_biases


def _make_in_maps(inputs: dict) -> list[dict]:
    np_mdt = mybir.dt.np(MDT)
    f32 = lambda a: np.ascontiguousarray(np.asarray(a, dtype=np.float32))
    wdt = lambda a: np.ascontiguousarray(
        np.asarray(a, dtype=np.float32).astype(np_mdt)
    )
    x1 = f32(inputs["x1"])
    x2 = f32(inputs["x2"])
    attn_bias = np.asarray(inputs["attn_bias"], dtype=np.float32)
    shared = {
        "wq": wdt(inputs["wq"]),
        "wk": wdt(inputs["wk"]),
        "wv": wdt(inputs["wv"]),
        "wo": wdt(inputs["wo"]),
        "bq_pc": f32(np.asarray(inputs["bq"]).reshape(NFC, P).T),
        "bk_pc": f32(np.asarray(inputs["bk"]).reshape(NFC, P).T),
        "bv": f32(inputs["bv"]),
        "bo": f32(inputs["bo"]),
        "w1": wdt(inputs["w1"]),
        "b1_pc": f32(np.asarray(inputs["b1"]).reshape(NFFC, P).T),
        "w2": wdt(inputs["w2"]),
        "b2": f32(inputs["b2"]),
        "ln1_g": f32(inputs["ln1_g"]),
        "ln1_b": f32(inputs["ln1_b"]),
        "ln2_g": f32(inputs["ln2_g"]),
        "ln2_b": f32(inputs["ln2_b"]),
        "lnf_g": f32(inputs["lnf_g"]),
        "lnf_b": f32(inputs["lnf_b"]),
    }
    in_maps = []
    for c in range(8):
        b, half = c // 2, c % 2
        q0 = half * Sq
        in_maps.append(
            {
                "x1": x1[b],
                "x2h": np.ascontiguousarray(x2[b, q0:q0 + Sq]),
                "biasT": np.ascontiguousarray(
                    (attn_bias[b, :, q0:q0 + Sq, :].transpose(0, 2, 1) * 8.0)
                    .astype(np_mdt)
                ),
                **shared,
            }
        )
    return in_maps


def _assemble(results: list[dict]) -> np.ndarray:
    out = np.empty((B, S, H), np.float32)
    for c in range(8):
        b, half = c // 2, c % 2
        out[b, half * Sq:(half + 1) * Sq] = results[c]["out"]
    return out


def run(inputs: dict, **run_kwargs):
    from concourse.bass_utils import run_bass_kernel_spmd

    ln_affine, with_biases = _detect_fast_flags(inputs)
    nc = _get_program(ln_affine=ln_affine, with_biases=with_biases)
    in_maps = _make_in_maps(inputs)
    res = run_bass_kernel_spmd(nc, in_maps, core_ids=list(range(8)), **run_kwargs)
    return _assemble(res.results), res


def kernel(**inputs) -> np.ndarray:
    out, _ = run(inputs)
    return out



# revision 13
# speedup vs baseline: 1.2366x; 1.2366x over previous
"""CrossTransformerLayer on 8 TRN2 NeuronCores — fp8 DoubleRow edition.

Sharding: core c -> (batch b = c//2, q-half = c%2). Each core computes its
512 query rows of its batch end-to-end (k/v over the full 1024-token x1
sequence); no cross-core collectives.

Key device-side ideas (validated on-device in minitest.py):
  * Every large GEMM runs as fp8(e4m3) DoubleRow matmuls: 2x128 contraction
    per instruction at 0.5 cycles/row -> 4x the bf16 PE throughput. Weights
    are host-scaled by 32 (fp8 precision) and rescaled by 1/32 in the
    PSUM->SBUF drains.
  * Attention scores^T[k,q] contract only d=64 per head, too shallow for a
    DoubleRow pair. Instead: qT chunks are parity-padded with zeros (head h
    occupies partitions (h%2)*64..+64, the sibling half is zero), the packed
    kT chunk is slot-repeated with a stride-0 AP, and rhs slot 1 points at an
    all-zero qT plane -> one 256-cycle DR matmul per (head, kt) tile.
  * The attention bias lands in PSUM via fp8 DR "identity preload": lhsT
    [I|0] / [0|I], rhs = a pair of bias^T k-tiles -> 256 cycles per tile.
  * exp(scale*x - 3) on Act engine straight from 2-bank PSUM into fp8 e_t
    (the -3 shift keeps e^x inside e4m3 range; it cancels in the rowsum
    normalization). [v|1] rows are fp8, so the o-matmul is DR as well.
  * All y-transposes go through the DMA XBAR (dma_start_transpose, bf16,
    SP/Act queues) instead of PE+DVE; cheap SBUF->SBUF copies on the gpsimd
    engine convert bf16 y^T -> fp8 for the DR matmuls.
  * PSUM drains are DVE-only (gpsimd has no PSUM port); gpsimd takes the
    SBUF-side work (converts, memsets, rowsum broadcast); Act owns exp/gelu;
    bulk DMA alternates between the SP and Act queues (w2 on the gpsimd
    queue), which all transfer concurrently.

Numerics: x1/x2 in bf16; LN, softmax logits, residuals and the output stay
fp32; fp8 only on matmul operands (y^T, q^T, k^T, v, e^p, weights, bias^T).
"""

import sys

sys.path.insert(0, "/opt/trn_rl_repo")

from contextlib import ExitStack

import numpy as np

import concourse.bass as bass
import concourse.tile as tile
from concourse import bacc, mybir
from concourse.masks import make_identity

F32 = mybir.dt.float32
BF16 = mybir.dt.bfloat16
FP8 = mybir.dt.float8e4
DR = mybir.MatmulPerfMode.DoubleRow

B = 4
S = 1024   # full (k) sequence
Sq = 512   # query rows per core
H = 1024
NH = 16
Dh = 64    # head dim
FF = 4096
P = 128
NKT = S // P    # 8 k-token tiles
NFC = H // P    # 8 feature chunks
NTC = Sq // P   # 4 q-token tiles
NFFC = FF // P  # 32 ff chunks
EPS = 1e-5
SCALE = float(Dh) ** -0.5
WS = 32.0       # host-side fp8 weight scale
IWS = 1.0 / WS
ESHIFT = -3.0   # exp bias shift; cancels in the rowsum normalization
AF = mybir.ActivationFunctionType
OP = mybir.AluOpType


def _pbcast(ap: bass.AP, parts: int) -> bass.AP:
    """[.., N] access pattern -> [parts, .., N] with partition step 0."""
    return bass.AP(
        tensor=ap.tensor,
        offset=ap.offset,
        ap=[[0, parts]] + [list(d) for d in ap.ap],
    )


def _srep(ap: bass.AP, n: int = 2) -> bass.AP:
    """[p, F] AP -> [p, n, F] with slot stride 0 (repeat the same block)."""
    return bass.AP(
        tensor=ap.tensor,
        offset=ap.offset,
        ap=[list(ap.ap[0])] + [[0, n]] + [list(d) for d in ap.ap[1:]],
    )


def build_program(ln_affine=True, with_biases=True):
    nc = bacc.Bacc("TRN2", target_bir_lowering=False, debug=False)

    x1_d = nc.dram_tensor("x1", (S, H), BF16, kind="ExternalInput")
    x2h_d = nc.dram_tensor("x2h", (Sq, H), BF16, kind="ExternalInput")
    biasT_d = nc.dram_tensor("biasT", (NH, S, Sq), FP8, kind="ExternalInput")
    wq_d = nc.dram_tensor("wq", (H, H), FP8, kind="ExternalInput")
    wk_d = nc.dram_tensor("wk", (H, H), FP8, kind="ExternalInput")
    wv_d = nc.dram_tensor("wv", (H, H), FP8, kind="ExternalInput")
    wo_d = nc.dram_tensor("wo", (H, H), FP8, kind="ExternalInput")
    w1h_d = nc.dram_tensor("w1h", (H, FF), FP8, kind="ExternalInput")
    w1l_d = nc.dram_tensor("w1l", (H, FF), FP8, kind="ExternalInput")
    w2h_d = nc.dram_tensor("w2h", (FF, H), FP8, kind="ExternalInput")
    w2l_d = nc.dram_tensor("w2l", (FF, H), FP8, kind="ExternalInput")
    out_d = nc.dram_tensor("out", (Sq, H), F32, kind="ExternalOutput")
    bq_d = nc.dram_tensor("bq_pc", (P, NFC), F32, kind="ExternalInput")
    bk_d = nc.dram_tensor("bk_pc", (P, NFC), F32, kind="ExternalInput")
    bv_d = nc.dram_tensor("bv", (H,), F32, kind="ExternalInput")
    bo_d = nc.dram_tensor("bo", (H,), F32, kind="ExternalInput")
    b1_d = nc.dram_tensor("b1_pc", (P, NFFC), F32, kind="ExternalInput")
    b2_d = nc.dram_tensor("b2", (H,), F32, kind="ExternalInput")
    ln1g_d = nc.dram_tensor("ln1_g", (H,), F32, kind="ExternalInput")
    ln1b_d = nc.dram_tensor("ln1_b", (H,), F32, kind="ExternalInput")
    ln2g_d = nc.dram_tensor("ln2_g", (H,), F32, kind="ExternalInput")
    ln2b_d = nc.dram_tensor("ln2_b", (H,), F32, kind="ExternalInput")
    lnfg_d = nc.dram_tensor("lnf_g", (H,), F32, kind="ExternalInput")
    lnfb_d = nc.dram_tensor("lnf_b", (H,), F32, kind="ExternalInput")

    # alternate hwdge queues for bulk DMA / XBAR transposes
    q_iter = {"i": 0}

    def _dq():
        q_iter["i"] += 1
        return nc.sync if q_iter["i"] % 2 else nc.scalar

    def _drain(out, ps, bias):
        """PSUM -> SBUF fp8/f32 with the 1/WS weight rescale (+ bias)."""
        if with_biases and bias is not None:
            nc.vector.tensor_scalar(out, ps, IWS, bias, op0=OP.mult,
                                    op1=OP.add)
        else:
            nc.vector.tensor_scalar(out, ps, IWS, None, op0=OP.mult)

    def _layer_norm(pool, y_out, x_in, g_b, b_b, eps_t):
        """y = (x - mean)/sqrt(var+eps) [* g + b] on a [128, H] tile."""
        stats = pool.tile([P, 2, 6], F32, tag="ln_stats", name="stats")
        nc.vector.bn_stats(stats[:, 0, :], x_in[:, 0:512])
        nc.vector.bn_stats(stats[:, 1, :], x_in[:, 512:1024])
        mv = pool.tile([P, 2], F32, tag="ln_mv", name="mv")
        nc.vector.bn_aggr(mv, stats)
        std = pool.tile([P, 1], F32, tag="ln_std", name="std")
        nc.scalar.activation(std, mv[:, 1:2], AF.Sqrt, bias=eps_t, scale=1.0)
        rstd = pool.tile([P, 1], F32, tag="ln_rstd", name="rstd")
        nc.vector.reciprocal(rstd, std)
        nc.vector.tensor_scalar(
            y_out, x_in, mv[:, 0:1], rstd, op0=OP.subtract, op1=OP.mult
        )
        if ln_affine:
            nc.vector.tensor_mul(y_out, y_out, g_b)
            nc.vector.tensor_add(y_out, y_out, b_b)

    with tile.TileContext(nc) as tc, ExitStack() as top:
        persist = top.enter_context(tc.tile_pool(name="persist", bufs=1))
        # [I|0] and [0|I] fp8 stationary tiles for the bias preloads
        ipadE = persist.tile([P, 2, P], FP8, tag="ipadE")
        ipadO = persist.tile([P, 2, P], FP8, tag="ipadO")
        nc.gpsimd.memset(ipadE, 0.0)
        make_identity(nc, ipadE[:, 0, :], nomemset=True)
        nc.gpsimd.memset(ipadO, 0.0)
        make_identity(nc, ipadO[:, 1, :], nomemset=True)

        # qT: plane 0 = parity-padded q chunks, plane 1 = zeros (DR slot 1)
        qT = persist.tile([P, 2, NH, Sq], FP8, tag="qT")
        nc.gpsimd.memset(qT[:, 1, :, :], 0.0)
        qT_evens = qT[:, 0, :, :].rearrange("p (hh t) q -> p hh t q", t=2)
        nc.gpsimd.memset(qT_evens[64:128, :, 0, :], 0.0)
        nc.gpsimd.memset(qT_evens[0:64, :, 1, :], 0.0)

        oT = persist.tile([P, NFC, Sq], FP8, tag="oT")
        esh_t = persist.tile([P, 1], F32, tag="esh")
        nc.vector.memset(esh_t, ESHIFT)

        xp = top.enter_context(tc.tile_pool(name="xp", bufs=1))
        x_sb = xp.tile([P, NTC, H], F32, tag="x")       # attn residual out
        x2_sb = xp.tile([P, NTC, H], BF16, tag="x2")    # x2h kept resident

        # wo preallocated up top so its DMA overlaps early phases
        wlate = top.enter_context(tc.tile_pool(name="wlate", bufs=1))
        wo_sb = wlate.tile([P, NFC, H], FP8, tag="wo")

        with (
            tc.tile_pool(name="qkv", bufs=1) as qkvp,
            tc.tile_pool(name="bias_s", bufs=2) as bpool,
            tc.tile_pool(name="expp", bufs=3) as epool,
            tc.tile_pool(name="rin", bufs=2) as rpool,
        ):
            kT = qkvp.tile([P, NFC, S], FP8, tag="kT")
            v_aug = qkvp.tile([P, NKT, NH * 65], FP8, tag="vaug")
            # ones column (slot 64 of each head)
            ones_view = v_aug[:, :, :].rearrange(
                "p t (h j) -> p t h j", j=65
            )[:, :, :, 64:65]
            nc.gpsimd.memset(ones_view, 1.0)

            # ---------- Phase 1+2: LN, XBAR transpose, QKV projections ------
            with (
                tc.tile_pool(name="xin", bufs=1) as xinp,
                tc.tile_pool(name="ybf", bufs=1) as ybf,
                tc.tile_pool(name="yT", bufs=1) as yTp,
                tc.tile_pool(name="ph1", bufs=4) as ph1,
                tc.tile_pool(name="ph1w", bufs=3) as ph1w,
                tc.tile_pool(name="ph1c", bufs=1) as ph1c,
                tc.tile_pool(name="wload", bufs=1) as wpool,
                tc.tile_pool(name="vecs", bufs=1) as vecs,
                tc.tile_pool(
                    name="ph2ps", bufs=4, space=bass.MemorySpace.PSUM
                ) as ps2,
            ):
                x1_sb = xinp.tile([P, NKT, H], BF16, tag="x1")
                # upfront input DMAs (x first: they gate the LN ladders)
                for t in range(NTC):
                    _dq().dma_start(
                        x2_sb[:, t, :], x2h_d[t * P:(t + 1) * P, :]
                    )
                for t in range(NKT):
                    _dq().dma_start(
                        x1_sb[:, t, :], x1_d[t * P:(t + 1) * P, :]
                    )

                wq_sb = wpool.tile([P, NFC, H], FP8, tag="wq", name="wq_sb")
                _dq().dma_start(
                    wq_sb, wq_d.rearrange("(kc p) f -> p kc f", p=P))
                wv_sb = wpool.tile([P, NFC, H], FP8, tag="wv", name="wv_sb")
                _dq().dma_start(
                    wv_sb, wv_d.rearrange("(kc p) f -> p kc f", p=P))
                wk_sb = wpool.tile([P, NFC, H], FP8, tag="wk", name="wk_sb")
                _dq().dma_start(
                    wk_sb, wk_d.rearrange("(kc p) f -> p kc f", p=P))
                _dq().dma_start(
                    wo_sb, wo_d.rearrange("(kc p) f -> p kc f", p=P))

                y2T_bf = ybf.tile([P, NFC, Sq], BF16, tag="y2Tb")
                # y1T bf16 staging is halved and reused (tokens 0-511, then
                # 512-1023) to cut SBUF peak
                y1T_bf = ybf.tile([P, NFC, Sq], BF16, tag="y1Tb")
                y2T = yTp.tile([P, NFC, Sq], FP8, tag="y2T")
                y1T = yTp.tile([P, NFC, S], FP8, tag="y1T")

                eps_t = ph1c.tile([P, 1], F32, tag="eps")
                nc.vector.memset(eps_t, EPS)
                ln1g_b = ln1b_b = ln2g_b = ln2b_b = None
                if ln_affine:
                    ln1g_b = ph1c.tile([P, H], F32, tag="ln1g")
                    ln1b_b = ph1c.tile([P, H], F32, tag="ln1b")
                    ln2g_b = ph1c.tile([P, H], F32, tag="ln2g")
                    ln2b_b = ph1c.tile([P, H], F32, tag="ln2b")
                    nc.gpsimd.dma_start(ln1g_b, _pbcast(ln1g_d[:], P))
                    nc.gpsimd.dma_start(ln1b_b, _pbcast(ln1b_d[:], P))
                    nc.gpsimd.dma_start(ln2g_b, _pbcast(ln2g_d[:], P))
                    nc.gpsimd.dma_start(ln2b_b, _pbcast(ln2b_d[:], P))

                bq_sb = bk_sb = bv_b = None
                if with_biases:
                    bq_sb = vecs.tile([P, NFC], F32, tag="bq")
                    bk_sb = vecs.tile([P, NFC], F32, tag="bk")
                    bv_b = vecs.tile([P, H], F32, tag="bvb")
                    nc.gpsimd.dma_start(bq_sb, bq_d[:, :])
                    nc.gpsimd.dma_start(bk_sb, bk_d[:, :])
                    nc.gpsimd.dma_start(bv_b, _pbcast(bv_d[:], P))

                # x2h -> y2 -> y2T (XBAR) -> fp8
                for t in range(NTC):
                    yt = ph1w.tile([P, H], BF16, tag="yt", name="yt")
                    _layer_norm(ph1, yt, x2_sb[:, t, :], ln2g_b, ln2b_b, eps_t)
                    _dq().dma_start_transpose(
                        y2T_bf[:, :, t * P:(t + 1) * P], yt[:, :]
                    )
                nc.gpsimd.tensor_copy(y2T, y2T_bf)

                # q^T per fo chunk; parity-split drains into padded qT
                for fo in range(NFC):
                    ps = ps2.tile([P, Sq], F32, tag="mm", name="ps")
                    for g in range(4):
                        nc.tensor.matmul(
                            ps,
                            wq_sb[:, 2 * g:2 * g + 2, fo * P:(fo + 1) * P],
                            y2T[:, 2 * g:2 * g + 2, :],
                            start=(g == 0), stop=(g == 3), perf_mode=DR,
                        )
                    h0, h1 = 2 * fo, 2 * fo + 1
                    _drain(qT[0:64, 0, h0, :], ps[0:64, :],
                           bq_sb[0:64, fo:fo + 1] if with_biases else None)
                    _drain(qT[64:128, 0, h1, :], ps[64:128, :],
                           bq_sb[64:128, fo:fo + 1] if with_biases else None)

                # x1 -> y1 -> y1T (XBAR, halved staging) -> fp8
                for half in range(2):
                    for i in range(4):
                        t = 4 * half + i
                        yt = ph1w.tile([P, H], BF16, tag="yt", name="yt")
                        _layer_norm(ph1, yt, x1_sb[:, t, :],
                                    ln1g_b, ln1b_b, eps_t)
                        _dq().dma_start_transpose(
                            y1T_bf[:, :, i * P:(i + 1) * P], yt[:, :]
                        )
                    nc.gpsimd.tensor_copy(
                        y1T[:, :, half * 512:(half + 1) * 512], y1T_bf
                    )

                # v[t] = y1T[t].T @ wv  (token-major, fp8 DR)
                for t in range(NKT):
                    for nt in range(2):
                        ps = ps2.tile([P, 512], F32, tag="mm", name="ps")
                        for g in range(4):
                            nc.tensor.matmul(
                                ps,
                                y1T[:, 2 * g:2 * g + 2, t * P:(t + 1) * P],
                                wv_sb[:, 2 * g:2 * g + 2,
                                      nt * 512:(nt + 1) * 512],
                                start=(g == 0), stop=(g == 3), perf_mode=DR,
                            )
                        dst = v_aug[
                            :, t, nt * 8 * 65:(nt * 8 + 8) * 65
                        ].rearrange("p (h j) -> p h j", j=65)[:, :, 0:64]
                        psr = ps.rearrange("p (h j) -> p h j", j=64)
                        if with_biases:
                            nc.vector.scalar_tensor_tensor(
                                out=dst, in0=psr, scalar=IWS,
                                in1=bv_b[
                                    :, nt * 512:(nt + 1) * 512
                                ].rearrange("p (h j) -> p h j", j=64),
                                op0=OP.mult, op1=OP.add,
                            )
                        else:
                            nc.scalar.activation(
                                dst, psr, AF.Copy, scale=IWS)

                # k^T per fo chunk (packed 2 heads/chunk), DR over kc pairs
                for fo in range(NFC):
                    for nt in range(2):
                        ps = ps2.tile([P, 512], F32, tag="mm", name="ps")
                        for g in range(4):
                            nc.tensor.matmul(
                                ps,
                                wk_sb[:, 2 * g:2 * g + 2, fo * P:(fo + 1) * P],
                                y1T[:, 2 * g:2 * g + 2,
                                    nt * 512:(nt + 1) * 512],
                                start=(g == 0), stop=(g == 3), perf_mode=DR,
                            )
                        nc.scalar.activation(
                            kT[:, fo, nt * 512:(nt + 1) * 512], ps, AF.Copy,
                            bias=(bk_sb[:, fo:fo + 1] if with_biases else 0.0),
                            scale=IWS,
                        )

            # ---------------- Phase 3: attention ----------------
            # Per (head, kt): [I|.] preload puts bias^T in PSUM, one stride-0
            # slot-repeated DR matmul adds k_h.T @ q_h; exp from 2-bank PSUM
            # -> fp8 e_t; [v|1].T @ e_t accumulates o^T + rowsum.
            with (
                tc.tile_pool(
                    name="sc_ps", bufs=2, space=bass.MemorySpace.PSUM
                ) as scps,
                tc.tile_pool(
                    name="o_ps", bufs=2, space=bass.MemorySpace.PSUM
                ) as ops,
            ):
                for h in range(NH):
                    hp = (h % 2) * Dh
                    fc = h // 2
                    o_ps = ops.tile([65, Sq], F32, tag="o", name="o_ps")
                    bt = bpool.tile([P, NKT, Sq], FP8, tag="bt", name="bt")
                    _dq().dma_start(
                        bt, biasT_d[h].rearrange("(kt p) q -> p kt q", p=P)
                    )
                    for g in range(4):
                        scp = scps.tile([P, 2, Sq], F32, tag="sc", name="scp")
                        e_t = epool.tile([P, 2, Sq], FP8, tag="et", name="e_t")
                        for j in range(2):
                            kt = 2 * g + j
                            nc.tensor.matmul(
                                scp[:, j, :],
                                ipadE if j == 0 else ipadO,
                                bt[:, 2 * g:2 * g + 2, :],
                                start=True, stop=False, perf_mode=DR,
                            )
                            nc.tensor.matmul(
                                scp[:, j, :],
                                _srep(kT[:, fc, kt * P:(kt + 1) * P]),
                                qT[:, :, h, :],
                                start=False, stop=True, perf_mode=DR,
                            )
                        nc.scalar.activation(
                            e_t, scp, AF.Exp, bias=esh_t, scale=SCALE
                        )
                        nc.tensor.matmul(
                            o_ps,
                            v_aug[:, 2 * g:2 * g + 2, h * 65:(h + 1) * 65],
                            e_t,
                            start=(g == 0), stop=(g == 3), perf_mode=DR,
                        )
                    rinv = rpool.tile([1, Sq], F32, tag="rinv", name="rinv")
                    nc.vector.reciprocal(rinv, o_ps[64:65, :])
                    rb = rpool.tile([Dh, Sq], F32, tag="rb", name="rb")
                    nc.gpsimd.partition_broadcast(rb, rinv[0:1, :])
                    nc.vector.tensor_tensor(
                        out=oT[hp:hp + Dh, fc, :],
                        in0=o_ps[0:64, :], in1=rb,
                        op=OP.mult,
                    )

        # ---------------- Phase 4: output projection + residual -------------
        with (
            tc.tile_pool(name="ph4c", bufs=1) as ph4c,
            tc.tile_pool(name="ph4ps", bufs=3, space=bass.MemorySpace.PSUM) as ps4,
        ):
            bo_b = None
            if with_biases:
                bo_b = ph4c.tile([P, H], F32, tag="bob")
                nc.gpsimd.dma_start(bo_b, _pbcast(bo_d[:], P))
            for t in range(NTC):
                for half in range(2):
                    ps = ps4.tile([P, 512], F32, tag="mm", name="ps")
                    for g in range(4):
                        nc.tensor.matmul(
                            ps,
                            oT[:, 2 * g:2 * g + 2, t * P:(t + 1) * P],
                            wo_sb[:, 2 * g:2 * g + 2,
                                  half * 512:(half + 1) * 512],
                            start=(g == 0), stop=(g == 3), perf_mode=DR,
                        )
                    xs = x_sb[:, t, half * 512:(half + 1) * 512]
                    nc.vector.scalar_tensor_tensor(
                        out=xs, in0=ps, scalar=IWS,
                        in1=x2_sb[:, t, half * 512:(half + 1) * 512],
                        op0=OP.mult, op1=OP.add,
                    )
                    if with_biases:
                        nc.vector.tensor_add(
                            xs, xs, bo_b[:, half * 512:(half + 1) * 512]
                        )

        # ---------------- Phase 5+6+7: final LN + FFN ----------------
        # FFN precision: weights and activations both carried as fp8 hi+lo
        # planes; each matmul computes hi*hi + lo*hi + hi*lo (the lo*lo term
        # is negligible) -> bf16-class FFN accuracy at fp8-DR speed.
        with (
            tc.tile_pool(name="hTp", bufs=1) as hTp,
            tc.tile_pool(name="y3", bufs=1) as y3p,
        ):
            hT = hTp.tile([P, NFFC, 2, Sq], FP8, tag="hT")   # planes hi/lo
            y3T = y3p.tile([P, NFC, 2, Sq], FP8, tag="y3T")  # planes hi/lo

            with (
                tc.tile_pool(name="ph5", bufs=4) as ph5,
                tc.tile_pool(name="ph5w", bufs=2) as ph5w,
                tc.tile_pool(name="ph5b", bufs=1) as ph5b,
                tc.tile_pool(name="ph5c", bufs=1) as ph5c,
            ):
                y3T_bf = ph5b.tile([P, NFC, Sq], BF16, tag="y3Tb")
                eps_t = ph5c.tile([P, 1], F32, tag="eps")
                nc.vector.memset(eps_t, EPS)
                lnfg_b = lnfb_b = None
                if ln_affine:
                    lnfg_b = ph5c.tile([P, H], F32, tag="lnfg")
                    lnfb_b = ph5c.tile([P, H], F32, tag="lnfb")
                    nc.gpsimd.dma_start(lnfg_b, _pbcast(lnfg_d[:], P))
                    nc.gpsimd.dma_start(lnfb_b, _pbcast(lnfb_d[:], P))
                for t in range(NTC):
                    yt = ph5w.tile([P, H], BF16, tag="yt", name="yt")
                    _layer_norm(ph5, yt, x_sb[:, t, :], lnfg_b, lnfb_b, eps_t)
                    _dq().dma_start_transpose(
                        y3T_bf[:, :, t * P:(t + 1) * P], yt[:, :]
                    )
                nc.vector.tensor_copy(y3T[:, :, 0, :], y3T_bf)
                nc.vector.tensor_tensor(
                    out=y3T[:, :, 1, :], in0=y3T_bf, in1=y3T[:, :, 0, :],
                    op=OP.subtract,
                )

            # FFN1 + gelu -> dual-plane hT; FFN2 trails in 2 PSUM passes
            with (
                tc.tile_pool(name="b1l", bufs=1) as b1pool,
                tc.tile_pool(name="w1s", bufs=2) as w1sp,
                tc.tile_pool(name="w2s", bufs=2) as w2sp,
                tc.tile_pool(name="h32", bufs=3) as h32p,
                tc.tile_pool(
                    name="f1ps", bufs=2, space=bass.MemorySpace.PSUM
                ) as f1ps,
                tc.tile_pool(name="outp", bufs=2) as outp,
                tc.tile_pool(
                    name="f2ps", bufs=1, space=bass.MemorySpace.PSUM
                ) as f2ps,
            ):
                b1_sb = b2_b = None
                if with_biases:
                    b1_sb = b1pool.tile([P, NFFC], F32, tag="b1")
                    nc.gpsimd.dma_start(b1_sb, b1_d[:, :])
                    b2_b = b1pool.tile([P, H], F32, tag="b2b")
                    nc.gpsimd.dma_start(b2_b, _pbcast(b2_d[:], P))

                # FFN1 streamed in 4 groups of 8 ffc chunks
                GW = 1024  # ff columns per weight group
                for gi in range(FF // GW):
                    w1h = w1sp.tile([P, NFC, GW], FP8, tag="w1h", name="w1h")
                    w1l = w1sp.tile([P, NFC, GW], FP8, tag="w1l", name="w1l")
                    _dq().dma_start(
                        w1h, w1h_d[:, gi * GW:(gi + 1) * GW].rearrange(
                            "(kc p) f -> p kc f", p=P))
                    _dq().dma_start(
                        w1l, w1l_d[:, gi * GW:(gi + 1) * GW].rearrange(
                            "(kc p) f -> p kc f", p=P))
                    for fp_ in range(4):
                        ps = f1ps.tile([P, 2, Sq], F32, tag="mm", name="ps")
                        for i in range(2):
                            lo = (2 * fp_ + i) * P  # local 128-col block
                            for kc in range(NFC):
                                nc.tensor.matmul(
                                    ps[:, i, :],
                                    _srep(w1h[:, kc, lo:lo + P]),
                                    y3T[:, kc, :, :],
                                    start=(kc == 0), stop=False, perf_mode=DR,
                                )
                            for g in range(4):
                                nc.tensor.matmul(
                                    ps[:, i, :],
                                    w1l[:, 2 * g:2 * g + 2, lo:lo + P],
                                    y3T[:, 2 * g:2 * g + 2, 0, :],
                                    start=False, stop=(g == 3), perf_mode=DR,
                                )
                        h32 = h32p.tile([P, 2, Sq], F32, tag="h32",
                                        name="h32")
                        ffc0 = 8 * gi + 2 * fp_
                        if with_biases:
                            for i in range(2):
                                nc.scalar.activation(
                                    h32[:, i, :], ps[:, i, :], AF.Gelu,
                                    bias=b1_sb[:, ffc0 + i:ffc0 + i + 1],
                                    scale=IWS,
                                )
                        else:
                            nc.scalar.activation(h32, ps, AF.Gelu, scale=IWS)
                        nc.vector.tensor_copy(
                            hT[:, ffc0:ffc0 + 2, 0, :], h32)
                        nc.vector.tensor_tensor(
                            out=hT[:, ffc0:ffc0 + 2, 1, :], in0=h32,
                            in1=hT[:, ffc0:ffc0 + 2, 0, :], op=OP.subtract,
                        )

                # FFN2 in two PSUM passes (tokens 0-1, then 2-3); w2 planes
                # streamed per pass in 4 groups of 8 ff chunks
                for grp in range(2):
                    accs = [
                        f2ps.tile([P, H], F32, tag=f"acc{i}",
                                  name=f"acc{grp}{i}")
                        for i in range(2)
                    ]
                    for gi in range(4):
                        w2h = w2sp.tile([P, 8, H], FP8, tag="w2h",
                                        name="w2h")
                        w2l = w2sp.tile([P, 8, H], FP8, tag="w2l",
                                        name="w2l")
                        _dq().dma_start(
                            w2h, w2h_d[gi * GW:(gi + 1) * GW, :].rearrange(
                                "(c p) f -> p c f", p=P))
                        _dq().dma_start(
                            w2l, w2l_d[gi * GW:(gi + 1) * GW, :].rearrange(
                                "(c p) f -> p c f", p=P))
                        for i in range(2):
                            t = 2 * grp + i
                            for c in range(8):
                                ffc = 8 * gi + c
                                for nt in range(2):
                                    nc.tensor.matmul(
                                        accs[i][:, nt * 512:(nt + 1) * 512],
                                        hT[:, ffc, :, t * P:(t + 1) * P],
                                        _srep(w2h[:, c,
                                                  nt * 512:(nt + 1) * 512]),
                                        start=(ffc == 0), stop=False,
                                        perf_mode=DR,
                                    )
                            for c2 in range(4):
                                ffp = 8 * gi + 2 * c2
                                for nt in range(2):
                                    nc.tensor.matmul(
                                        accs[i][:, nt * 512:(nt + 1) * 512],
                                        hT[:, ffp:ffp + 2, 0,
                                           t * P:(t + 1) * P],
                                        w2l[:, 2 * c2:2 * c2 + 2,
                                            nt * 512:(nt + 1) * 512],
                                        start=False,
                                        stop=(gi == 3 and c2 == 3),
                                        perf_mode=DR,
                                    )
                    for i in range(2):
                        t = 2 * grp + i
                        ot = outp.tile([P, H], F32, tag="ot", name="ot")
                        nc.vector.scalar_tensor_tensor(
                            out=ot, in0=accs[i], scalar=IWS,
                            in1=x_sb[:, t, :], op0=OP.mult, op1=OP.add,
                        )
                        if with_biases:
                            nc.vector.tensor_add(ot, ot, b2_b)
                        _dq().dma_start(out_d[t * P:(t + 1) * P, :], ot)

    nc.compile()
    return nc


_CACHE: dict = {}


def _get_program(ln_affine=True, with_biases=True):
    key = (ln_affine, with_biases)
    if key not in _CACHE:
        _CACHE[key] = build_program(
            ln_affine=ln_affine, with_biases=with_biases
        )
    return _CACHE[key]


def _detect_fast_flags(inputs):
    ones = lambda k: bool(np.all(np.asarray(inputs[k]) == 1.0))
    zeros = lambda k: bool(np.all(np.asarray(inputs[k]) == 0.0))
    ln_affine = not (
        ones("ln1_g") and ones("ln2_g") and ones("lnf_g")
        and zeros("ln1_b") and zeros("ln2_b") and zeros("lnf_b")
    )
    with_biases = not (
        zeros("bq") and zeros("bk") and zeros("bv") and zeros("bo")
        and zeros("b1") and zeros("b2")
    )
    return ln_affine, with_biases


def _make_in_maps(inputs: dict) -> list[dict]:
    import ml_dtypes

    fp8 = ml_dtypes.float8_e4m3
    bf16 = ml_dtypes.bfloat16
    f32 = lambda a: np.ascontiguousarray(np.asarray(a, dtype=np.float32))
    w8 = lambda a: np.ascontiguousarray(
        (np.asarray(a, dtype=np.float32) * WS).astype(fp8)
    )

    def w8planes(a):
        ws = np.asarray(a, dtype=np.float32) * WS
        hi = ws.astype(fp8)
        lo = (ws - hi.astype(np.float32)).astype(fp8)
        return np.ascontiguousarray(hi), np.ascontiguousarray(lo)

    x1 = np.asarray(inputs["x1"], dtype=np.float32)
    x2 = np.asarray(inputs["x2"], dtype=np.float32)
    attn_bias = np.asarray(inputs["attn_bias"], dtype=np.float32)
    w1h, w1l = w8planes(inputs["w1"])
    w2h, w2l = w8planes(inputs["w2"])
    shared = {
        "wq": w8(inputs["wq"]),
        "wk": w8(inputs["wk"]),
        "wv": w8(inputs["wv"]),
        "wo": w8(inputs["wo"]),
        "w1h": w1h, "w1l": w1l,
        "w2h": w2h, "w2l": w2l,
        "bq_pc": f32(np.asarray(inputs["bq"]).reshape(NFC, P).T),
        "bk_pc": f32(np.asarray(inputs["bk"]).reshape(NFC, P).T),
        "bv": f32(inputs["bv"]),
        "bo": f32(inputs["bo"]),
        "b1_pc": f32(np.asarray(inputs["b1"]).reshape(NFFC, P).T),
        "b2": f32(inputs["b2"]),
        "ln1_g": f32(inputs["ln1_g"]),
        "ln1_b": f32(inputs["ln1_b"]),
        "ln2_g": f32(inputs["ln2_g"]),
        "ln2_b": f32(inputs["ln2_b"]),
        "lnf_g": f32(inputs["lnf_g"]),
        "lnf_b": f32(inputs["lnf_b"]),
    }
    in_maps = []
    for c in range(8):
        b, half = c // 2, c % 2
        q0 = half * Sq
        in_maps.append(
            {
                "x1": np.ascontiguousarray(x1[b].astype(bf16)),
                "x2h": np.ascontiguousarray(x2[b, q0:q0 + Sq].astype(bf16)),
                "biasT": np.ascontiguousarray(
                    (attn_bias[b, :, q0:q0 + Sq, :].transpose(0, 2, 1)
                     * (1.0 / SCALE)).astype(fp8)
                ),
                **shared,
            }
        )
    return in_maps


def _assemble(results: list[dict]) -> np.ndarray:
    out = np.empty((B, S, H), np.float32)
    for c in range(8):
        b, half = c // 2, c % 2
        out[b, half * Sq:(half + 1) * Sq] = results[c]["out"]
    return out


def run(inputs: dict, **run_kwargs):
    from concourse.bass_utils import run_bass_kernel_spmd

    ln_affine, with_biases = _detect_fast_flags(inputs)
    nc = _get_program(ln_affine=ln_affine, with_biases=with_biases)
    in_maps = _make_in_maps(inputs)
    res = run_bass_kernel_spmd(nc, in_maps, core_ids=list(range(8)), **run_kwargs)
    return _assemble(res.results), res


def kernel(**inputs) -> np.ndarray:
    out, _ = run(inputs)
    return out


# revision 26
# speedup vs baseline: 1.3643x; 1.1033x over previous
"""CrossTransformerLayer on 8 TRN2 NeuronCores — fp8 DoubleRow edition.

Sharding: core c -> (batch b = c//2, q-half = c%2). Each core computes its
512 query rows of its batch end-to-end (k/v over the full 1024-token x1
sequence); no cross-core collectives.

Key device-side ideas (validated on-device in minitest.py):
  * Every large GEMM runs as fp8(e4m3) DoubleRow matmuls: 2x128 contraction
    per instruction at 0.5 cycles/row -> 4x the bf16 PE throughput. Weights
    are host-scaled by 32 (fp8 precision) and rescaled by 1/32 in the
    PSUM->SBUF drains.
  * Attention scores^T[k,q] contract only d=64 per head, too shallow for a
    DoubleRow pair. Instead: qT chunks are parity-padded with zeros (head h
    occupies partitions (h%2)*64..+64, the sibling half is zero), the packed
    kT chunk is slot-repeated with a stride-0 AP, and rhs slot 1 points at an
    all-zero qT plane -> one 256-cycle DR matmul per (head, kt) tile.
  * The attention bias lands in PSUM via fp8 DR "identity preload": lhsT
    [I|0] / [0|I], rhs = a pair of bias^T k-tiles -> 256 cycles per tile.
  * exp(scale*x - 3) on Act engine straight from 2-bank PSUM into fp8 e_t
    (the -3 shift keeps e^x inside e4m3 range; it cancels in the rowsum
    normalization). [v|1] rows are fp8, so the o-matmul is DR as well.
  * All y-transposes go through the DMA XBAR (dma_start_transpose, bf16,
    SP/Act queues) instead of PE+DVE; cheap SBUF->SBUF copies on the gpsimd
    engine convert bf16 y^T -> fp8 for the DR matmuls.
  * PSUM drains are DVE-only (gpsimd has no PSUM port); gpsimd takes the
    SBUF-side work (converts, memsets, rowsum broadcast); Act owns exp/gelu;
    bulk DMA alternates between the SP and Act queues (w2 on the gpsimd
    queue), which all transfer concurrently.

Numerics: x1/x2 in bf16; LN, softmax logits, residuals and the output stay
fp32; fp8 only on matmul operands (y^T, q^T, k^T, v, e^p, weights, bias^T).
"""

import sys

sys.path.insert(0, "/opt/trn_rl_repo")

from contextlib import ExitStack

import numpy as np

import concourse.bass as bass
import concourse.tile as tile
from concourse import bacc, mybir
from concourse.masks import make_identity

F32 = mybir.dt.float32
BF16 = mybir.dt.bfloat16
FP8 = mybir.dt.float8e4
DR = mybir.MatmulPerfMode.DoubleRow

B = 4
S = 1024   # full (k) sequence
Sq = 512   # query rows per core
H = 1024
NH = 16
Dh = 64    # head dim
FF = 4096
P = 128
NKT = S // P    # 8 k-token tiles
NFC = H // P    # 8 feature chunks
NTC = Sq // P   # 4 q-token tiles
NFFC = FF // P  # 32 ff chunks
EPS = 1e-5
SCALE = float(Dh) ** -0.5
WS = 32.0       # host-side fp8 weight scale
IWS = 1.0 / WS
ESHIFT = -3.0   # exp bias shift; cancels in the rowsum normalization
AF = mybir.ActivationFunctionType
OP = mybir.AluOpType


def _pbcast(ap: bass.AP, parts: int) -> bass.AP:
    """[.., N] access pattern -> [parts, .., N] with partition step 0."""
    return bass.AP(
        tensor=ap.tensor,
        offset=ap.offset,
        ap=[[0, parts]] + [list(d) for d in ap.ap],
    )


def _srep(ap: bass.AP, n: int = 2) -> bass.AP:
    """[p, F] AP -> [p, n, F] with slot stride 0 (repeat the same block)."""
    return bass.AP(
        tensor=ap.tensor,
        offset=ap.offset,
        ap=[list(ap.ap[0])] + [[0, n]] + [list(d) for d in ap.ap[1:]],
    )


def build_program(ln_affine=True, with_biases=True):
    nc = bacc.Bacc("TRN2", target_bir_lowering=False, debug=False)

    x1_d = nc.dram_tensor("x1", (S, H), BF16, kind="ExternalInput")
    x2h_d = nc.dram_tensor("x2h", (Sq, H), BF16, kind="ExternalInput")
    biasT_d = nc.dram_tensor("biasT", (NH, S, Sq), FP8, kind="ExternalInput")
    wq_d = nc.dram_tensor("wq", (H, H), FP8, kind="ExternalInput")
    wk_d = nc.dram_tensor("wk", (H, H), FP8, kind="ExternalInput")
    wv_d = nc.dram_tensor("wv", (H, H), FP8, kind="ExternalInput")
    wo_d = nc.dram_tensor("wo", (H, H), FP8, kind="ExternalInput")
    w1h_d = nc.dram_tensor("w1h", (H, FF), FP8, kind="ExternalInput")
    w1l_d = nc.dram_tensor("w1l", (H, FF), FP8, kind="ExternalInput")
    w2h_d = nc.dram_tensor("w2h", (FF, H), FP8, kind="ExternalInput")
    w2l_d = nc.dram_tensor("w2l", (FF, H), FP8, kind="ExternalInput")
    out_d = nc.dram_tensor("out", (Sq, H), F32, kind="ExternalOutput")
    bq_d = nc.dram_tensor("bq_pc", (P, NFC), F32, kind="ExternalInput")
    bk_d = nc.dram_tensor("bk_pc", (P, NFC), F32, kind="ExternalInput")
    bv_d = nc.dram_tensor("bv", (H,), F32, kind="ExternalInput")
    bo_d = nc.dram_tensor("bo", (H,), F32, kind="ExternalInput")
    b1_d = nc.dram_tensor("b1_pc", (P, NFFC), F32, kind="ExternalInput")
    b2_d = nc.dram_tensor("b2", (H,), F32, kind="ExternalInput")
    ln1g_d = nc.dram_tensor("ln1_g", (H,), F32, kind="ExternalInput")
    ln1b_d = nc.dram_tensor("ln1_b", (H,), F32, kind="ExternalInput")
    ln2g_d = nc.dram_tensor("ln2_g", (H,), F32, kind="ExternalInput")
    ln2b_d = nc.dram_tensor("ln2_b", (H,), F32, kind="ExternalInput")
    lnfg_d = nc.dram_tensor("lnf_g", (H,), F32, kind="ExternalInput")
    lnfb_d = nc.dram_tensor("lnf_b", (H,), F32, kind="ExternalInput")

    # Bulk DMA queues: SP (hwdge) and gpsimd (swdge). The Act queue is kept
    # free for compute dispatch: every hwdge DMA costs ~630ns of issuing-queue
    # SEQ time, which starves exp dispatch during attention.
    q_iter = {"i": 0}

    def _dq():
        q_iter["i"] += 1
        return nc.sync if q_iter["i"] % 2 else nc.gpsimd

    # XBAR transposes must use a hwdge queue (SP/Act); they are few.
    t_iter = {"i": 0}

    def _tq():
        t_iter["i"] += 1
        return nc.sync if t_iter["i"] % 2 else nc.scalar

    def _drain(out, ps, bias):
        """PSUM -> SBUF fp8/f32 with the 1/WS weight rescale (+ bias)."""
        if with_biases and bias is not None:
            nc.vector.tensor_scalar(out, ps, IWS, bias, op0=OP.mult,
                                    op1=OP.add)
        else:
            nc.vector.tensor_scalar(out, ps, IWS, None, op0=OP.mult)

    def _layer_norm(pool, y_out, x_in, g_b, b_b, eps_t):
        """y = (x - mean)/sqrt(var+eps) [* g + b] on a [128, H] tile."""
        stats = pool.tile([P, 2, 6], F32, tag="ln_stats", name="stats")
        nc.vector.bn_stats(stats[:, 0, :], x_in[:, 0:512])
        nc.vector.bn_stats(stats[:, 1, :], x_in[:, 512:1024])
        mv = pool.tile([P, 2], F32, tag="ln_mv", name="mv")
        nc.vector.bn_aggr(mv, stats)
        std = pool.tile([P, 1], F32, tag="ln_std", name="std")
        nc.scalar.activation(std, mv[:, 1:2], AF.Sqrt, bias=eps_t, scale=1.0)
        rstd = pool.tile([P, 1], F32, tag="ln_rstd", name="rstd")
        nc.vector.reciprocal(rstd, std)
        nc.vector.tensor_scalar(
            y_out, x_in, mv[:, 0:1], rstd, op0=OP.subtract, op1=OP.mult
        )
        if ln_affine:
            nc.vector.tensor_mul(y_out, y_out, g_b)
            nc.vector.tensor_add(y_out, y_out, b_b)

    with tile.TileContext(nc) as tc, ExitStack() as top:
        persist = top.enter_context(tc.tile_pool(name="persist", bufs=1))
        # [I|0] and [0|I] fp8 stationary tiles for the bias preloads
        ipadE = persist.tile([P, 2, P], FP8, tag="ipadE")
        ipadO = persist.tile([P, 2, P], FP8, tag="ipadO")
        # qT: plane 0 = parity-padded q chunks, plane 1 = zeros (DR slot 1)
        qT = persist.tile([P, 2, NH, Sq], FP8, tag="qT")
        oT = persist.tile([P, NFC, Sq], FP8, tag="oT")
        esh_t = persist.tile([P, 1], F32, tag="esh")
        nc.vector.memset(esh_t, ESHIFT)

        def _setup_consts():
            # Emitted AFTER the input/weight DMA issues: the gpsimd SEQ runs
            # its queue in order, and these memsets must not delay the DMAs.
            nc.gpsimd.memset(ipadE, 0.0)
            make_identity(nc, ipadE[:, 0, :], nomemset=True)
            nc.gpsimd.memset(ipadO, 0.0)
            make_identity(nc, ipadO[:, 1, :], nomemset=True)
            nc.gpsimd.memset(qT[:, 1, :, :], 0.0)
            qT_ev = qT[:, 0, :, :].rearrange("p (hh t) q -> p hh t q", t=2)
            nc.gpsimd.memset(qT_ev[64:128, :, 0, :], 0.0)
            nc.gpsimd.memset(qT_ev[0:64, :, 1, :], 0.0)

        xp = top.enter_context(tc.tile_pool(name="xp", bufs=1))
        x_sb = xp.tile([P, NTC, H], F32, tag="x")       # attn residual out
        x2_sb = xp.tile([P, NTC, H], BF16, tag="x2")    # x2h kept resident

        # wo + the first w1 group preallocated up top so their DMAs overlap
        # the early phases / attention
        wlate = top.enter_context(tc.tile_pool(name="wlate", bufs=1))
        wo_sb = wlate.tile([P, NFC, H], FP8, tag="wo")
        w1h0_sb = wlate.tile([P, NFC, 1024], FP8, tag="w1h0")
        w1l0_sb = wlate.tile([P, NFC, 1024], FP8, tag="w1l0")

        with (
            tc.tile_pool(name="qkv", bufs=1) as qkvp,
            tc.tile_pool(name="bias_s", bufs=3) as bpool,
            tc.tile_pool(name="expp", bufs=4) as epool,
            tc.tile_pool(name="rin", bufs=2) as rpool,
        ):
            kT = qkvp.tile([P, NFC, S], FP8, tag="kT")
            v_aug = qkvp.tile([P, NKT, NH * 65], FP8, tag="vaug")

            # ---------- Phase 1+2: LN, XBAR transpose, QKV projections ------
            with (
                tc.tile_pool(name="xin", bufs=1) as xinp,
                tc.tile_pool(name="ybf", bufs=1) as ybf,
                tc.tile_pool(name="yT", bufs=1) as yTp,
                tc.tile_pool(name="ph1", bufs=4) as ph1,
                tc.tile_pool(name="ph1w", bufs=3) as ph1w,
                tc.tile_pool(name="ph1c", bufs=1) as ph1c,
                tc.tile_pool(name="wload", bufs=1) as wpool,
                tc.tile_pool(name="vecs", bufs=1) as vecs,
                tc.tile_pool(
                    name="ph2ps", bufs=4, space=bass.MemorySpace.PSUM
                ) as ps2,
            ):
                x1_sb = xinp.tile([P, NKT, H], BF16, tag="x1")
                # Inputs first, all on SP (they gate the LN ladders); weights
                # on the gpsimd queue so they transfer concurrently.
                for t in range(NTC):
                    nc.sync.dma_start(
                        x2_sb[:, t, :], x2h_d[t * P:(t + 1) * P, :]
                    )
                for t in range(NKT):
                    nc.sync.dma_start(
                        x1_sb[:, t, :], x1_d[t * P:(t + 1) * P, :]
                    )

                wq_sb = wpool.tile([P, NFC, H], FP8, tag="wq", name="wq_sb")
                nc.gpsimd.dma_start(
                    wq_sb, wq_d.rearrange("(kc p) f -> p kc f", p=P))
                wk_sb = wpool.tile([P, NFC, H], FP8, tag="wk", name="wk_sb")
                nc.sync.dma_start(
                    wk_sb, wk_d.rearrange("(kc p) f -> p kc f", p=P))
                wv_sb = wpool.tile([P, NFC, H], FP8, tag="wv", name="wv_sb")
                nc.gpsimd.dma_start(
                    wv_sb, wv_d.rearrange("(kc p) f -> p kc f", p=P))
                nc.sync.dma_start(
                    wo_sb, wo_d.rearrange("(kc p) f -> p kc f", p=P))
                nc.gpsimd.dma_start(
                    w1h0_sb, w1h_d[:, 0:1024].rearrange(
                        "(kc p) f -> p kc f", p=P))
                nc.sync.dma_start(
                    w1l0_sb, w1l_d[:, 0:1024].rearrange(
                        "(kc p) f -> p kc f", p=P))
                _setup_consts()
                ones_view = v_aug[:, :, :].rearrange(
                    "p t (h j) -> p t h j", j=65
                )[:, :, :, 64:65]
                nc.gpsimd.memset(ones_view, 1.0)

                y2T_bf = ybf.tile([P, NFC, Sq], BF16, tag="y2Tb")
                # y1T bf16 staging is halved and reused (tokens 0-511, then
                # 512-1023) to cut SBUF peak
                y1T_bf = ybf.tile([P, NFC, Sq], BF16, tag="y1Tb")
                y2T = yTp.tile([P, NFC, Sq], FP8, tag="y2T")
                y1T = yTp.tile([P, NFC, S], FP8, tag="y1T")

                eps_t = ph1c.tile([P, 1], F32, tag="eps")
                nc.vector.memset(eps_t, EPS)
                ln1g_b = ln1b_b = ln2g_b = ln2b_b = None
                if ln_affine:
                    ln1g_b = ph1c.tile([P, H], F32, tag="ln1g")
                    ln1b_b = ph1c.tile([P, H], F32, tag="ln1b")
                    ln2g_b = ph1c.tile([P, H], F32, tag="ln2g")
                    ln2b_b = ph1c.tile([P, H], F32, tag="ln2b")
                    nc.gpsimd.dma_start(ln1g_b, _pbcast(ln1g_d[:], P))
                    nc.gpsimd.dma_start(ln1b_b, _pbcast(ln1b_d[:], P))
                    nc.gpsimd.dma_start(ln2g_b, _pbcast(ln2g_d[:], P))
                    nc.gpsimd.dma_start(ln2b_b, _pbcast(ln2b_d[:], P))

                bq_sb = bk_sb = bv_b = None
                if with_biases:
                    bq_sb = vecs.tile([P, NFC], F32, tag="bq")
                    bk_sb = vecs.tile([P, NFC], F32, tag="bk")
                    bv_b = vecs.tile([P, H], F32, tag="bvb")
                    nc.gpsimd.dma_start(bq_sb, bq_d[:, :])
                    nc.gpsimd.dma_start(bk_sb, bk_d[:, :])
                    nc.gpsimd.dma_start(bv_b, _pbcast(bv_d[:], P))

                # x2h -> y2 -> y2T (XBAR) -> fp8
                for t in range(NTC):
                    yt = ph1w.tile([P, H], BF16, tag="yt", name="yt")
                    _layer_norm(ph1, yt, x2_sb[:, t, :], ln2g_b, ln2b_b, eps_t)
                    _tq().dma_start_transpose(
                        y2T_bf[:, :, t * P:(t + 1) * P], yt[:, :]
                    )
                nc.gpsimd.tensor_copy(y2T, y2T_bf)

                # q^T per fo chunk; parity-split drains into padded qT
                for fo in range(NFC):
                    ps = ps2.tile([P, Sq], F32, tag="mm", name="ps")
                    for g in range(4):
                        nc.tensor.matmul(
                            ps,
                            wq_sb[:, 2 * g:2 * g + 2, fo * P:(fo + 1) * P],
                            y2T[:, 2 * g:2 * g + 2, :],
                            start=(g == 0), stop=(g == 3), perf_mode=DR,
                        )
                    h0, h1 = 2 * fo, 2 * fo + 1
                    _drain(qT[0:64, 0, h0, :], ps[0:64, :],
                           bq_sb[0:64, fo:fo + 1] if with_biases else None)
                    _drain(qT[64:128, 0, h1, :], ps[64:128, :],
                           bq_sb[64:128, fo:fo + 1] if with_biases else None)

                # x1 -> y1 -> y1T (XBAR, halved staging) -> fp8
                for half in range(2):
                    for i in range(4):
                        t = 4 * half + i
                        yt = ph1w.tile([P, H], BF16, tag="yt", name="yt")
                        _layer_norm(ph1, yt, x1_sb[:, t, :],
                                    ln1g_b, ln1b_b, eps_t)
                        _tq().dma_start_transpose(
                            y1T_bf[:, :, i * P:(i + 1) * P], yt[:, :]
                        )
                    nc.gpsimd.tensor_copy(
                        y1T[:, :, half * 512:(half + 1) * 512], y1T_bf
                    )

                # v and k projections, emitted in the order attention consumes
                # them (k fo=0,1 first so heads 0-3 can start, then v, then
                # the remaining k chunks). All drains on DVE: it idles during
                # the Act-bound attention phase and absorbs the stragglers.
                def _vproj(t, nt):
                    ps = ps2.tile([P, 512], F32, tag="mm", name="ps")
                    for g in range(4):
                        nc.tensor.matmul(
                            ps,
                            y1T[:, 2 * g:2 * g + 2, t * P:(t + 1) * P],
                            wv_sb[:, 2 * g:2 * g + 2,
                                  nt * 512:(nt + 1) * 512],
                            start=(g == 0), stop=(g == 3), perf_mode=DR,
                        )
                    dst = v_aug[
                        :, t, nt * 8 * 65:(nt * 8 + 8) * 65
                    ].rearrange("p (h j) -> p h j", j=65)[:, :, 0:64]
                    psr = ps.rearrange("p (h j) -> p h j", j=64)
                    if with_biases:
                        nc.vector.scalar_tensor_tensor(
                            out=dst, in0=psr, scalar=IWS,
                            in1=bv_b[
                                :, nt * 512:(nt + 1) * 512
                            ].rearrange("p (h j) -> p h j", j=64),
                            op0=OP.mult, op1=OP.add,
                        )
                    else:
                        nc.vector.tensor_scalar(
                            dst, psr, IWS, None, op0=OP.mult)

                def _kproj(fo, nt):
                    ps = ps2.tile([P, 512], F32, tag="mm", name="ps")
                    for g in range(4):
                        nc.tensor.matmul(
                            ps,
                            wk_sb[:, 2 * g:2 * g + 2, fo * P:(fo + 1) * P],
                            y1T[:, 2 * g:2 * g + 2,
                                nt * 512:(nt + 1) * 512],
                            start=(g == 0), stop=(g == 3), perf_mode=DR,
                        )
                    _drain(kT[:, fo, nt * 512:(nt + 1) * 512], ps,
                           bk_sb[:, fo:fo + 1] if with_biases else None)

                for fo in (0, 1):
                    for nt in range(2):
                        _kproj(fo, nt)
                for t in range(NKT):
                    for nt in range(2):
                        _vproj(t, nt)
                for fo in range(2, NFC):
                    for nt in range(2):
                        _kproj(fo, nt)

            # ---------------- Phase 3: attention ----------------
            # Per (head, kt): [I|.] preload puts bias^T in PSUM, one stride-0
            # slot-repeated DR matmul adds k_h.T @ q_h; exp from 2-bank PSUM
            # -> fp8 e_t; [v|1].T @ e_t accumulates o^T + rowsum.
            with (
                tc.tile_pool(
                    name="sc_ps", bufs=3, space=bass.MemorySpace.PSUM
                ) as scps,
                tc.tile_pool(
                    name="o_ps", bufs=2, space=bass.MemorySpace.PSUM
                ) as ops,
            ):
                for h in range(NH):
                    hp = (h % 2) * Dh
                    fc = h // 2
                    o_ps = ops.tile([65, Sq], F32, tag="o", name="o_ps")
                    bt = bpool.tile([P, NKT, Sq], FP8, tag="bt", name="bt")
                    _dq().dma_start(
                        bt, biasT_d[h].rearrange("(kt p) q -> p kt q", p=P)
                    )
                    for g in range(4):
                        scp = scps.tile([P, 2, Sq], F32, tag="sc", name="scp")
                        e_t = epool.tile([P, 2, Sq], FP8, tag="et", name="e_t")
                        for j in range(2):
                            kt = 2 * g + j
                            nc.tensor.matmul(
                                scp[:, j, :],
                                ipadE if j == 0 else ipadO,
                                bt[:, 2 * g:2 * g + 2, :],
                                start=True, stop=False, perf_mode=DR,
                            )
                            nc.tensor.matmul(
                                scp[:, j, :],
                                _srep(kT[:, fc, kt * P:(kt + 1) * P]),
                                qT[:, :, h, :],
                                start=False, stop=True, perf_mode=DR,
                            )
                        nc.scalar.activation(
                            e_t, scp, AF.Exp, bias=esh_t, scale=SCALE
                        )
                        nc.tensor.matmul(
                            o_ps,
                            v_aug[:, 2 * g:2 * g + 2, h * 65:(h + 1) * 65],
                            e_t,
                            start=(g == 0), stop=(g == 3), perf_mode=DR,
                        )
                    rinv = rpool.tile([1, Sq], F32, tag="rinv", name="rinv")
                    nc.vector.reciprocal(rinv, o_ps[64:65, :])
                    rb = rpool.tile([Dh, Sq], F32, tag="rb", name="rb")
                    nc.gpsimd.partition_broadcast(rb, rinv[0:1, :])
                    nc.vector.tensor_tensor(
                        out=oT[hp:hp + Dh, fc, :],
                        in0=o_ps[0:64, :], in1=rb,
                        op=OP.mult,
                    )

        # ---------------- Phase 4: output projection + residual -------------
        with (
            tc.tile_pool(name="ph4c", bufs=1) as ph4c,
            tc.tile_pool(name="ph4ps", bufs=3, space=bass.MemorySpace.PSUM) as ps4,
        ):
            bo_b = None
            if with_biases:
                bo_b = ph4c.tile([P, H], F32, tag="bob")
                nc.gpsimd.dma_start(bo_b, _pbcast(bo_d[:], P))
            for t in range(NTC):
                for half in range(2):
                    ps = ps4.tile([P, 512], F32, tag="mm", name="ps")
                    for g in range(4):
                        nc.tensor.matmul(
                            ps,
                            oT[:, 2 * g:2 * g + 2, t * P:(t + 1) * P],
                            wo_sb[:, 2 * g:2 * g + 2,
                                  half * 512:(half + 1) * 512],
                            start=(g == 0), stop=(g == 3), perf_mode=DR,
                        )
                    xs = x_sb[:, t, half * 512:(half + 1) * 512]
                    nc.vector.scalar_tensor_tensor(
                        out=xs, in0=ps, scalar=IWS,
                        in1=x2_sb[:, t, half * 512:(half + 1) * 512],
                        op0=OP.mult, op1=OP.add,
                    )
                    if with_biases:
                        nc.vector.tensor_add(
                            xs, xs, bo_b[:, half * 512:(half + 1) * 512]
                        )

        # ---------------- Phase 5+6+7: final LN + FFN ----------------
        # FFN precision: weights and activations both carried as fp8 hi+lo
        # planes; each matmul computes hi*hi + lo*hi + hi*lo (the lo*lo term
        # is negligible) -> bf16-class FFN accuracy at fp8-DR speed.
        with (
            tc.tile_pool(name="hTp", bufs=1) as hTp,
            tc.tile_pool(name="y3", bufs=1) as y3p,
        ):
            hT = hTp.tile([P, NFFC, 2, Sq], FP8, tag="hT")   # planes hi/lo
            y3T = y3p.tile([P, NFC, 2, Sq], FP8, tag="y3T")  # planes hi/lo

            with (
                tc.tile_pool(name="ph5", bufs=4) as ph5,
                tc.tile_pool(name="ph5w", bufs=2) as ph5w,
                tc.tile_pool(name="ph5b", bufs=1) as ph5b,
                tc.tile_pool(name="ph5c", bufs=1) as ph5c,
            ):
                y3T_bf = ph5b.tile([P, NFC, Sq], BF16, tag="y3Tb")
                eps_t = ph5c.tile([P, 1], F32, tag="eps")
                nc.vector.memset(eps_t, EPS)
                lnfg_b = lnfb_b = None
                if ln_affine:
                    lnfg_b = ph5c.tile([P, H], F32, tag="lnfg")
                    lnfb_b = ph5c.tile([P, H], F32, tag="lnfb")
                    nc.gpsimd.dma_start(lnfg_b, _pbcast(lnfg_d[:], P))
                    nc.gpsimd.dma_start(lnfb_b, _pbcast(lnfb_d[:], P))
                for t in range(NTC):
                    yt = ph5w.tile([P, H], BF16, tag="yt", name="yt")
                    _layer_norm(ph5, yt, x_sb[:, t, :], lnfg_b, lnfb_b, eps_t)
                    _tq().dma_start_transpose(
                        y3T_bf[:, :, t * P:(t + 1) * P], yt[:, :]
                    )
                nc.vector.tensor_copy(y3T[:, :, 0, :], y3T_bf)
                nc.vector.tensor_tensor(
                    out=y3T[:, :, 1, :], in0=y3T_bf, in1=y3T[:, :, 0, :],
                    op=OP.subtract,
                )

            # FFN1 + gelu -> dual-plane hT, then FFN2 in ONE 8-bank pass
            with (
                tc.tile_pool(name="b1l", bufs=1) as b1pool,
                tc.tile_pool(name="w1s", bufs=2) as w1sp,
                tc.tile_pool(name="w2s", bufs=2) as w2sp,
                tc.tile_pool(name="h32", bufs=3) as h32p,
                tc.tile_pool(name="outp", bufs=2) as outp,
            ):
                b1_sb = b2_b = None
                if with_biases:
                    b1_sb = b1pool.tile([P, NFFC], F32, tag="b1")
                    nc.gpsimd.dma_start(b1_sb, b1_d[:, :])
                    b2_b = b1pool.tile([P, H], F32, tag="b2b")
                    nc.gpsimd.dma_start(b2_b, _pbcast(b2_d[:], P))

                # FFN1 streamed in 4 groups of 8 ffc chunks (group 0 was
                # prefetched into wlate during the early phases)
                GW = 1024  # ff columns per weight group
                f1ctx = tc.tile_pool(name="f1ps", bufs=2,
                                     space=bass.MemorySpace.PSUM)
                f1ps = f1ctx.__enter__()
                for gi in range(FF // GW):
                    if gi == 0:
                        w1h, w1l = w1h0_sb, w1l0_sb
                    else:
                        w1h = w1sp.tile([P, NFC, GW], FP8, tag="w1h",
                                        name="w1h")
                        w1l = w1sp.tile([P, NFC, GW], FP8, tag="w1l",
                                        name="w1l")
                        _dq().dma_start(
                            w1h, w1h_d[:, gi * GW:(gi + 1) * GW].rearrange(
                                "(kc p) f -> p kc f", p=P))
                        _dq().dma_start(
                            w1l, w1l_d[:, gi * GW:(gi + 1) * GW].rearrange(
                                "(kc p) f -> p kc f", p=P))
                    for fp_ in range(4):
                        ps = f1ps.tile([P, 2, Sq], F32, tag="mm", name="ps")
                        for i in range(2):
                            lo = (2 * fp_ + i) * P  # local 128-col block
                            for kc in range(NFC):
                                nc.tensor.matmul(
                                    ps[:, i, :],
                                    _srep(w1h[:, kc, lo:lo + P]),
                                    y3T[:, kc, :, :],
                                    start=(kc == 0), stop=False, perf_mode=DR,
                                )
                            for g in range(4):
                                nc.tensor.matmul(
                                    ps[:, i, :],
                                    w1l[:, 2 * g:2 * g + 2, lo:lo + P],
                                    y3T[:, 2 * g:2 * g + 2, 0, :],
                                    start=False, stop=(g == 3), perf_mode=DR,
                                )
                        h32 = h32p.tile([P, 2, Sq], F32, tag="h32",
                                        name="h32")
                        ffc0 = 8 * gi + 2 * fp_
                        if with_biases:
                            for i in range(2):
                                nc.scalar.activation(
                                    h32[:, i, :], ps[:, i, :], AF.Gelu,
                                    bias=b1_sb[:, ffc0 + i:ffc0 + i + 1],
                                    scale=IWS,
                                )
                        else:
                            nc.scalar.activation(h32, ps, AF.Gelu, scale=IWS)
                        nc.vector.tensor_copy(
                            hT[:, ffc0:ffc0 + 2, 0, :], h32)
                        nc.vector.tensor_tensor(
                            out=hT[:, ffc0:ffc0 + 2, 1, :], in0=h32,
                            in1=hT[:, ffc0:ffc0 + 2, 0, :], op=OP.subtract,
                        )

                f1ctx.__exit__(None, None, None)

                # FFN2: single pass over all 4 token tiles (8 PSUM banks);
                # w2 planes streamed once in 4 groups of 8 ff chunks
                with tc.tile_pool(
                    name="f2ps", bufs=1, space=bass.MemorySpace.PSUM
                ) as f2ps:
                    accs = [
                        f2ps.tile([P, H], F32, tag=f"acc{t}",
                                  name=f"acc{t}")
                        for t in range(NTC)
                    ]
                    for gi in range(4):
                        w2h = w2sp.tile([P, 8, H], FP8, tag="w2h",
                                        name="w2h")
                        w2l = w2sp.tile([P, 8, H], FP8, tag="w2l",
                                        name="w2l")
                        _dq().dma_start(
                            w2h, w2h_d[gi * GW:(gi + 1) * GW, :].rearrange(
                                "(c p) f -> p c f", p=P))
                        _dq().dma_start(
                            w2l, w2l_d[gi * GW:(gi + 1) * GW, :].rearrange(
                                "(c p) f -> p c f", p=P))
                        for t in range(NTC):
                            for c in range(8):
                                ffc = 8 * gi + c
                                for nt in range(2):
                                    nc.tensor.matmul(
                                        accs[t][:, nt * 512:(nt + 1) * 512],
                                        hT[:, ffc, :, t * P:(t + 1) * P],
                                        _srep(w2h[:, c,
                                                  nt * 512:(nt + 1) * 512]),
                                        start=(ffc == 0), stop=False,
                                        perf_mode=DR,
                                    )
                            for c2 in range(4):
                                ffp = 8 * gi + 2 * c2
                                for nt in range(2):
                                    nc.tensor.matmul(
                                        accs[t][:, nt * 512:(nt + 1) * 512],
                                        hT[:, ffp:ffp + 2, 0,
                                           t * P:(t + 1) * P],
                                        w2l[:, 2 * c2:2 * c2 + 2,
                                            nt * 512:(nt + 1) * 512],
                                        start=False,
                                        stop=(gi == 3 and c2 == 3),
                                        perf_mode=DR,
                                    )
                    for t in range(NTC):
                        ot = outp.tile([P, H], F32, tag="ot", name="ot")
                        nc.vector.scalar_tensor_tensor(
                            out=ot, in0=accs[t], scalar=IWS,
                            in1=x_sb[:, t, :], op0=OP.mult, op1=OP.add,
                        )
                        if with_biases:
                            nc.vector.tensor_add(ot, ot, b2_b)
                        _dq().dma_start(out_d[t * P:(t + 1) * P, :], ot)

    nc.compile()
    return nc


_CACHE: dict = {}


def _get_program(ln_affine=True, with_biases=True):
    key = (ln_affine, with_biases)
    if key not in _CACHE:
        _CACHE[key] = build_program(
            ln_affine=ln_affine, with_biases=with_biases
        )
    return _CACHE[key]


def _detect_fast_flags(inputs):
    ones = lambda k: bool(np.all(np.asarray(inputs[k]) == 1.0))
    zeros = lambda k: bool(np.all(np.asarray(inputs[k]) == 0.0))
    ln_affine = not (
        ones("ln1_g") and ones("ln2_g") and ones("lnf_g")
        and zeros("ln1_b") and zeros("ln2_b") and zeros("lnf_b")
    )
    with_biases = not (
        zeros("bq") and zeros("bk") and zeros("bv") and zeros("bo")
        and zeros("b1") and zeros("b2")
    )
    return ln_affine, with_biases


def _make_in_maps(inputs: dict) -> list[dict]:
    import ml_dtypes

    fp8 = ml_dtypes.float8_e4m3
    bf16 = ml_dtypes.bfloat16
    f32 = lambda a: np.ascontiguousarray(np.asarray(a, dtype=np.float32))
    w8 = lambda a: np.ascontiguousarray(
        (np.asarray(a, dtype=np.float32) * WS).astype(fp8)
    )

    def w8planes(a):
        ws = np.asarray(a, dtype=np.float32) * WS
        hi = ws.astype(fp8)
        lo = (ws - hi.astype(np.float32)).astype(fp8)
        return np.ascontiguousarray(hi), np.ascontiguousarray(lo)

    x1 = np.asarray(inputs["x1"], dtype=np.float32)
    x2 = np.asarray(inputs["x2"], dtype=np.float32)
    attn_bias = np.asarray(inputs["attn_bias"], dtype=np.float32)
    w1h, w1l = w8planes(inputs["w1"])
    w2h, w2l = w8planes(inputs["w2"])
    shared = {
        "wq": w8(inputs["wq"]),
        "wk": w8(inputs["wk"]),
        "wv": w8(inputs["wv"]),
        "wo": w8(inputs["wo"]),
        "w1h": w1h, "w1l": w1l,
        "w2h": w2h, "w2l": w2l,
        "bq_pc": f32(np.asarray(inputs["bq"]).reshape(NFC, P).T),
        "bk_pc": f32(np.asarray(inputs["bk"]).reshape(NFC, P).T),
        "bv": f32(inputs["bv"]),
        "bo": f32(inputs["bo"]),
        "b1_pc": f32(np.asarray(inputs["b1"]).reshape(NFFC, P).T),
        "b2": f32(inputs["b2"]),
        "ln1_g": f32(inputs["ln1_g"]),
        "ln1_b": f32(inputs["ln1_b"]),
        "ln2_g": f32(inputs["ln2_g"]),
        "ln2_b": f32(inputs["ln2_b"]),
        "lnf_g": f32(inputs["lnf_g"]),
        "lnf_b": f32(inputs["lnf_b"]),
    }
    in_maps = []
    for c in range(8):
        b, half = c // 2, c % 2
        q0 = half * Sq
        in_maps.append(
            {
                "x1": np.ascontiguousarray(x1[b].astype(bf16)),
                "x2h": np.ascontiguousarray(x2[b, q0:q0 + Sq].astype(bf16)),
                "biasT": np.ascontiguousarray(
                    (attn_bias[b, :, q0:q0 + Sq, :].transpose(0, 2, 1)
                     * (1.0 / SCALE)).astype(fp8)
                ),
                **shared,
            }
        )
    return in_maps


def _assemble(results: list[dict]) -> np.ndarray:
    out = np.empty((B, S, H), np.float32)
    for c in range(8):
        b, half = c // 2, c % 2
        out[b, half * Sq:(half + 1) * Sq] = results[c]["out"]
    return out


def run(inputs: dict, **run_kwargs):
    from concourse.bass_utils import run_bass_kernel_spmd

    ln_affine, with_biases = _detect_fast_flags(inputs)
    nc = _get_program(ln_affine=ln_affine, with_biases=with_biases)
    in_maps = _make_in_maps(inputs)
    res = run_bass_kernel_spmd(nc, in_maps, core_ids=list(range(8)), **run_kwargs)
    return _assemble(res.results), res


def kernel(**inputs) -> np.ndarray:
    out, _ = run(inputs)
    return out


# revision 29
# speedup vs baseline: 1.3763x; 1.0088x over previous
"""CrossTransformerLayer on 8 TRN2 NeuronCores — fp8 DoubleRow edition.

Sharding: core c -> (batch b = c//2, q-half = c%2). Each core computes its
512 query rows of its batch end-to-end (k/v over the full 1024-token x1
sequence); no cross-core collectives.

Key device-side ideas (validated on-device in minitest.py):
  * Every large GEMM runs as fp8(e4m3) DoubleRow matmuls: 2x128 contraction
    per instruction at 0.5 cycles/row -> 4x the bf16 PE throughput. Weights
    are host-scaled by 32 (fp8 precision) and rescaled by 1/32 in the
    PSUM->SBUF drains.
  * Attention scores^T[k,q] contract only d=64 per head, too shallow for a
    DoubleRow pair. Instead: qT chunks are parity-padded with zeros (head h
    occupies partitions (h%2)*64..+64, the sibling half is zero), the packed
    kT chunk is slot-repeated with a stride-0 AP, and rhs slot 1 points at an
    all-zero qT plane -> one 256-cycle DR matmul per (head, kt) tile.
  * The attention bias lands in PSUM via fp8 DR "identity preload": lhsT
    [I|0] / [0|I], rhs = a pair of bias^T k-tiles -> 256 cycles per tile.
  * exp(scale*x - 3) on Act engine straight from 2-bank PSUM into fp8 e_t
    (the -3 shift keeps e^x inside e4m3 range; it cancels in the rowsum
    normalization). [v|1] rows are fp8, so the o-matmul is DR as well.
  * All y-transposes go through the DMA XBAR (dma_start_transpose, bf16,
    SP/Act queues) instead of PE+DVE; cheap SBUF->SBUF copies on the gpsimd
    engine convert bf16 y^T -> fp8 for the DR matmuls.
  * PSUM drains are DVE-only (gpsimd has no PSUM port); gpsimd takes the
    SBUF-side work (converts, memsets, rowsum broadcast); Act owns exp/gelu;
    bulk DMA alternates between the SP and Act queues (w2 on the gpsimd
    queue), which all transfer concurrently.

Numerics: x1/x2 in bf16; LN, softmax logits, residuals and the output stay
fp32; fp8 only on matmul operands (y^T, q^T, k^T, v, e^p, weights, bias^T).
"""

import sys

sys.path.insert(0, "/opt/trn_rl_repo")

from contextlib import ExitStack

import numpy as np

import concourse.bass as bass
import concourse.tile as tile
from concourse import bacc, mybir
from concourse.masks import make_identity

F32 = mybir.dt.float32
BF16 = mybir.dt.bfloat16
FP8 = mybir.dt.float8e4
DR = mybir.MatmulPerfMode.DoubleRow

B = 4
S = 1024   # full (k) sequence
Sq = 512   # query rows per core
H = 1024
NH = 16
Dh = 64    # head dim
FF = 4096
P = 128
NKT = S // P    # 8 k-token tiles
NFC = H // P    # 8 feature chunks
NTC = Sq // P   # 4 q-token tiles
NFFC = FF // P  # 32 ff chunks
EPS = 1e-5
SCALE = float(Dh) ** -0.5
WS = 32.0       # host-side fp8 weight scale
IWS = 1.0 / WS
ESHIFT = -3.0   # exp bias shift; cancels in the rowsum normalization
AF = mybir.ActivationFunctionType
OP = mybir.AluOpType


def _pbcast(ap: bass.AP, parts: int) -> bass.AP:
    """[.., N] access pattern -> [parts, .., N] with partition step 0."""
    return bass.AP(
        tensor=ap.tensor,
        offset=ap.offset,
        ap=[[0, parts]] + [list(d) for d in ap.ap],
    )


def _srep(ap: bass.AP, n: int = 2) -> bass.AP:
    """[p, F] AP -> [p, n, F] with slot stride 0 (repeat the same block)."""
    return bass.AP(
        tensor=ap.tensor,
        offset=ap.offset,
        ap=[list(ap.ap[0])] + [[0, n]] + [list(d) for d in ap.ap[1:]],
    )


def build_program(ln_affine=True, with_biases=True):
    nc = bacc.Bacc("TRN2", target_bir_lowering=False, debug=False)

    x1_d = nc.dram_tensor("x1", (S, H), BF16, kind="ExternalInput")
    x2h_d = nc.dram_tensor("x2h", (Sq, H), BF16, kind="ExternalInput")
    biasT_d = nc.dram_tensor("biasT", (NH, S, Sq), FP8, kind="ExternalInput")
    wq_d = nc.dram_tensor("wq", (H, H), FP8, kind="ExternalInput")
    wk_d = nc.dram_tensor("wk", (H, H), FP8, kind="ExternalInput")
    wv_d = nc.dram_tensor("wv", (H, H), FP8, kind="ExternalInput")
    wo_d = nc.dram_tensor("wo", (H, H), FP8, kind="ExternalInput")
    w1h_d = nc.dram_tensor("w1h", (H, FF), FP8, kind="ExternalInput")
    w1l_d = nc.dram_tensor("w1l", (H, FF), FP8, kind="ExternalInput")
    w2h_d = nc.dram_tensor("w2h", (FF, H), FP8, kind="ExternalInput")
    w2l_d = nc.dram_tensor("w2l", (FF, H), FP8, kind="ExternalInput")
    out_d = nc.dram_tensor("out", (Sq, H), F32, kind="ExternalOutput")
    bq_d = nc.dram_tensor("bq_pc", (P, NFC), F32, kind="ExternalInput")
    bk_d = nc.dram_tensor("bk_pc", (P, NFC), F32, kind="ExternalInput")
    bv_d = nc.dram_tensor("bv", (H,), F32, kind="ExternalInput")
    bo_d = nc.dram_tensor("bo", (H,), F32, kind="ExternalInput")
    b1_d = nc.dram_tensor("b1_pc", (P, NFFC), F32, kind="ExternalInput")
    b2_d = nc.dram_tensor("b2", (H,), F32, kind="ExternalInput")
    ln1g_d = nc.dram_tensor("ln1_g", (H,), F32, kind="ExternalInput")
    ln1b_d = nc.dram_tensor("ln1_b", (H,), F32, kind="ExternalInput")
    ln2g_d = nc.dram_tensor("ln2_g", (H,), F32, kind="ExternalInput")
    ln2b_d = nc.dram_tensor("ln2_b", (H,), F32, kind="ExternalInput")
    lnfg_d = nc.dram_tensor("lnf_g", (H,), F32, kind="ExternalInput")
    lnfb_d = nc.dram_tensor("lnf_b", (H,), F32, kind="ExternalInput")

    # Bulk DMA queues: SP (hwdge) and gpsimd (swdge). The Act queue is kept
    # free for compute dispatch: every hwdge DMA costs ~630ns of issuing-queue
    # SEQ time, which starves exp dispatch during attention.
    q_iter = {"i": 0}

    def _dq():
        q_iter["i"] += 1
        return nc.sync if q_iter["i"] % 2 else nc.gpsimd

    # XBAR transposes must use a hwdge queue (SP/Act); they are few.
    t_iter = {"i": 0}

    def _tq():
        t_iter["i"] += 1
        return nc.sync if t_iter["i"] % 2 else nc.scalar

    def _drain(out, ps, bias):
        """PSUM -> SBUF fp8/f32 with the 1/WS weight rescale (+ bias)."""
        if with_biases and bias is not None:
            nc.vector.tensor_scalar(out, ps, IWS, bias, op0=OP.mult,
                                    op1=OP.add)
        else:
            nc.vector.tensor_scalar(out, ps, IWS, None, op0=OP.mult)

    def _layer_norm(pool, y_out, x_in, g_b, b_b, eps_t):
        """y = (x - mean)/sqrt(var+eps) [* g + b] on a [128, H] tile."""
        stats = pool.tile([P, 2, 6], F32, tag="ln_stats", name="stats")
        nc.vector.bn_stats(stats[:, 0, :], x_in[:, 0:512])
        nc.vector.bn_stats(stats[:, 1, :], x_in[:, 512:1024])
        mv = pool.tile([P, 2], F32, tag="ln_mv", name="mv")
        nc.vector.bn_aggr(mv, stats)
        std = pool.tile([P, 1], F32, tag="ln_std", name="std")
        nc.scalar.activation(std, mv[:, 1:2], AF.Sqrt, bias=eps_t, scale=1.0)
        rstd = pool.tile([P, 1], F32, tag="ln_rstd", name="rstd")
        nc.vector.reciprocal(rstd, std)
        nc.vector.tensor_scalar(
            y_out, x_in, mv[:, 0:1], rstd, op0=OP.subtract, op1=OP.mult
        )
        if ln_affine:
            nc.vector.tensor_mul(y_out, y_out, g_b)
            nc.vector.tensor_add(y_out, y_out, b_b)

    with tile.TileContext(nc) as tc, ExitStack() as top:
        persist = top.enter_context(tc.tile_pool(name="persist", bufs=1))
        # [I|0] and [0|I] fp8 stationary tiles for the bias preloads
        ipadE = persist.tile([P, 2, P], FP8, tag="ipadE")
        ipadO = persist.tile([P, 2, P], FP8, tag="ipadO")
        # qT: plane 0 = parity-padded q chunks, plane 1 = zeros (DR slot 1)
        qT = persist.tile([P, 2, NH, Sq], FP8, tag="qT")
        oT = persist.tile([P, NFC, Sq], FP8, tag="oT")
        esh_t = persist.tile([P, 1], F32, tag="esh")
        nc.vector.memset(esh_t, ESHIFT)

        def _setup_consts():
            # Emitted AFTER the input/weight DMA issues: the gpsimd SEQ runs
            # its queue in order, and these memsets must not delay the DMAs.
            nc.gpsimd.memset(ipadE, 0.0)
            make_identity(nc, ipadE[:, 0, :], nomemset=True)
            nc.gpsimd.memset(ipadO, 0.0)
            make_identity(nc, ipadO[:, 1, :], nomemset=True)
            nc.gpsimd.memset(qT[:, 1, :, :], 0.0)
            qT_ev = qT[:, 0, :, :].rearrange("p (hh t) q -> p hh t q", t=2)
            nc.gpsimd.memset(qT_ev[64:128, :, 0, :], 0.0)
            nc.gpsimd.memset(qT_ev[0:64, :, 1, :], 0.0)

        xp = top.enter_context(tc.tile_pool(name="xp", bufs=1))
        x_sb = xp.tile([P, NTC, H], F32, tag="x")       # attn residual out
        x2_sb = xp.tile([P, NTC, H], BF16, tag="x2")    # x2h kept resident

        # wo + the first w1 group preallocated up top so their DMAs overlap
        # the early phases / attention
        wlate = top.enter_context(tc.tile_pool(name="wlate", bufs=1))
        wo_sb = wlate.tile([P, NFC, H], FP8, tag="wo")
        w1h0_sb = wlate.tile([P, NFC, 1024], FP8, tag="w1h0")
        w1l0_sb = wlate.tile([P, NFC, 1024], FP8, tag="w1l0")

        with (
            tc.tile_pool(name="qkv", bufs=1) as qkvp,
            tc.tile_pool(name="bias_s", bufs=3) as bpool,
            tc.tile_pool(name="expp", bufs=4) as epool,
            tc.tile_pool(name="rin", bufs=2) as rpool,
        ):
            kT = qkvp.tile([P, NFC, S], FP8, tag="kT")
            v_aug = qkvp.tile([P, NKT, NH * 65], FP8, tag="vaug")

            # ---------- Phase 1+2: LN, XBAR transpose, QKV projections ------
            with (
                tc.tile_pool(name="xin", bufs=1) as xinp,
                tc.tile_pool(name="ybf", bufs=1) as ybf,
                tc.tile_pool(name="yT", bufs=1) as yTp,
                tc.tile_pool(name="ph1", bufs=4) as ph1,
                tc.tile_pool(name="ph1w", bufs=3) as ph1w,
                tc.tile_pool(name="ph1c", bufs=1) as ph1c,
                tc.tile_pool(name="wload", bufs=1) as wpool,
                tc.tile_pool(name="vecs", bufs=1) as vecs,
                tc.tile_pool(
                    name="ph2ps", bufs=4, space=bass.MemorySpace.PSUM
                ) as ps2,
            ):
                x1_sb = xinp.tile([P, NKT, H], BF16, tag="x1")
                # Inputs first, all on SP (they gate the LN ladders); weights
                # on the gpsimd queue so they transfer concurrently.
                for t in range(NTC):
                    nc.sync.dma_start(
                        x2_sb[:, t, :], x2h_d[t * P:(t + 1) * P, :]
                    )
                for t in range(NKT):
                    nc.sync.dma_start(
                        x1_sb[:, t, :], x1_d[t * P:(t + 1) * P, :]
                    )

                wq_sb = wpool.tile([P, NFC, H], FP8, tag="wq", name="wq_sb")
                nc.gpsimd.dma_start(
                    wq_sb, wq_d.rearrange("(kc p) f -> p kc f", p=P))
                wk_sb = wpool.tile([P, NFC, H], FP8, tag="wk", name="wk_sb")
                nc.sync.dma_start(
                    wk_sb, wk_d.rearrange("(kc p) f -> p kc f", p=P))
                wv_sb = wpool.tile([P, NFC, H], FP8, tag="wv", name="wv_sb")
                nc.gpsimd.dma_start(
                    wv_sb, wv_d.rearrange("(kc p) f -> p kc f", p=P))
                nc.sync.dma_start(
                    wo_sb, wo_d.rearrange("(kc p) f -> p kc f", p=P))
                nc.gpsimd.dma_start(
                    w1h0_sb, w1h_d[:, 0:1024].rearrange(
                        "(kc p) f -> p kc f", p=P))
                nc.sync.dma_start(
                    w1l0_sb, w1l_d[:, 0:1024].rearrange(
                        "(kc p) f -> p kc f", p=P))
                _setup_consts()
                ones_view = v_aug[:, :, :].rearrange(
                    "p t (h j) -> p t h j", j=65
                )[:, :, :, 64:65]
                nc.gpsimd.memset(ones_view, 1.0)

                y2T_bf = ybf.tile([P, NFC, Sq], BF16, tag="y2Tb")
                # y1T bf16 staging is halved and reused (tokens 0-511, then
                # 512-1023) to cut SBUF peak
                y1T_bf = ybf.tile([P, NFC, Sq], BF16, tag="y1Tb")
                y2T = yTp.tile([P, NFC, Sq], FP8, tag="y2T")
                y1T = yTp.tile([P, NFC, S], FP8, tag="y1T")

                eps_t = ph1c.tile([P, 1], F32, tag="eps")
                nc.vector.memset(eps_t, EPS)
                ln1g_b = ln1b_b = ln2g_b = ln2b_b = None
                if ln_affine:
                    ln1g_b = ph1c.tile([P, H], F32, tag="ln1g")
                    ln1b_b = ph1c.tile([P, H], F32, tag="ln1b")
                    ln2g_b = ph1c.tile([P, H], F32, tag="ln2g")
                    ln2b_b = ph1c.tile([P, H], F32, tag="ln2b")
                    nc.gpsimd.dma_start(ln1g_b, _pbcast(ln1g_d[:], P))
                    nc.gpsimd.dma_start(ln1b_b, _pbcast(ln1b_d[:], P))
                    nc.gpsimd.dma_start(ln2g_b, _pbcast(ln2g_d[:], P))
                    nc.gpsimd.dma_start(ln2b_b, _pbcast(ln2b_d[:], P))

                bq_sb = bk_sb = bv_b = None
                if with_biases:
                    bq_sb = vecs.tile([P, NFC], F32, tag="bq")
                    bk_sb = vecs.tile([P, NFC], F32, tag="bk")
                    bv_b = vecs.tile([P, H], F32, tag="bvb")
                    nc.gpsimd.dma_start(bq_sb, bq_d[:, :])
                    nc.gpsimd.dma_start(bk_sb, bk_d[:, :])
                    nc.gpsimd.dma_start(bv_b, _pbcast(bv_d[:], P))

                # x2h -> y2 -> y2T (XBAR) -> fp8
                for t in range(NTC):
                    yt = ph1w.tile([P, H], BF16, tag="yt", name="yt")
                    _layer_norm(ph1, yt, x2_sb[:, t, :], ln2g_b, ln2b_b, eps_t)
                    _tq().dma_start_transpose(
                        y2T_bf[:, :, t * P:(t + 1) * P], yt[:, :]
                    )
                nc.vector.tensor_copy(y2T, y2T_bf)

                # q^T per fo chunk; parity-split drains into padded qT
                for fo in range(NFC):
                    ps = ps2.tile([P, Sq], F32, tag="mm", name="ps")
                    for g in range(4):
                        nc.tensor.matmul(
                            ps,
                            wq_sb[:, 2 * g:2 * g + 2, fo * P:(fo + 1) * P],
                            y2T[:, 2 * g:2 * g + 2, :],
                            start=(g == 0), stop=(g == 3), perf_mode=DR,
                        )
                    h0, h1 = 2 * fo, 2 * fo + 1
                    nc.scalar.activation(
                        qT[0:64, 0, h0, :], ps[0:64, :], AF.Copy,
                        bias=(bq_sb[0:64, fo:fo + 1] if with_biases else 0.0),
                        scale=IWS,
                    )
                    nc.scalar.activation(
                        qT[64:128, 0, h1, :], ps[64:128, :], AF.Copy,
                        bias=(bq_sb[64:128, fo:fo + 1] if with_biases
                              else 0.0),
                        scale=IWS,
                    )

                # x1 -> y1 -> y1T (XBAR, halved staging) -> fp8
                for half in range(2):
                    for i in range(4):
                        t = 4 * half + i
                        yt = ph1w.tile([P, H], BF16, tag="yt", name="yt")
                        _layer_norm(ph1, yt, x1_sb[:, t, :],
                                    ln1g_b, ln1b_b, eps_t)
                        _tq().dma_start_transpose(
                            y1T_bf[:, :, i * P:(i + 1) * P], yt[:, :]
                        )
                    nc.vector.tensor_copy(
                        y1T[:, :, half * 512:(half + 1) * 512], y1T_bf
                    )

                # v and k projections, emitted in the order attention consumes
                # them (k fo=0,1 first so heads 0-3 can start, then v, then
                # the remaining k chunks). All drains on DVE: it idles during
                # the Act-bound attention phase and absorbs the stragglers.
                def _vproj(t, nt):
                    ps = ps2.tile([P, 512], F32, tag="mm", name="ps")
                    for g in range(4):
                        nc.tensor.matmul(
                            ps,
                            y1T[:, 2 * g:2 * g + 2, t * P:(t + 1) * P],
                            wv_sb[:, 2 * g:2 * g + 2,
                                  nt * 512:(nt + 1) * 512],
                            start=(g == 0), stop=(g == 3), perf_mode=DR,
                        )
                    dst = v_aug[
                        :, t, nt * 8 * 65:(nt * 8 + 8) * 65
                    ].rearrange("p (h j) -> p h j", j=65)[:, :, 0:64]
                    psr = ps.rearrange("p (h j) -> p h j", j=64)
                    if with_biases:
                        nc.vector.scalar_tensor_tensor(
                            out=dst, in0=psr, scalar=IWS,
                            in1=bv_b[
                                :, nt * 512:(nt + 1) * 512
                            ].rearrange("p (h j) -> p h j", j=64),
                            op0=OP.mult, op1=OP.add,
                        )
                    else:
                        nc.vector.tensor_scalar(
                            dst, psr, IWS, None, op0=OP.mult)

                def _kproj(fo, nt):
                    ps = ps2.tile([P, 512], F32, tag="mm", name="ps")
                    for g in range(4):
                        nc.tensor.matmul(
                            ps,
                            wk_sb[:, 2 * g:2 * g + 2, fo * P:(fo + 1) * P],
                            y1T[:, 2 * g:2 * g + 2,
                                nt * 512:(nt + 1) * 512],
                            start=(g == 0), stop=(g == 3), perf_mode=DR,
                        )
                    _drain(kT[:, fo, nt * 512:(nt + 1) * 512], ps,
                           bk_sb[:, fo:fo + 1] if with_biases else None)

                for fo in (0, 1):
                    for nt in range(2):
                        _kproj(fo, nt)
                for t in range(NKT):
                    for nt in range(2):
                        _vproj(t, nt)
                for fo in range(2, NFC):
                    for nt in range(2):
                        _kproj(fo, nt)

            # ---------------- Phase 3: attention ----------------
            # Per (head, kt): [I|.] preload puts bias^T in PSUM, one stride-0
            # slot-repeated DR matmul adds k_h.T @ q_h; exp from 2-bank PSUM
            # -> fp8 e_t; [v|1].T @ e_t accumulates o^T + rowsum.
            with (
                tc.tile_pool(
                    name="sc_ps", bufs=3, space=bass.MemorySpace.PSUM
                ) as scps,
                tc.tile_pool(
                    name="o_ps", bufs=2, space=bass.MemorySpace.PSUM
                ) as ops,
            ):
                for h in range(NH):
                    hp = (h % 2) * Dh
                    fc = h // 2
                    o_ps = ops.tile([65, Sq], F32, tag="o", name="o_ps")
                    bt = bpool.tile([P, NKT, Sq], FP8, tag="bt", name="bt")
                    _dq().dma_start(
                        bt, biasT_d[h].rearrange("(kt p) q -> p kt q", p=P)
                    )
                    for g in range(4):
                        scp = scps.tile([P, 2, Sq], F32, tag="sc", name="scp")
                        e_t = epool.tile([P, 2, Sq], FP8, tag="et", name="e_t")
                        for j in range(2):
                            kt = 2 * g + j
                            nc.tensor.matmul(
                                scp[:, j, :],
                                ipadE if j == 0 else ipadO,
                                bt[:, 2 * g:2 * g + 2, :],
                                start=True, stop=False, perf_mode=DR,
                            )
                            nc.tensor.matmul(
                                scp[:, j, :],
                                _srep(kT[:, fc, kt * P:(kt + 1) * P]),
                                qT[:, :, h, :],
                                start=False, stop=True, perf_mode=DR,
                            )
                        nc.scalar.activation(
                            e_t, scp, AF.Exp, bias=esh_t, scale=SCALE
                        )
                        nc.tensor.matmul(
                            o_ps,
                            v_aug[:, 2 * g:2 * g + 2, h * 65:(h + 1) * 65],
                            e_t,
                            start=(g == 0), stop=(g == 3), perf_mode=DR,
                        )
                    rinv = rpool.tile([1, Sq], F32, tag="rinv", name="rinv")
                    nc.vector.reciprocal(rinv, o_ps[64:65, :])
                    rb = rpool.tile([Dh, Sq], F32, tag="rb", name="rb")
                    nc.gpsimd.partition_broadcast(rb, rinv[0:1, :])
                    nc.vector.tensor_tensor(
                        out=oT[hp:hp + Dh, fc, :],
                        in0=o_ps[0:64, :], in1=rb,
                        op=OP.mult,
                    )

        # ---------------- Phase 4: output projection + residual -------------
        with (
            tc.tile_pool(name="ph4c", bufs=1) as ph4c,
            tc.tile_pool(name="ph4ps", bufs=3, space=bass.MemorySpace.PSUM) as ps4,
        ):
            bo_b = None
            if with_biases:
                bo_b = ph4c.tile([P, H], F32, tag="bob")
                nc.gpsimd.dma_start(bo_b, _pbcast(bo_d[:], P))
            for t in range(NTC):
                for half in range(2):
                    ps = ps4.tile([P, 512], F32, tag="mm", name="ps")
                    for g in range(4):
                        nc.tensor.matmul(
                            ps,
                            oT[:, 2 * g:2 * g + 2, t * P:(t + 1) * P],
                            wo_sb[:, 2 * g:2 * g + 2,
                                  half * 512:(half + 1) * 512],
                            start=(g == 0), stop=(g == 3), perf_mode=DR,
                        )
                    xs = x_sb[:, t, half * 512:(half + 1) * 512]
                    nc.vector.scalar_tensor_tensor(
                        out=xs, in0=ps, scalar=IWS,
                        in1=x2_sb[:, t, half * 512:(half + 1) * 512],
                        op0=OP.mult, op1=OP.add,
                    )
                    if with_biases:
                        nc.vector.tensor_add(
                            xs, xs, bo_b[:, half * 512:(half + 1) * 512]
                        )

        # ---------------- Phase 5+6+7: final LN + FFN ----------------
        # FFN precision: weights and activations both carried as fp8 hi+lo
        # planes; each matmul computes hi*hi + lo*hi + hi*lo (the lo*lo term
        # is negligible) -> bf16-class FFN accuracy at fp8-DR speed.
        with (
            tc.tile_pool(name="hTp", bufs=1) as hTp,
            tc.tile_pool(name="y3", bufs=1) as y3p,
        ):
            hT = hTp.tile([P, NFFC, 2, Sq], FP8, tag="hT")   # planes hi/lo
            y3T = y3p.tile([P, NFC, 2, Sq], FP8, tag="y3T")  # planes hi/lo

            with (
                tc.tile_pool(name="ph5", bufs=4) as ph5,
                tc.tile_pool(name="ph5w", bufs=2) as ph5w,
                tc.tile_pool(name="ph5b", bufs=1) as ph5b,
                tc.tile_pool(name="ph5c", bufs=1) as ph5c,
            ):
                y3T_bf = ph5b.tile([P, NFC, Sq], BF16, tag="y3Tb")
                eps_t = ph5c.tile([P, 1], F32, tag="eps")
                nc.vector.memset(eps_t, EPS)
                lnfg_b = lnfb_b = None
                if ln_affine:
                    lnfg_b = ph5c.tile([P, H], F32, tag="lnfg")
                    lnfb_b = ph5c.tile([P, H], F32, tag="lnfb")
                    nc.gpsimd.dma_start(lnfg_b, _pbcast(lnfg_d[:], P))
                    nc.gpsimd.dma_start(lnfb_b, _pbcast(lnfb_d[:], P))
                for t in range(NTC):
                    yt = ph5w.tile([P, H], BF16, tag="yt", name="yt")
                    _layer_norm(ph5, yt, x_sb[:, t, :], lnfg_b, lnfb_b, eps_t)
                    _tq().dma_start_transpose(
                        y3T_bf[:, :, t * P:(t + 1) * P], yt[:, :]
                    )
                nc.vector.tensor_copy(y3T[:, :, 0, :], y3T_bf)
                nc.vector.tensor_tensor(
                    out=y3T[:, :, 1, :], in0=y3T_bf, in1=y3T[:, :, 0, :],
                    op=OP.subtract,
                )

            # FFN1 + gelu -> dual-plane hT, then FFN2 in ONE 8-bank pass
            with (
                tc.tile_pool(name="b1l", bufs=1) as b1pool,
                tc.tile_pool(name="w1s", bufs=2) as w1sp,
                tc.tile_pool(name="w2s", bufs=2) as w2sp,
                tc.tile_pool(name="h32", bufs=3) as h32p,
                tc.tile_pool(name="outp", bufs=2) as outp,
            ):
                b1_sb = b2_b = None
                if with_biases:
                    b1_sb = b1pool.tile([P, NFFC], F32, tag="b1")
                    nc.gpsimd.dma_start(b1_sb, b1_d[:, :])
                    b2_b = b1pool.tile([P, H], F32, tag="b2b")
                    nc.gpsimd.dma_start(b2_b, _pbcast(b2_d[:], P))

                # FFN1 streamed in 4 groups of 8 ffc chunks (group 0 was
                # prefetched into wlate during the early phases)
                GW = 1024  # ff columns per weight group
                f1ctx = tc.tile_pool(name="f1ps", bufs=2,
                                     space=bass.MemorySpace.PSUM)
                f1ps = f1ctx.__enter__()
                for gi in range(FF // GW):
                    if gi == 0:
                        w1h, w1l = w1h0_sb, w1l0_sb
                    else:
                        w1h = w1sp.tile([P, NFC, GW], FP8, tag="w1h",
                                        name="w1h")
                        w1l = w1sp.tile([P, NFC, GW], FP8, tag="w1l",
                                        name="w1l")
                        _dq().dma_start(
                            w1h, w1h_d[:, gi * GW:(gi + 1) * GW].rearrange(
                                "(kc p) f -> p kc f", p=P))
                        _dq().dma_start(
                            w1l, w1l_d[:, gi * GW:(gi + 1) * GW].rearrange(
                                "(kc p) f -> p kc f", p=P))
                    for fp_ in range(4):
                        ps = f1ps.tile([P, 2, Sq], F32, tag="mm", name="ps")
                        for i in range(2):
                            lo = (2 * fp_ + i) * P  # local 128-col block
                            for kc in range(NFC):
                                nc.tensor.matmul(
                                    ps[:, i, :],
                                    _srep(w1h[:, kc, lo:lo + P]),
                                    y3T[:, kc, :, :],
                                    start=(kc == 0), stop=False, perf_mode=DR,
                                )
                            for g in range(4):
                                nc.tensor.matmul(
                                    ps[:, i, :],
                                    w1l[:, 2 * g:2 * g + 2, lo:lo + P],
                                    y3T[:, 2 * g:2 * g + 2, 0, :],
                                    start=False, stop=(g == 3), perf_mode=DR,
                                )
                        h32 = h32p.tile([P, 2, Sq], F32, tag="h32",
                                        name="h32")
                        ffc0 = 8 * gi + 2 * fp_
                        if with_biases:
                            for i in range(2):
                                nc.scalar.activation(
                                    h32[:, i, :], ps[:, i, :], AF.Gelu,
                                    bias=b1_sb[:, ffc0 + i:ffc0 + i + 1],
                                    scale=IWS,
                                )
                        else:
                            nc.scalar.activation(h32, ps, AF.Gelu, scale=IWS)
                        nc.vector.tensor_copy(
                            hT[:, ffc0:ffc0 + 2, 0, :], h32)
                        nc.vector.tensor_tensor(
                            out=hT[:, ffc0:ffc0 + 2, 1, :], in0=h32,
                            in1=hT[:, ffc0:ffc0 + 2, 0, :], op=OP.subtract,
                        )

                # prefetch w2 group 0 while FFN1 still runs
                w2tiles = []
                for gi in range(4):
                    w2h = w2sp.tile([P, 8, H], FP8, tag="w2h", name="w2h")
                    w2l = w2sp.tile([P, 8, H], FP8, tag="w2l", name="w2l")
                    _dq().dma_start(
                        w2h, w2h_d[gi * GW:(gi + 1) * GW, :].rearrange(
                            "(c p) f -> p c f", p=P))
                    _dq().dma_start(
                        w2l, w2l_d[gi * GW:(gi + 1) * GW, :].rearrange(
                            "(c p) f -> p c f", p=P))
                    w2tiles.append((w2h, w2l))

                f1ctx.__exit__(None, None, None)

                # FFN2: single pass over all 4 token tiles (8 PSUM banks);
                # w2 planes streamed once in 4 groups of 8 ff chunks
                with tc.tile_pool(
                    name="f2ps", bufs=1, space=bass.MemorySpace.PSUM
                ) as f2ps:
                    accs = [
                        f2ps.tile([P, H], F32, tag=f"acc{t}",
                                  name=f"acc{t}")
                        for t in range(NTC)
                    ]
                    for gi in range(4):
                        w2h, w2l = w2tiles[gi]
                        for t in range(NTC):
                            for c in range(8):
                                ffc = 8 * gi + c
                                for nt in range(2):
                                    nc.tensor.matmul(
                                        accs[t][:, nt * 512:(nt + 1) * 512],
                                        hT[:, ffc, :, t * P:(t + 1) * P],
                                        _srep(w2h[:, c,
                                                  nt * 512:(nt + 1) * 512]),
                                        start=(ffc == 0), stop=False,
                                        perf_mode=DR,
                                    )
                            for c2 in range(4):
                                ffp = 8 * gi + 2 * c2
                                for nt in range(2):
                                    nc.tensor.matmul(
                                        accs[t][:, nt * 512:(nt + 1) * 512],
                                        hT[:, ffp:ffp + 2, 0,
                                           t * P:(t + 1) * P],
                                        w2l[:, 2 * c2:2 * c2 + 2,
                                            nt * 512:(nt + 1) * 512],
                                        start=False,
                                        stop=(gi == 3 and c2 == 3),
                                        perf_mode=DR,
                                    )
                    for t in range(NTC):
                        ot = outp.tile([P, H], F32, tag="ot", name="ot")
                        nc.vector.scalar_tensor_tensor(
                            out=ot, in0=accs[t], scalar=IWS,
                            in1=x_sb[:, t, :], op0=OP.mult, op1=OP.add,
                        )
                        if with_biases:
                            nc.vector.tensor_add(ot, ot, b2_b)
                        _dq().dma_start(out_d[t * P:(t + 1) * P, :], ot)

    nc.compile()
    return nc


_CACHE: dict = {}


def _get_program(ln_affine=True, with_biases=True):
    key = (ln_affine, with_biases)
    if key not in _CACHE:
        _CACHE[key] = build_program(
            ln_affine=ln_affine, with_biases=with_biases
        )
    return _CACHE[key]


def _detect_fast_flags(inputs):
    ones = lambda k: bool(np.all(np.asarray(inputs[k]) == 1.0))
    zeros = lambda k: bool(np.all(np.asarray(inputs[k]) == 0.0))
    ln_affine = not (
        ones("ln1_g") and ones("ln2_g") and ones("lnf_g")
        and zeros("ln1_b") and zeros("ln2_b") and zeros("lnf_b")
    )
    with_biases = not (
        zeros("bq") and zeros("bk") and zeros("bv") and zeros("bo")
        and zeros("b1") and zeros("b2")
    )
    return ln_affine, with_biases


def _make_in_maps(inputs: dict) -> list[dict]:
    import ml_dtypes

    fp8 = ml_dtypes.float8_e4m3
    bf16 = ml_dtypes.bfloat16
    f32 = lambda a: np.ascontiguousarray(np.asarray(a, dtype=np.float32))
    w8 = lambda a: np.ascontiguousarray(
        (np.asarray(a, dtype=np.float32) * WS).astype(fp8)
    )

    def w8planes(a):
        ws = np.asarray(a, dtype=np.float32) * WS
        hi = ws.astype(fp8)
        lo = (ws - hi.astype(np.float32)).astype(fp8)
        return np.ascontiguousarray(hi), np.ascontiguousarray(lo)

    x1 = np.asarray(inputs["x1"], dtype=np.float32)
    x2 = np.asarray(inputs["x2"], dtype=np.float32)
    attn_bias = np.asarray(inputs["attn_bias"], dtype=np.float32)
    w1h, w1l = w8planes(inputs["w1"])
    w2h, w2l = w8planes(inputs["w2"])
    shared = {
        "wq": w8(inputs["wq"]),
        "wk": w8(inputs["wk"]),
        "wv": w8(inputs["wv"]),
        "wo": w8(inputs["wo"]),
        "w1h": w1h, "w1l": w1l,
        "w2h": w2h, "w2l": w2l,
        "bq_pc": f32(np.asarray(inputs["bq"]).reshape(NFC, P).T),
        "bk_pc": f32(np.asarray(inputs["bk"]).reshape(NFC, P).T),
        "bv": f32(inputs["bv"]),
        "bo": f32(inputs["bo"]),
        "b1_pc": f32(np.asarray(inputs["b1"]).reshape(NFFC, P).T),
        "b2": f32(inputs["b2"]),
        "ln1_g": f32(inputs["ln1_g"]),
        "ln1_b": f32(inputs["ln1_b"]),
        "ln2_g": f32(inputs["ln2_g"]),
        "ln2_b": f32(inputs["ln2_b"]),
        "lnf_g": f32(inputs["lnf_g"]),
        "lnf_b": f32(inputs["lnf_b"]),
    }
    in_maps = []
    for c in range(8):
        b, half = c // 2, c % 2
        q0 = half * Sq
        in_maps.append(
            {
                "x1": np.ascontiguousarray(x1[b].astype(bf16)),
                "x2h": np.ascontiguousarray(x2[b, q0:q0 + Sq].astype(bf16)),
                "biasT": np.ascontiguousarray(
                    (attn_bias[b, :, q0:q0 + Sq, :].transpose(0, 2, 1)
                     * (1.0 / SCALE)).astype(fp8)
                ),
                **shared,
            }
        )
    return in_maps


def _assemble(results: list[dict]) -> np.ndarray:
    out = np.empty((B, S, H), np.float32)
    for c in range(8):
        b, half = c // 2, c % 2
        out[b, half * Sq:(half + 1) * Sq] = results[c]["out"]
    return out


def run(inputs: dict, **run_kwargs):
    from concourse.bass_utils import run_bass_kernel_spmd

    ln_affine, with_biases = _detect_fast_flags(inputs)
    nc = _get_program(ln_affine=ln_affine, with_biases=with_biases)
    in_maps = _make_in_maps(inputs)
    res = run_bass_kernel_spmd(nc, in_maps, core_ids=list(range(8)), **run_kwargs)
    return _assemble(res.results), res


def kernel(**inputs) -> np.ndarray:
    out, _ = run(inputs)
    return out


# revision 38
# speedup vs baseline: 1.4759x; 1.0724x over previous
"""CrossTransformerLayer on 8 TRN2 NeuronCores — fp8 DoubleRow edition.

Sharding: core c -> (batch b = c//2, q-half = c%2). Each core computes its
512 query rows of its batch end-to-end (k/v over the full 1024-token x1
sequence); no cross-core collectives.

Key device-side ideas (validated on-device in minitest.py):
  * Every large GEMM runs as fp8(e4m3) DoubleRow matmuls: 2x128 contraction
    per instruction at 0.5 cycles/row -> 4x the bf16 PE throughput. Weights
    are host-scaled by 32 (fp8 precision) and rescaled by 1/32 in the
    PSUM->SBUF drains.
  * Attention scores^T[k,q] contract only d=64 per head, too shallow for a
    DoubleRow pair. Instead: qT chunks are parity-padded with zeros (head h
    occupies partitions (h%2)*64..+64, the sibling half is zero), the packed
    kT chunk is slot-repeated with a stride-0 AP, and rhs slot 1 points at an
    all-zero qT plane -> one 256-cycle DR matmul per (head, kt) tile.
  * The attention bias lands in PSUM via fp8 DR "identity preload": lhsT
    [I|0] / [0|I], rhs = a pair of bias^T k-tiles -> 256 cycles per tile.
  * exp(scale*x - 3) on Act engine straight from 2-bank PSUM into fp8 e_t
    (the -3 shift keeps e^x inside e4m3 range; it cancels in the rowsum
    normalization). [v|1] rows are fp8, so the o-matmul is DR as well.
  * All y-transposes go through the DMA XBAR (dma_start_transpose, bf16,
    SP/Act queues) instead of PE+DVE; cheap SBUF->SBUF copies on the gpsimd
    engine convert bf16 y^T -> fp8 for the DR matmuls.
  * PSUM drains are DVE-only (gpsimd has no PSUM port); gpsimd takes the
    SBUF-side work (converts, memsets, rowsum broadcast); Act owns exp/gelu;
    bulk DMA alternates between the SP and Act queues (w2 on the gpsimd
    queue), which all transfer concurrently.

Numerics: x1/x2 in bf16; LN, softmax logits, residuals and the output stay
fp32; fp8 only on matmul operands (y^T, q^T, k^T, v, e^p, weights, bias^T).
"""

import sys

sys.path.insert(0, "/opt/trn_rl_repo")

from contextlib import ExitStack

import numpy as np

import concourse.bass as bass
import concourse.tile as tile
from concourse import bacc, mybir
from concourse.masks import make_identity

F32 = mybir.dt.float32
BF16 = mybir.dt.bfloat16
FP8 = mybir.dt.float8e4
DR = mybir.MatmulPerfMode.DoubleRow

B = 4
S = 1024   # full (k) sequence
Sq = 512   # query rows per core
H = 1024
NH = 16
Dh = 64    # head dim
FF = 4096
P = 128
NKT = S // P    # 8 k-token tiles
NFC = H // P    # 8 feature chunks
NTC = Sq // P   # 4 q-token tiles
NFFC = FF // P  # 32 ff chunks
EPS = 1e-5
SCALE = float(Dh) ** -0.5
WS = 32.0       # host-side fp8 weight scale
IWS = 1.0 / WS
ESHIFT = -3.0   # exp bias shift; cancels in the rowsum normalization
AF = mybir.ActivationFunctionType
OP = mybir.AluOpType


def _pbcast(ap: bass.AP, parts: int) -> bass.AP:
    """[.., N] access pattern -> [parts, .., N] with partition step 0."""
    return bass.AP(
        tensor=ap.tensor,
        offset=ap.offset,
        ap=[[0, parts]] + [list(d) for d in ap.ap],
    )


def _srep(ap: bass.AP, n: int = 2) -> bass.AP:
    """[p, F] AP -> [p, n, F] with slot stride 0 (repeat the same block)."""
    return bass.AP(
        tensor=ap.tensor,
        offset=ap.offset,
        ap=[list(ap.ap[0])] + [[0, n]] + [list(d) for d in ap.ap[1:]],
    )


def build_program(ln_affine=True, with_biases=True):
    nc = bacc.Bacc("TRN2", target_bir_lowering=False, debug=False)

    x1_d = nc.dram_tensor("x1", (S, H), BF16, kind="ExternalInput")
    x2h_d = nc.dram_tensor("x2h", (Sq, H), BF16, kind="ExternalInput")
    biasT_d = nc.dram_tensor("biasT", (NH, S, Sq), FP8, kind="ExternalInput")
    wq_d = nc.dram_tensor("wq", (H, H), FP8, kind="ExternalInput")
    wk_d = nc.dram_tensor("wk", (H, H), FP8, kind="ExternalInput")
    wv_d = nc.dram_tensor("wv", (H, H), FP8, kind="ExternalInput")
    wo_d = nc.dram_tensor("wo", (H, H), FP8, kind="ExternalInput")
    w1h_d = nc.dram_tensor("w1h", (H, FF), FP8, kind="ExternalInput")
    w1l_d = nc.dram_tensor("w1l", (H, FF), FP8, kind="ExternalInput")
    w2h_d = nc.dram_tensor("w2h", (FF, H), FP8, kind="ExternalInput")
    w2l_d = nc.dram_tensor("w2l", (FF, H), FP8, kind="ExternalInput")
    out_d = nc.dram_tensor("out", (Sq, H), F32, kind="ExternalOutput")
    bq_d = nc.dram_tensor("bq_pc", (P, NFC), F32, kind="ExternalInput")
    bk_d = nc.dram_tensor("bk_pc", (P, NFC), F32, kind="ExternalInput")
    bv_d = nc.dram_tensor("bv", (H,), F32, kind="ExternalInput")
    bo_d = nc.dram_tensor("bo", (H,), F32, kind="ExternalInput")
    b1_d = nc.dram_tensor("b1_pc", (P, NFFC), F32, kind="ExternalInput")
    b2_d = nc.dram_tensor("b2", (H,), F32, kind="ExternalInput")
    ln1g_d = nc.dram_tensor("ln1_g", (H,), F32, kind="ExternalInput")
    ln1b_d = nc.dram_tensor("ln1_b", (H,), F32, kind="ExternalInput")
    ln2g_d = nc.dram_tensor("ln2_g", (H,), F32, kind="ExternalInput")
    ln2b_d = nc.dram_tensor("ln2_b", (H,), F32, kind="ExternalInput")
    lnfg_d = nc.dram_tensor("lnf_g", (H,), F32, kind="ExternalInput")
    lnfb_d = nc.dram_tensor("lnf_b", (H,), F32, kind="ExternalInput")

    # Bulk DMA queues: SP (hwdge) and gpsimd (swdge). The Act queue is kept
    # free for compute dispatch: every hwdge DMA costs ~630ns of issuing-queue
    # SEQ time, which starves exp dispatch during attention.
    q_iter = {"i": 0}

    def _dq():
        q_iter["i"] += 1
        return nc.sync if q_iter["i"] % 2 else nc.gpsimd

    # XBAR transposes must use a hwdge queue (SP/Act); they are few.
    t_iter = {"i": 0}

    def _tq():
        t_iter["i"] += 1
        return nc.sync if t_iter["i"] % 2 else nc.scalar

    def _drain(out, ps, bias):
        """PSUM -> SBUF fp8/f32 with the 1/WS weight rescale (+ bias)."""
        if with_biases and bias is not None:
            nc.vector.tensor_scalar(out, ps, IWS, bias, op0=OP.mult,
                                    op1=OP.add)
        else:
            nc.vector.tensor_scalar(out, ps, IWS, None, op0=OP.mult)

    def _layer_norm(pool, y_out, x_in, g_b, b_b, eps_t):
        """y = (x - mean)/sqrt(var+eps) [* g + b] on a [128, H] tile."""
        stats = pool.tile([P, 2, 6], F32, tag="ln_stats", name="stats")
        nc.vector.bn_stats(stats[:, 0, :], x_in[:, 0:512])
        nc.vector.bn_stats(stats[:, 1, :], x_in[:, 512:1024])
        mv = pool.tile([P, 2], F32, tag="ln_mv", name="mv")
        nc.vector.bn_aggr(mv, stats)
        std = pool.tile([P, 1], F32, tag="ln_std", name="std")
        nc.scalar.activation(std, mv[:, 1:2], AF.Sqrt, bias=eps_t, scale=1.0)
        rstd = pool.tile([P, 1], F32, tag="ln_rstd", name="rstd")
        nc.vector.reciprocal(rstd, std)
        nc.vector.tensor_scalar(
            y_out, x_in, mv[:, 0:1], rstd, op0=OP.subtract, op1=OP.mult
        )
        if ln_affine:
            nc.vector.tensor_mul(y_out, y_out, g_b)
            nc.vector.tensor_add(y_out, y_out, b_b)

    with tile.TileContext(nc) as tc, ExitStack() as top:
        persist = top.enter_context(tc.tile_pool(name="persist", bufs=1))
        # [I|0] and [0|I] fp8 stationary tiles for the bias preloads
        ipadE = persist.tile([P, 2, P], FP8, tag="ipadE")
        ipadO = persist.tile([P, 2, P], FP8, tag="ipadO")
        # qT: plane 0 = parity-padded q chunks, plane 1 = zeros (DR slot 1)
        qT = persist.tile([P, 2, NH, Sq], FP8, tag="qT")
        oT = persist.tile([P, NFC, Sq], FP8, tag="oT")
        esh_t = persist.tile([P, 1], F32, tag="esh")
        nc.vector.memset(esh_t, ESHIFT)

        def _setup_consts():
            # Emitted AFTER the input/weight DMA issues: the gpsimd SEQ runs
            # its queue in order, and these memsets must not delay the DMAs.
            nc.gpsimd.memset(ipadE, 0.0)
            make_identity(nc, ipadE[:, 0, :], nomemset=True)
            nc.gpsimd.memset(ipadO, 0.0)
            make_identity(nc, ipadO[:, 1, :], nomemset=True)
            nc.gpsimd.memset(qT[:, 1, :, :], 0.0)
            qT_ev = qT[:, 0, :, :].rearrange("p (hh t) q -> p hh t q", t=2)
            nc.gpsimd.memset(qT_ev[64:128, :, 0, :], 0.0)
            nc.gpsimd.memset(qT_ev[0:64, :, 1, :], 0.0)

        xp = top.enter_context(tc.tile_pool(name="xp", bufs=1))
        x_sb = xp.tile([P, NTC, H], BF16, tag="x")      # attn residual out
        x2_sb = xp.tile([P, NTC, H], BF16, tag="x2")    # x2h kept resident

        # wo + the first w1 group preallocated up top so their DMAs overlap
        # the early phases / attention
        wlate = top.enter_context(tc.tile_pool(name="wlate", bufs=1))
        wo_sb = wlate.tile([P, NFC, H], FP8, tag="wo")
        w1h0_sb = wlate.tile([P, NFC, 1024], FP8, tag="w1h0")
        w1l0_sb = wlate.tile([P, NFC, 1024], FP8, tag="w1l0")

        with (
            tc.tile_pool(name="qkv", bufs=1) as qkvp,
            tc.tile_pool(name="bias_s", bufs=3) as bpool,
            tc.tile_pool(name="expp", bufs=4) as epool,
            tc.tile_pool(name="rin", bufs=2) as rpool,
        ):
            kT = qkvp.tile([P, NFC, S], FP8, tag="kT")
            v_aug = qkvp.tile([P, NKT, NH * 65], FP8, tag="vaug")

            # ---------- Phase 1+2: LN, XBAR transpose, QKV projections ------
            with (
                tc.tile_pool(name="xin", bufs=1) as xinp,
                tc.tile_pool(name="ybf", bufs=1) as ybf,
                tc.tile_pool(name="yT", bufs=1) as yTp,
                tc.tile_pool(name="ph1", bufs=4) as ph1,
                tc.tile_pool(name="ph1w", bufs=3) as ph1w,
                tc.tile_pool(name="ph1c", bufs=1) as ph1c,
                tc.tile_pool(name="wload", bufs=1) as wpool,
                tc.tile_pool(name="vecs", bufs=1) as vecs,
                tc.tile_pool(
                    name="ph2ps", bufs=4, space=bass.MemorySpace.PSUM
                ) as ps2,
            ):
                x1_sb = xinp.tile([P, NKT, H], BF16, tag="x1")
                # Inputs first, all on SP (they gate the LN ladders); weights
                # on the gpsimd queue so they transfer concurrently.
                for t in range(NTC):
                    nc.sync.dma_start(
                        x2_sb[:, t, :], x2h_d[t * P:(t + 1) * P, :]
                    )
                for t in range(NKT):
                    nc.sync.dma_start(
                        x1_sb[:, t, :], x1_d[t * P:(t + 1) * P, :]
                    )

                wq_sb = wpool.tile([P, NFC, H], FP8, tag="wq", name="wq_sb")
                nc.gpsimd.dma_start(
                    wq_sb, wq_d.rearrange("(kc p) f -> p kc f", p=P))
                wk_sb = wpool.tile([P, NFC, H], FP8, tag="wk", name="wk_sb")
                nc.sync.dma_start(
                    wk_sb, wk_d.rearrange("(kc p) f -> p kc f", p=P))
                wv_sb = wpool.tile([P, NFC, H], FP8, tag="wv", name="wv_sb")
                nc.gpsimd.dma_start(
                    wv_sb, wv_d.rearrange("(kc p) f -> p kc f", p=P))
                nc.sync.dma_start(
                    wo_sb, wo_d.rearrange("(kc p) f -> p kc f", p=P))
                nc.gpsimd.dma_start(
                    w1h0_sb, w1h_d[:, 0:1024].rearrange(
                        "(kc p) f -> p kc f", p=P))
                nc.sync.dma_start(
                    w1l0_sb, w1l_d[:, 0:1024].rearrange(
                        "(kc p) f -> p kc f", p=P))
                _setup_consts()
                ones_view = v_aug[:, :, :].rearrange(
                    "p t (h j) -> p t h j", j=65
                )[:, :, :, 64:65]
                nc.gpsimd.memset(ones_view, 1.0)

                y2T_bf = ybf.tile([P, NFC, Sq], BF16, tag="y2Tb")
                # y1T bf16 staging is halved and reused (tokens 0-511, then
                # 512-1023) to cut SBUF peak
                y1T_bf = ybf.tile([P, NFC, Sq], BF16, tag="y1Tb")
                y2T = yTp.tile([P, NFC, Sq], FP8, tag="y2T")
                y1T = yTp.tile([P, NFC, S], FP8, tag="y1T")

                eps_t = ph1c.tile([P, 1], F32, tag="eps")
                nc.vector.memset(eps_t, EPS)
                ln1g_b = ln1b_b = ln2g_b = ln2b_b = None
                if ln_affine:
                    ln1g_b = ph1c.tile([P, H], F32, tag="ln1g")
                    ln1b_b = ph1c.tile([P, H], F32, tag="ln1b")
                    ln2g_b = ph1c.tile([P, H], F32, tag="ln2g")
                    ln2b_b = ph1c.tile([P, H], F32, tag="ln2b")
                    nc.gpsimd.dma_start(ln1g_b, _pbcast(ln1g_d[:], P))
                    nc.gpsimd.dma_start(ln1b_b, _pbcast(ln1b_d[:], P))
                    nc.gpsimd.dma_start(ln2g_b, _pbcast(ln2g_d[:], P))
                    nc.gpsimd.dma_start(ln2b_b, _pbcast(ln2b_d[:], P))

                bq_sb = bk_sb = bv_b = None
                if with_biases:
                    bq_sb = vecs.tile([P, NFC], F32, tag="bq")
                    bk_sb = vecs.tile([P, NFC], F32, tag="bk")
                    bv_b = vecs.tile([P, H], F32, tag="bvb")
                    nc.gpsimd.dma_start(bq_sb, bq_d[:, :])
                    nc.gpsimd.dma_start(bk_sb, bk_d[:, :])
                    nc.gpsimd.dma_start(bv_b, _pbcast(bv_d[:], P))

                # x2h -> y2 -> y2T (XBAR) -> fp8
                for t in range(NTC):
                    yt = ph1w.tile([P, H], BF16, tag="yt", name="yt")
                    _layer_norm(ph1, yt, x2_sb[:, t, :], ln2g_b, ln2b_b, eps_t)
                    _tq().dma_start_transpose(
                        y2T_bf[:, :, t * P:(t + 1) * P], yt[:, :]
                    )
                nc.vector.tensor_copy(y2T, y2T_bf)

                # q^T per fo chunk; parity-split drains into padded qT
                for fo in range(NFC):
                    ps = ps2.tile([P, Sq], F32, tag="mm", name="ps")
                    for g in range(4):
                        nc.tensor.matmul(
                            ps,
                            wq_sb[:, 2 * g:2 * g + 2, fo * P:(fo + 1) * P],
                            y2T[:, 2 * g:2 * g + 2, :],
                            start=(g == 0), stop=(g == 3), perf_mode=DR,
                        )
                    h0, h1 = 2 * fo, 2 * fo + 1
                    nc.scalar.activation(
                        qT[0:64, 0, h0, :], ps[0:64, :], AF.Copy,
                        bias=(bq_sb[0:64, fo:fo + 1] if with_biases else 0.0),
                        scale=IWS,
                    )
                    nc.scalar.activation(
                        qT[64:128, 0, h1, :], ps[64:128, :], AF.Copy,
                        bias=(bq_sb[64:128, fo:fo + 1] if with_biases
                              else 0.0),
                        scale=IWS,
                    )

                # x1 -> y1 -> y1T (XBAR, halved staging) -> fp8
                for half in range(2):
                    for i in range(4):
                        t = 4 * half + i
                        yt = ph1w.tile([P, H], BF16, tag="yt", name="yt")
                        _layer_norm(ph1, yt, x1_sb[:, t, :],
                                    ln1g_b, ln1b_b, eps_t)
                        _tq().dma_start_transpose(
                            y1T_bf[:, :, i * P:(i + 1) * P], yt[:, :]
                        )
                    nc.vector.tensor_copy(
                        y1T[:, :, half * 512:(half + 1) * 512], y1T_bf
                    )

                # v and k projections, emitted in the order attention consumes
                # them (k fo=0,1 first so heads 0-3 can start, then v, then
                # the remaining k chunks). All drains on DVE: it idles during
                # the Act-bound attention phase and absorbs the stragglers.
                def _vproj(t, nt, act):
                    ps = ps2.tile([P, 512], F32, tag="mm", name="ps")
                    for g in range(4):
                        nc.tensor.matmul(
                            ps,
                            y1T[:, 2 * g:2 * g + 2, t * P:(t + 1) * P],
                            wv_sb[:, 2 * g:2 * g + 2,
                                  nt * 512:(nt + 1) * 512],
                            start=(g == 0), stop=(g == 3), perf_mode=DR,
                        )
                    dst = v_aug[
                        :, t, nt * 8 * 65:(nt * 8 + 8) * 65
                    ].rearrange("p (h j) -> p h j", j=65)[:, :, 0:64]
                    psr = ps.rearrange("p (h j) -> p h j", j=64)
                    if with_biases:
                        nc.vector.scalar_tensor_tensor(
                            out=dst, in0=psr, scalar=IWS,
                            in1=bv_b[
                                :, nt * 512:(nt + 1) * 512
                            ].rearrange("p (h j) -> p h j", j=64),
                            op0=OP.mult, op1=OP.add,
                        )
                    elif act:
                        nc.scalar.activation(dst, psr, AF.Copy, scale=IWS)
                    else:
                        nc.vector.tensor_scalar(
                            dst, psr, IWS, None, op0=OP.mult)

                def _kproj(fo, nt, act):
                    ps = ps2.tile([P, 512], F32, tag="mm", name="ps")
                    for g in range(4):
                        nc.tensor.matmul(
                            ps,
                            wk_sb[:, 2 * g:2 * g + 2, fo * P:(fo + 1) * P],
                            y1T[:, 2 * g:2 * g + 2,
                                nt * 512:(nt + 1) * 512],
                            start=(g == 0), stop=(g == 3), perf_mode=DR,
                        )
                    if act:
                        nc.scalar.activation(
                            kT[:, fo, nt * 512:(nt + 1) * 512], ps, AF.Copy,
                            bias=(bk_sb[:, fo:fo + 1] if with_biases
                                  else 0.0),
                            scale=IWS,
                        )
                    else:
                        _drain(kT[:, fo, nt * 512:(nt + 1) * 512], ps,
                               bk_sb[:, fo:fo + 1] if with_biases else None)

                # drains split by attention consumer order: the early chunks
                # (k fo0-3, v nt0) drain on Act right ahead of its exps; the
                # late chunks drain on DVE, overlapping the early exps.
                for fo in (0, 1):
                    for nt in range(2):
                        _kproj(fo, nt, act=True)
                for t in range(NKT):
                    _vproj(t, 0, act=True)
                for fo in (2, 3):
                    for nt in range(2):
                        _kproj(fo, nt, act=True)
                for t in range(NKT):
                    _vproj(t, 1, act=False)
                for fo in range(4, NFC):
                    for nt in range(2):
                        _kproj(fo, nt, act=False)

            # ---------------- Phase 3: attention ----------------
            # Per (head, kt): [I|.] preload puts bias^T in PSUM, one stride-0
            # slot-repeated DR matmul adds k_h.T @ q_h; exp from 2-bank PSUM
            # -> fp8 e_t; [v|1].T @ e_t accumulates o^T + rowsum.
            with (
                tc.tile_pool(
                    name="sc_ps", bufs=2, space=bass.MemorySpace.PSUM
                ) as scps,
                tc.tile_pool(
                    name="o_ps", bufs=2, space=bass.MemorySpace.PSUM
                ) as ops,
            ):
                for h in range(NH):
                    hp = (h % 2) * Dh
                    fc = h // 2
                    o_ps = ops.tile([65, Sq], F32, tag="o", name="o_ps")
                    bt = bpool.tile([P, NKT, Sq], FP8, tag="bt", name="bt")
                    _dq().dma_start(
                        bt, biasT_d[h].rearrange("(kt p) q -> p kt q", p=P)
                    )
                    for g in range(4):
                        scp = scps.tile([P, 2, Sq], F32, tag="sc", name="scp")
                        e_t = epool.tile([P, 2, Sq], FP8, tag="et", name="e_t")
                        for j in range(2):
                            kt = 2 * g + j
                            nc.tensor.matmul(
                                scp[:, j, :],
                                ipadE if j == 0 else ipadO,
                                bt[:, 2 * g:2 * g + 2, :],
                                start=True, stop=False, perf_mode=DR,
                            )
                            nc.tensor.matmul(
                                scp[:, j, :],
                                _srep(kT[:, fc, kt * P:(kt + 1) * P]),
                                qT[:, :, h, :],
                                start=False, stop=True, perf_mode=DR,
                            )
                        nc.scalar.activation(
                            e_t, scp, AF.Exp, bias=esh_t, scale=SCALE
                        )
                        nc.tensor.matmul(
                            o_ps,
                            v_aug[:, 2 * g:2 * g + 2, h * 65:(h + 1) * 65],
                            e_t,
                            start=(g == 0), stop=(g == 3), perf_mode=DR,
                        )
                    rinv = rpool.tile([1, Sq], F32, tag="rinv", name="rinv")
                    nc.vector.reciprocal(rinv, o_ps[64:65, :])
                    rb = rpool.tile([Dh, Sq], F32, tag="rb", name="rb")
                    nc.gpsimd.partition_broadcast(rb, rinv[0:1, :])
                    nc.vector.tensor_tensor(
                        out=oT[hp:hp + Dh, fc, :],
                        in0=o_ps[0:64, :], in1=rb,
                        op=OP.mult,
                    )

        # ---------------- Phase 4: output projection + residual -------------
        with (
            tc.tile_pool(name="ph4c", bufs=1) as ph4c,
            tc.tile_pool(name="ph4ps", bufs=3, space=bass.MemorySpace.PSUM) as ps4,
        ):
            bo_b = None
            if with_biases:
                bo_b = ph4c.tile([P, H], F32, tag="bob")
                nc.gpsimd.dma_start(bo_b, _pbcast(bo_d[:], P))
            for t in range(NTC):
                for half in range(2):
                    ps = ps4.tile([P, 512], F32, tag="mm", name="ps")
                    for g in range(4):
                        nc.tensor.matmul(
                            ps,
                            oT[:, 2 * g:2 * g + 2, t * P:(t + 1) * P],
                            wo_sb[:, 2 * g:2 * g + 2,
                                  half * 512:(half + 1) * 512],
                            start=(g == 0), stop=(g == 3), perf_mode=DR,
                        )
                    xs = x_sb[:, t, half * 512:(half + 1) * 512]
                    nc.vector.scalar_tensor_tensor(
                        out=xs, in0=ps, scalar=IWS,
                        in1=x2_sb[:, t, half * 512:(half + 1) * 512],
                        op0=OP.mult, op1=OP.add,
                    )
                    if with_biases:
                        nc.vector.tensor_add(
                            xs, xs, bo_b[:, half * 512:(half + 1) * 512]
                        )

        # ---------------- Phase 5+6+7: final LN + FFN ----------------
        # FFN precision: weights and activations both carried as fp8 hi+lo
        # planes; each matmul computes hi*hi + lo*hi + hi*lo (the lo*lo term
        # is negligible) -> bf16-class FFN accuracy at fp8-DR speed.
        with (
            tc.tile_pool(name="hTp", bufs=1) as hTp,
            tc.tile_pool(name="y3", bufs=1) as y3p,
        ):
            hT = hTp.tile([P, NFFC, 2, Sq], FP8, tag="hT")   # planes hi/lo
            y3T = y3p.tile([P, NFC, 2, Sq], FP8, tag="y3T")  # planes hi/lo

            with (
                tc.tile_pool(name="ph5", bufs=4) as ph5,
                tc.tile_pool(name="ph5w", bufs=2) as ph5w,
                tc.tile_pool(name="ph5b", bufs=1) as ph5b,
                tc.tile_pool(name="ph5c", bufs=1) as ph5c,
            ):
                y3T_bf = ph5b.tile([P, NFC, Sq], BF16, tag="y3Tb")
                eps_t = ph5c.tile([P, 1], F32, tag="eps")
                nc.vector.memset(eps_t, EPS)
                lnfg_b = lnfb_b = None
                if ln_affine:
                    lnfg_b = ph5c.tile([P, H], F32, tag="lnfg")
                    lnfb_b = ph5c.tile([P, H], F32, tag="lnfb")
                    nc.gpsimd.dma_start(lnfg_b, _pbcast(lnfg_d[:], P))
                    nc.gpsimd.dma_start(lnfb_b, _pbcast(lnfb_d[:], P))
                for t in range(NTC):
                    yt = ph5w.tile([P, H], BF16, tag="yt", name="yt")
                    _layer_norm(ph5, yt, x_sb[:, t, :], lnfg_b, lnfb_b, eps_t)
                    _tq().dma_start_transpose(
                        y3T_bf[:, :, t * P:(t + 1) * P], yt[:, :]
                    )
                nc.vector.tensor_copy(y3T[:, :, 0, :], y3T_bf)
                nc.vector.tensor_tensor(
                    out=y3T[:, :, 1, :], in0=y3T_bf, in1=y3T[:, :, 0, :],
                    op=OP.subtract,
                )

            # FFN1 + gelu -> dual-plane hT, then FFN2 in ONE 8-bank pass
            with (
                tc.tile_pool(name="b1l", bufs=1) as b1pool,
                tc.tile_pool(name="w1s", bufs=2) as w1sp,
                tc.tile_pool(name="w2s", bufs=2) as w2sp,
                tc.tile_pool(name="h32", bufs=1) as h32p,
                tc.tile_pool(name="outp", bufs=1) as outp,
            ):
                b1_sb = b2_b = None
                if with_biases:
                    b1_sb = b1pool.tile([P, NFFC], F32, tag="b1")
                    nc.gpsimd.dma_start(b1_sb, b1_d[:, :])
                    b2_b = b1pool.tile([P, H], F32, tag="b2b")
                    nc.gpsimd.dma_start(b2_b, _pbcast(b2_d[:], P))

                # FFN1 streamed in 4 groups of 8 ffc chunks (group 0 was
                # prefetched into wlate during the early phases)
                GW = 1024  # ff columns per weight group
                f1ctx = tc.tile_pool(name="f1ps", bufs=2,
                                     space=bass.MemorySpace.PSUM)
                f1ps = f1ctx.__enter__()
                for gi in range(FF // GW):
                    if gi == 0:
                        w1h, w1l = w1h0_sb, w1l0_sb
                    else:
                        w1h = w1sp.tile([P, NFC, GW], FP8, tag="w1h",
                                        name="w1h")
                        w1l = w1sp.tile([P, NFC, GW], FP8, tag="w1l",
                                        name="w1l")
                        _dq().dma_start(
                            w1h, w1h_d[:, gi * GW:(gi + 1) * GW].rearrange(
                                "(kc p) f -> p kc f", p=P))
                        _dq().dma_start(
                            w1l, w1l_d[:, gi * GW:(gi + 1) * GW].rearrange(
                                "(kc p) f -> p kc f", p=P))
                    for fp_ in range(4):
                        ps = f1ps.tile([P, 2, Sq], F32, tag="mm", name="ps")
                        for i in range(2):
                            lo = (2 * fp_ + i) * P  # local 128-col block
                            for kc in range(NFC):
                                nc.tensor.matmul(
                                    ps[:, i, :],
                                    _srep(w1h[:, kc, lo:lo + P]),
                                    y3T[:, kc, :, :],
                                    start=(kc == 0), stop=False, perf_mode=DR,
                                )
                            for g in range(4):
                                nc.tensor.matmul(
                                    ps[:, i, :],
                                    w1l[:, 2 * g:2 * g + 2, lo:lo + P],
                                    y3T[:, 2 * g:2 * g + 2, 0, :],
                                    start=False, stop=(g == 3), perf_mode=DR,
                                )
                        h32 = h32p.tile([P, 2, Sq], F32, tag="h32",
                                        name="h32")
                        ffc0 = 8 * gi + 2 * fp_
                        if with_biases:
                            for i in range(2):
                                nc.scalar.activation(
                                    h32[:, i, :], ps[:, i, :], AF.Gelu,
                                    bias=b1_sb[:, ffc0 + i:ffc0 + i + 1],
                                    scale=IWS,
                                )
                        else:
                            nc.scalar.activation(h32, ps, AF.Gelu, scale=IWS)
                        nc.vector.tensor_copy(
                            hT[:, ffc0:ffc0 + 2, 0, :], h32)
                        nc.vector.tensor_tensor(
                            out=hT[:, ffc0:ffc0 + 2, 1, :], in0=h32,
                            in1=hT[:, ffc0:ffc0 + 2, 0, :], op=OP.subtract,
                        )

                # prefetch w2 group 0 while FFN1 still runs
                w2tiles = []
                for gi in range(4):
                    w2h = w2sp.tile([P, 8, H], FP8, tag="w2h", name="w2h")
                    w2l = w2sp.tile([P, 8, H], FP8, tag="w2l", name="w2l")
                    _dq().dma_start(
                        w2h, w2h_d[gi * GW:(gi + 1) * GW, :].rearrange(
                            "(c p) f -> p c f", p=P))
                    _dq().dma_start(
                        w2l, w2l_d[gi * GW:(gi + 1) * GW, :].rearrange(
                            "(c p) f -> p c f", p=P))
                    w2tiles.append((w2h, w2l))

                f1ctx.__exit__(None, None, None)

                # FFN2: single pass over all 4 token tiles (8 PSUM banks);
                # w2 planes streamed once in 4 groups of 8 ff chunks
                with tc.tile_pool(
                    name="f2ps", bufs=1, space=bass.MemorySpace.PSUM
                ) as f2ps:
                    accs = [
                        f2ps.tile([P, H], F32, tag=f"acc{t}",
                                  name=f"acc{t}")
                        for t in range(NTC)
                    ]
                    for gi in range(4):
                        w2h, w2l = w2tiles[gi]
                        for t in range(NTC):
                            for c in range(8):
                                ffc = 8 * gi + c
                                for nt in range(2):
                                    nc.tensor.matmul(
                                        accs[t][:, nt * 512:(nt + 1) * 512],
                                        hT[:, ffc, :, t * P:(t + 1) * P],
                                        _srep(w2h[:, c,
                                                  nt * 512:(nt + 1) * 512]),
                                        start=(ffc == 0), stop=False,
                                        perf_mode=DR,
                                    )
                            for c2 in range(4):
                                ffp = 8 * gi + 2 * c2
                                for nt in range(2):
                                    nc.tensor.matmul(
                                        accs[t][:, nt * 512:(nt + 1) * 512],
                                        hT[:, ffp:ffp + 2, 0,
                                           t * P:(t + 1) * P],
                                        w2l[:, 2 * c2:2 * c2 + 2,
                                            nt * 512:(nt + 1) * 512],
                                        start=False,
                                        stop=(gi == 3 and c2 == 3),
                                        perf_mode=DR,
                                    )
                    for t in range(NTC):
                        ot = outp.tile([P, H], F32, tag="ot", name="ot")
                        nc.vector.scalar_tensor_tensor(
                            out=ot, in0=accs[t], scalar=IWS,
                            in1=x_sb[:, t, :], op0=OP.mult, op1=OP.add,
                        )
                        if with_biases:
                            nc.vector.tensor_add(ot, ot, b2_b)
                        _dq().dma_start(out_d[t * P:(t + 1) * P, :], ot)
            hTctx.__exit__(None, None, None)

    nc.compile()
    return nc


_CACHE: dict = {}


def _get_program(ln_affine=True, with_biases=True):
    key = (ln_affine, with_biases)
    if key not in _CACHE:
        _CACHE[key] = build_program(
            ln_affine=ln_affine, with_biases=with_biases
        )
    return _CACHE[key]


def _detect_fast_flags(inputs):
    ones = lambda k: bool(np.all(np.asarray(inputs[k]) == 1.0))
    zeros = lambda k: bool(np.all(np.asarray(inputs[k]) == 0.0))
    ln_affine = not (
        ones("ln1_g") and ones("ln2_g") and ones("lnf_g")
        and zeros("ln1_b") and zeros("ln2_b") and zeros("lnf_b")
    )
    with_biases = not (
        zeros("bq") and zeros("bk") and zeros("bv") and zeros("bo")
        and zeros("b1") and zeros("b2")
    )
    return ln_affine, with_biases


def _make_in_maps(inputs: dict) -> list[dict]:
    import ml_dtypes

    fp8 = ml_dtypes.float8_e4m3
    bf16 = ml_dtypes.bfloat16
    f32 = lambda a: np.ascontiguousarray(np.asarray(a, dtype=np.float32))
    w8 = lambda a: np.ascontiguousarray(
        (np.asarray(a, dtype=np.float32) * WS).astype(fp8)
    )

    def w8planes(a):
        ws = np.asarray(a, dtype=np.float32) * WS
        hi = ws.astype(fp8)
        lo = (ws - hi.astype(np.float32)).astype(fp8)
        return np.ascontiguousarray(hi), np.ascontiguousarray(lo)

    x1 = np.asarray(inputs["x1"], dtype=np.float32)
    x2 = np.asarray(inputs["x2"], dtype=np.float32)
    attn_bias = np.asarray(inputs["attn_bias"], dtype=np.float32)
    w1h, w1l = w8planes(inputs["w1"])
    w2h, w2l = w8planes(inputs["w2"])
    shared = {
        "wq": w8(inputs["wq"]),
        "wk": w8(inputs["wk"]),
        "wv": w8(inputs["wv"]),
        "wo": w8(inputs["wo"]),
        "w1h": w1h, "w1l": w1l,
        "w2h": w2h, "w2l": w2l,
        "bq_pc": f32(np.asarray(inputs["bq"]).reshape(NFC, P).T),
        "bk_pc": f32(np.asarray(inputs["bk"]).reshape(NFC, P).T),
        "bv": f32(inputs["bv"]),
        "bo": f32(inputs["bo"]),
        "b1_pc": f32(np.asarray(inputs["b1"]).reshape(NFFC, P).T),
        "b2": f32(inputs["b2"]),
        "ln1_g": f32(inputs["ln1_g"]),
        "ln1_b": f32(inputs["ln1_b"]),
        "ln2_g": f32(inputs["ln2_g"]),
        "ln2_b": f32(inputs["ln2_b"]),
        "lnf_g": f32(inputs["lnf_g"]),
        "lnf_b": f32(inputs["lnf_b"]),
    }
    in_maps = []
    for c in range(8):
        b, half = c // 2, c % 2
        q0 = half * Sq
        in_maps.append(
            {
                "x1": np.ascontiguousarray(x1[b].astype(bf16)),
                "x2h": np.ascontiguousarray(x2[b, q0:q0 + Sq].astype(bf16)),
                "biasT": np.ascontiguousarray(
                    (attn_bias[b, :, q0:q0 + Sq, :].transpose(0, 2, 1)
                     * (1.0 / SCALE)).astype(fp8)
                ),
                **shared,
            }
        )
    return in_maps


def _assemble(results: list[dict]) -> np.ndarray:
    out = np.empty((B, S, H), np.float32)
    for c in range(8):
        b, half = c // 2, c % 2
        out[b, half * Sq:(half + 1) * Sq] = results[c]["out"]
    return out


def run(inputs: dict, **run_kwargs):
    from concourse.bass_utils import run_bass_kernel_spmd

    ln_affine, with_biases = _detect_fast_flags(inputs)
    nc = _get_program(ln_affine=ln_affine, with_biases=with_biases)
    in_maps = _make_in_maps(inputs)
    res = run_bass_kernel_spmd(nc, in_maps, core_ids=list(range(8)), **run_kwargs)
    return _assemble(res.results), res


def kernel(**inputs) -> np.ndarray:
    out, _ = run(inputs)
    return out


# revision 39
# speedup vs baseline: 1.4899x; 1.0094x over previous
"""CrossTransformerLayer on 8 TRN2 NeuronCores — fp8 DoubleRow edition.

Sharding: core c -> (batch b = c//2, q-half = c%2). Each core computes its
512 query rows of its batch end-to-end (k/v over the full 1024-token x1
sequence); no cross-core collectives.

Key device-side ideas (validated on-device in minitest.py):
  * Every large GEMM runs as fp8(e4m3) DoubleRow matmuls: 2x128 contraction
    per instruction at 0.5 cycles/row -> 4x the bf16 PE throughput. Weights
    are host-scaled by 32 (fp8 precision) and rescaled by 1/32 in the
    PSUM->SBUF drains.
  * Attention scores^T[k,q] contract only d=64 per head, too shallow for a
    DoubleRow pair. Instead: qT chunks are parity-padded with zeros (head h
    occupies partitions (h%2)*64..+64, the sibling half is zero), the packed
    kT chunk is slot-repeated with a stride-0 AP, and rhs slot 1 points at an
    all-zero qT plane -> one 256-cycle DR matmul per (head, kt) tile.
  * The attention bias lands in PSUM via fp8 DR "identity preload": lhsT
    [I|0] / [0|I], rhs = a pair of bias^T k-tiles -> 256 cycles per tile.
  * exp(scale*x - 3) on Act engine straight from 2-bank PSUM into fp8 e_t
    (the -3 shift keeps e^x inside e4m3 range; it cancels in the rowsum
    normalization). [v|1] rows are fp8, so the o-matmul is DR as well.
  * All y-transposes go through the DMA XBAR (dma_start_transpose, bf16,
    SP/Act queues) instead of PE+DVE; cheap SBUF->SBUF copies on the gpsimd
    engine convert bf16 y^T -> fp8 for the DR matmuls.
  * PSUM drains are DVE-only (gpsimd has no PSUM port); gpsimd takes the
    SBUF-side work (converts, memsets, rowsum broadcast); Act owns exp/gelu;
    bulk DMA alternates between the SP and Act queues (w2 on the gpsimd
    queue), which all transfer concurrently.

Numerics: x1/x2 in bf16; LN, softmax logits, residuals and the output stay
fp32; fp8 only on matmul operands (y^T, q^T, k^T, v, e^p, weights, bias^T).
"""

import sys

sys.path.insert(0, "/opt/trn_rl_repo")

from contextlib import ExitStack

import numpy as np

import concourse.bass as bass
import concourse.tile as tile
from concourse import bacc, mybir
from concourse.masks import make_identity

F32 = mybir.dt.float32
BF16 = mybir.dt.bfloat16
FP8 = mybir.dt.float8e4
DR = mybir.MatmulPerfMode.DoubleRow

B = 4
S = 1024   # full (k) sequence
Sq = 512   # query rows per core
H = 1024
NH = 16
Dh = 64    # head dim
FF = 4096
P = 128
NKT = S // P    # 8 k-token tiles
NFC = H // P    # 8 feature chunks
NTC = Sq // P   # 4 q-token tiles
NFFC = FF // P  # 32 ff chunks
EPS = 1e-5
SCALE = float(Dh) ** -0.5
WS = 32.0       # host-side fp8 weight scale
IWS = 1.0 / WS
ESHIFT = -3.0   # exp bias shift; cancels in the rowsum normalization
AF = mybir.ActivationFunctionType
OP = mybir.AluOpType


def _pbcast(ap: bass.AP, parts: int) -> bass.AP:
    """[.., N] access pattern -> [parts, .., N] with partition step 0."""
    return bass.AP(
        tensor=ap.tensor,
        offset=ap.offset,
        ap=[[0, parts]] + [list(d) for d in ap.ap],
    )


def _srep(ap: bass.AP, n: int = 2) -> bass.AP:
    """[p, F] AP -> [p, n, F] with slot stride 0 (repeat the same block)."""
    return bass.AP(
        tensor=ap.tensor,
        offset=ap.offset,
        ap=[list(ap.ap[0])] + [[0, n]] + [list(d) for d in ap.ap[1:]],
    )


def build_program(ln_affine=True, with_biases=True):
    nc = bacc.Bacc("TRN2", target_bir_lowering=False, debug=False)

    x1_d = nc.dram_tensor("x1", (S, H), BF16, kind="ExternalInput")
    x2h_d = nc.dram_tensor("x2h", (Sq, H), BF16, kind="ExternalInput")
    biasT_d = nc.dram_tensor("biasT", (NH, S, Sq), FP8, kind="ExternalInput")
    wq_d = nc.dram_tensor("wq", (H, H), FP8, kind="ExternalInput")
    wk_d = nc.dram_tensor("wk", (H, H), FP8, kind="ExternalInput")
    wv_d = nc.dram_tensor("wv", (H, H), FP8, kind="ExternalInput")
    wo_d = nc.dram_tensor("wo", (H, H), FP8, kind="ExternalInput")
    w1h_d = nc.dram_tensor("w1h", (H, FF), FP8, kind="ExternalInput")
    w1l_d = nc.dram_tensor("w1l", (H, FF), FP8, kind="ExternalInput")
    w2h_d = nc.dram_tensor("w2h", (FF, H), FP8, kind="ExternalInput")
    w2l_d = nc.dram_tensor("w2l", (FF, H), FP8, kind="ExternalInput")
    out_d = nc.dram_tensor("out", (Sq, H), F32, kind="ExternalOutput")
    bq_d = nc.dram_tensor("bq_pc", (P, NFC), F32, kind="ExternalInput")
    bk_d = nc.dram_tensor("bk_pc", (P, NFC), F32, kind="ExternalInput")
    bv_d = nc.dram_tensor("bv", (H,), F32, kind="ExternalInput")
    bo_d = nc.dram_tensor("bo", (H,), F32, kind="ExternalInput")
    b1_d = nc.dram_tensor("b1_pc", (P, NFFC), F32, kind="ExternalInput")
    b2_d = nc.dram_tensor("b2", (H,), F32, kind="ExternalInput")
    ln1g_d = nc.dram_tensor("ln1_g", (H,), F32, kind="ExternalInput")
    ln1b_d = nc.dram_tensor("ln1_b", (H,), F32, kind="ExternalInput")
    ln2g_d = nc.dram_tensor("ln2_g", (H,), F32, kind="ExternalInput")
    ln2b_d = nc.dram_tensor("ln2_b", (H,), F32, kind="ExternalInput")
    lnfg_d = nc.dram_tensor("lnf_g", (H,), F32, kind="ExternalInput")
    lnfb_d = nc.dram_tensor("lnf_b", (H,), F32, kind="ExternalInput")

    # Bulk DMA queues: SP (hwdge) and gpsimd (swdge). The Act queue is kept
    # free for compute dispatch: every hwdge DMA costs ~630ns of issuing-queue
    # SEQ time, which starves exp dispatch during attention.
    q_iter = {"i": 0}

    def _dq():
        q_iter["i"] += 1
        return nc.sync if q_iter["i"] % 2 else nc.gpsimd

    # XBAR transposes must use a hwdge queue (SP/Act); they are few.
    t_iter = {"i": 0}

    def _tq():
        t_iter["i"] += 1
        return nc.sync if t_iter["i"] % 2 else nc.scalar

    def _drain(out, ps, bias):
        """PSUM -> SBUF fp8/f32 with the 1/WS weight rescale (+ bias)."""
        if with_biases and bias is not None:
            nc.vector.tensor_scalar(out, ps, IWS, bias, op0=OP.mult,
                                    op1=OP.add)
        else:
            nc.vector.tensor_scalar(out, ps, IWS, None, op0=OP.mult)

    def _layer_norm(pool, y_out, x_in, g_b, b_b, eps_t):
        """y = (x - mean)/sqrt(var+eps) [* g + b] on a [128, H] tile."""
        stats = pool.tile([P, 2, 6], F32, tag="ln_stats", name="stats")
        nc.vector.bn_stats(stats[:, 0, :], x_in[:, 0:512])
        nc.vector.bn_stats(stats[:, 1, :], x_in[:, 512:1024])
        mv = pool.tile([P, 2], F32, tag="ln_mv", name="mv")
        nc.vector.bn_aggr(mv, stats)
        std = pool.tile([P, 1], F32, tag="ln_std", name="std")
        nc.scalar.activation(std, mv[:, 1:2], AF.Sqrt, bias=eps_t, scale=1.0)
        rstd = pool.tile([P, 1], F32, tag="ln_rstd", name="rstd")
        nc.vector.reciprocal(rstd, std)
        nc.vector.tensor_scalar(
            y_out, x_in, mv[:, 0:1], rstd, op0=OP.subtract, op1=OP.mult
        )
        if ln_affine:
            nc.vector.tensor_mul(y_out, y_out, g_b)
            nc.vector.tensor_add(y_out, y_out, b_b)

    with tile.TileContext(nc) as tc, ExitStack() as top:
        persist = top.enter_context(tc.tile_pool(name="persist", bufs=1))
        # [I|0] and [0|I] fp8 stationary tiles for the bias preloads
        ipadE = persist.tile([P, 2, P], FP8, tag="ipadE")
        ipadO = persist.tile([P, 2, P], FP8, tag="ipadO")
        # qT: plane 0 = parity-padded q chunks, plane 1 = zeros (DR slot 1)
        qT = persist.tile([P, 2, NH, Sq], FP8, tag="qT")
        oT = persist.tile([P, NFC, Sq], FP8, tag="oT")
        esh_t = persist.tile([P, 1], F32, tag="esh")
        nc.vector.memset(esh_t, ESHIFT)

        def _setup_consts():
            # Emitted AFTER the input/weight DMA issues: the gpsimd SEQ runs
            # its queue in order, and these memsets must not delay the DMAs.
            nc.gpsimd.memset(ipadE, 0.0)
            make_identity(nc, ipadE[:, 0, :], nomemset=True)
            nc.gpsimd.memset(ipadO, 0.0)
            make_identity(nc, ipadO[:, 1, :], nomemset=True)
            nc.gpsimd.memset(qT[:, 1, :, :], 0.0)
            qT_ev = qT[:, 0, :, :].rearrange("p (hh t) q -> p hh t q", t=2)
            nc.gpsimd.memset(qT_ev[64:128, :, 0, :], 0.0)
            nc.gpsimd.memset(qT_ev[0:64, :, 1, :], 0.0)

        xp = top.enter_context(tc.tile_pool(name="xp", bufs=1))
        x_sb = xp.tile([P, NTC, H], BF16, tag="x")      # attn residual out
        x2_sb = xp.tile([P, NTC, H], BF16, tag="x2")    # x2h kept resident

        # wo + the first w1 group preallocated up top so their DMAs overlap
        # the early phases / attention
        wlate = top.enter_context(tc.tile_pool(name="wlate", bufs=1))
        wo_sb = wlate.tile([P, NFC, H], FP8, tag="wo")
        w1h0_sb = wlate.tile([P, NFC, 1024], FP8, tag="w1h0")
        w1l0_sb = wlate.tile([P, NFC, 1024], FP8, tag="w1l0")

        with (
            tc.tile_pool(name="qkv", bufs=1) as qkvp,
            tc.tile_pool(name="bias_s", bufs=3) as bpool,
            tc.tile_pool(name="expp", bufs=4) as epool,
            tc.tile_pool(name="rin", bufs=2) as rpool,
        ):
            kT = qkvp.tile([P, NFC, S], FP8, tag="kT")
            v_aug = qkvp.tile([P, NKT, NH * 65], FP8, tag="vaug")

            # ---------- Phase 1+2: LN, XBAR transpose, QKV projections ------
            with (
                tc.tile_pool(name="xin", bufs=1) as xinp,
                tc.tile_pool(name="ybf", bufs=1) as ybf,
                tc.tile_pool(name="yT", bufs=1) as yTp,
                tc.tile_pool(name="ph1", bufs=4) as ph1,
                tc.tile_pool(name="ph1w", bufs=3) as ph1w,
                tc.tile_pool(name="ph1c", bufs=1) as ph1c,
                tc.tile_pool(name="wload", bufs=1) as wpool,
                tc.tile_pool(name="vecs", bufs=1) as vecs,
                tc.tile_pool(
                    name="ph2ps", bufs=4, space=bass.MemorySpace.PSUM
                ) as ps2,
            ):
                x1_sb = xinp.tile([P, NKT, H], BF16, tag="x1")
                # Inputs first, all on SP (they gate the LN ladders); weights
                # on the gpsimd queue so they transfer concurrently.
                for t in range(NTC):
                    nc.sync.dma_start(
                        x2_sb[:, t, :], x2h_d[t * P:(t + 1) * P, :]
                    )
                for t in range(NKT):
                    nc.sync.dma_start(
                        x1_sb[:, t, :], x1_d[t * P:(t + 1) * P, :]
                    )

                wq_sb = wpool.tile([P, NFC, H], FP8, tag="wq", name="wq_sb")
                nc.gpsimd.dma_start(
                    wq_sb, wq_d.rearrange("(kc p) f -> p kc f", p=P))
                wk_sb = wpool.tile([P, NFC, H], FP8, tag="wk", name="wk_sb")
                nc.sync.dma_start(
                    wk_sb, wk_d.rearrange("(kc p) f -> p kc f", p=P))
                wv_sb = wpool.tile([P, NFC, H], FP8, tag="wv", name="wv_sb")
                nc.gpsimd.dma_start(
                    wv_sb, wv_d.rearrange("(kc p) f -> p kc f", p=P))
                nc.sync.dma_start(
                    wo_sb, wo_d.rearrange("(kc p) f -> p kc f", p=P))
                nc.gpsimd.dma_start(
                    w1h0_sb, w1h_d[:, 0:1024].rearrange(
                        "(kc p) f -> p kc f", p=P))
                nc.sync.dma_start(
                    w1l0_sb, w1l_d[:, 0:1024].rearrange(
                        "(kc p) f -> p kc f", p=P))
                _setup_consts()
                ones_view = v_aug[:, :, :].rearrange(
                    "p t (h j) -> p t h j", j=65
                )[:, :, :, 64:65]
                nc.gpsimd.memset(ones_view, 1.0)

                y2T_bf = ybf.tile([P, NFC, Sq], BF16, tag="y2Tb")
                # y1T bf16 staging is halved and reused (tokens 0-511, then
                # 512-1023) to cut SBUF peak
                y1T_bf = ybf.tile([P, NFC, Sq], BF16, tag="y1Tb")
                y2T = yTp.tile([P, NFC, Sq], FP8, tag="y2T")
                y1T = yTp.tile([P, NFC, S], FP8, tag="y1T")

                eps_t = ph1c.tile([P, 1], F32, tag="eps")
                nc.vector.memset(eps_t, EPS)
                ln1g_b = ln1b_b = ln2g_b = ln2b_b = None
                if ln_affine:
                    ln1g_b = ph1c.tile([P, H], F32, tag="ln1g")
                    ln1b_b = ph1c.tile([P, H], F32, tag="ln1b")
                    ln2g_b = ph1c.tile([P, H], F32, tag="ln2g")
                    ln2b_b = ph1c.tile([P, H], F32, tag="ln2b")
                    nc.gpsimd.dma_start(ln1g_b, _pbcast(ln1g_d[:], P))
                    nc.gpsimd.dma_start(ln1b_b, _pbcast(ln1b_d[:], P))
                    nc.gpsimd.dma_start(ln2g_b, _pbcast(ln2g_d[:], P))
                    nc.gpsimd.dma_start(ln2b_b, _pbcast(ln2b_d[:], P))

                bq_sb = bk_sb = bv_b = None
                if with_biases:
                    bq_sb = vecs.tile([P, NFC], F32, tag="bq")
                    bk_sb = vecs.tile([P, NFC], F32, tag="bk")
                    bv_b = vecs.tile([P, H], F32, tag="bvb")
                    nc.gpsimd.dma_start(bq_sb, bq_d[:, :])
                    nc.gpsimd.dma_start(bk_sb, bk_d[:, :])
                    nc.gpsimd.dma_start(bv_b, _pbcast(bv_d[:], P))

                # x2h -> y2 -> y2T (XBAR) -> fp8
                for t in range(NTC):
                    yt = ph1w.tile([P, H], BF16, tag="yt", name="yt")
                    _layer_norm(ph1, yt, x2_sb[:, t, :], ln2g_b, ln2b_b, eps_t)
                    _tq().dma_start_transpose(
                        y2T_bf[:, :, t * P:(t + 1) * P], yt[:, :]
                    )
                nc.vector.tensor_copy(y2T, y2T_bf)

                # q^T per fo chunk; parity-split drains into padded qT
                for fo in range(NFC):
                    ps = ps2.tile([P, Sq], F32, tag="mm", name="ps")
                    for g in range(4):
                        nc.tensor.matmul(
                            ps,
                            wq_sb[:, 2 * g:2 * g + 2, fo * P:(fo + 1) * P],
                            y2T[:, 2 * g:2 * g + 2, :],
                            start=(g == 0), stop=(g == 3), perf_mode=DR,
                        )
                    h0, h1 = 2 * fo, 2 * fo + 1
                    nc.scalar.activation(
                        qT[0:64, 0, h0, :], ps[0:64, :], AF.Copy,
                        bias=(bq_sb[0:64, fo:fo + 1] if with_biases else 0.0),
                        scale=IWS,
                    )
                    nc.scalar.activation(
                        qT[64:128, 0, h1, :], ps[64:128, :], AF.Copy,
                        bias=(bq_sb[64:128, fo:fo + 1] if with_biases
                              else 0.0),
                        scale=IWS,
                    )

                # x1 -> y1 -> y1T (XBAR, halved staging) -> fp8
                for half in range(2):
                    for i in range(4):
                        t = 4 * half + i
                        yt = ph1w.tile([P, H], BF16, tag="yt", name="yt")
                        _layer_norm(ph1, yt, x1_sb[:, t, :],
                                    ln1g_b, ln1b_b, eps_t)
                        _tq().dma_start_transpose(
                            y1T_bf[:, :, i * P:(i + 1) * P], yt[:, :]
                        )
                    nc.vector.tensor_copy(
                        y1T[:, :, half * 512:(half + 1) * 512], y1T_bf
                    )

                # v and k projections, emitted in the order attention consumes
                # them (k fo=0,1 first so heads 0-3 can start, then v, then
                # the remaining k chunks). All drains on DVE: it idles during
                # the Act-bound attention phase and absorbs the stragglers.
                def _vproj(t, nt, act):
                    ps = ps2.tile([P, 512], F32, tag="mm", name="ps")
                    for g in range(4):
                        nc.tensor.matmul(
                            ps,
                            y1T[:, 2 * g:2 * g + 2, t * P:(t + 1) * P],
                            wv_sb[:, 2 * g:2 * g + 2,
                                  nt * 512:(nt + 1) * 512],
                            start=(g == 0), stop=(g == 3), perf_mode=DR,
                        )
                    dst = v_aug[
                        :, t, nt * 8 * 65:(nt * 8 + 8) * 65
                    ].rearrange("p (h j) -> p h j", j=65)[:, :, 0:64]
                    psr = ps.rearrange("p (h j) -> p h j", j=64)
                    if with_biases:
                        nc.vector.scalar_tensor_tensor(
                            out=dst, in0=psr, scalar=IWS,
                            in1=bv_b[
                                :, nt * 512:(nt + 1) * 512
                            ].rearrange("p (h j) -> p h j", j=64),
                            op0=OP.mult, op1=OP.add,
                        )
                    elif act:
                        nc.scalar.activation(dst, psr, AF.Copy, scale=IWS)
                    else:
                        nc.vector.tensor_scalar(
                            dst, psr, IWS, None, op0=OP.mult)

                def _kproj(fo, nt, act):
                    ps = ps2.tile([P, 512], F32, tag="mm", name="ps")
                    for g in range(4):
                        nc.tensor.matmul(
                            ps,
                            wk_sb[:, 2 * g:2 * g + 2, fo * P:(fo + 1) * P],
                            y1T[:, 2 * g:2 * g + 2,
                                nt * 512:(nt + 1) * 512],
                            start=(g == 0), stop=(g == 3), perf_mode=DR,
                        )
                    if act:
                        nc.scalar.activation(
                            kT[:, fo, nt * 512:(nt + 1) * 512], ps, AF.Copy,
                            bias=(bk_sb[:, fo:fo + 1] if with_biases
                                  else 0.0),
                            scale=IWS,
                        )
                    else:
                        _drain(kT[:, fo, nt * 512:(nt + 1) * 512], ps,
                               bk_sb[:, fo:fo + 1] if with_biases else None)

                # drains split by attention consumer order: the early chunks
                # (k fo0-3, v nt0) drain on Act right ahead of its exps; the
                # late chunks drain on DVE, overlapping the early exps.
                for fo in (0, 1):
                    for nt in range(2):
                        _kproj(fo, nt, act=True)
                for t in range(NKT):
                    _vproj(t, 0, act=True)
                for fo in (2, 3):
                    for nt in range(2):
                        _kproj(fo, nt, act=True)
                for t in range(NKT):
                    _vproj(t, 1, act=False)
                for fo in range(4, NFC):
                    for nt in range(2):
                        _kproj(fo, nt, act=False)

            # ---------------- Phase 3: attention ----------------
            # Per (head, kt): [I|.] preload puts bias^T in PSUM, one stride-0
            # slot-repeated DR matmul adds k_h.T @ q_h; exp from 2-bank PSUM
            # -> fp8 e_t; [v|1].T @ e_t accumulates o^T + rowsum.
            with (
                tc.tile_pool(
                    name="sc_ps", bufs=2, space=bass.MemorySpace.PSUM
                ) as scps,
                tc.tile_pool(
                    name="o_ps", bufs=2, space=bass.MemorySpace.PSUM
                ) as ops,
            ):
                for h in range(NH):
                    hp = (h % 2) * Dh
                    fc = h // 2
                    o_ps = ops.tile([65, Sq], F32, tag="o", name="o_ps")
                    bt = bpool.tile([P, NKT, Sq], FP8, tag="bt", name="bt")
                    _dq().dma_start(
                        bt, biasT_d[h].rearrange("(kt p) q -> p kt q", p=P)
                    )
                    for g in range(4):
                        scp = scps.tile([P, 2, Sq], F32, tag="sc", name="scp")
                        e_t = epool.tile([P, 2, Sq], FP8, tag="et", name="e_t")
                        for j in range(2):
                            kt = 2 * g + j
                            nc.tensor.matmul(
                                scp[:, j, :],
                                ipadE if j == 0 else ipadO,
                                bt[:, 2 * g:2 * g + 2, :],
                                start=True, stop=False, perf_mode=DR,
                            )
                            nc.tensor.matmul(
                                scp[:, j, :],
                                _srep(kT[:, fc, kt * P:(kt + 1) * P]),
                                qT[:, :, h, :],
                                start=False, stop=True, perf_mode=DR,
                            )
                        nc.scalar.activation(
                            e_t, scp, AF.Exp, bias=esh_t, scale=SCALE
                        )
                        nc.tensor.matmul(
                            o_ps,
                            v_aug[:, 2 * g:2 * g + 2, h * 65:(h + 1) * 65],
                            e_t,
                            start=(g == 0), stop=(g == 3), perf_mode=DR,
                        )
                    rinv = rpool.tile([1, Sq], F32, tag="rinv", name="rinv")
                    nc.vector.reciprocal(rinv, o_ps[64:65, :])
                    rb = rpool.tile([Dh, Sq], F32, tag="rb", name="rb")
                    nc.gpsimd.partition_broadcast(rb, rinv[0:1, :])
                    nc.vector.tensor_tensor(
                        out=oT[hp:hp + Dh, fc, :],
                        in0=o_ps[0:64, :], in1=rb,
                        op=OP.mult,
                    )

        # ---------------- Phase 4: output projection + residual -------------
        with (
            tc.tile_pool(name="ph4c", bufs=1) as ph4c,
            tc.tile_pool(name="ph4ps", bufs=3, space=bass.MemorySpace.PSUM) as ps4,
        ):
            bo_b = None
            if with_biases:
                bo_b = ph4c.tile([P, H], F32, tag="bob")
                nc.gpsimd.dma_start(bo_b, _pbcast(bo_d[:], P))
            for t in range(NTC):
                for half in range(2):
                    ps = ps4.tile([P, 512], F32, tag="mm", name="ps")
                    for g in range(4):
                        nc.tensor.matmul(
                            ps,
                            oT[:, 2 * g:2 * g + 2, t * P:(t + 1) * P],
                            wo_sb[:, 2 * g:2 * g + 2,
                                  half * 512:(half + 1) * 512],
                            start=(g == 0), stop=(g == 3), perf_mode=DR,
                        )
                    xs = x_sb[:, t, half * 512:(half + 1) * 512]
                    nc.vector.scalar_tensor_tensor(
                        out=xs, in0=ps, scalar=IWS,
                        in1=x2_sb[:, t, half * 512:(half + 1) * 512],
                        op0=OP.mult, op1=OP.add,
                    )
                    if with_biases:
                        nc.vector.tensor_add(
                            xs, xs, bo_b[:, half * 512:(half + 1) * 512]
                        )

        # ---------------- Phase 5+6+7: final LN + FFN ----------------
        # FFN precision: weights and activations both carried as fp8 hi+lo
        # planes; each matmul computes hi*hi + lo*hi + hi*lo (the lo*lo term
        # is negligible) -> bf16-class FFN accuracy at fp8-DR speed.
        with (
            tc.tile_pool(name="hTp", bufs=1) as hTp,
            tc.tile_pool(name="y3", bufs=1) as y3p,
        ):
            hT = hTp.tile([P, NFFC, 2, Sq], FP8, tag="hT")   # planes hi/lo
            y3T = y3p.tile([P, NFC, 2, Sq], FP8, tag="y3T")  # planes hi/lo

            with (
                tc.tile_pool(name="ph5", bufs=4) as ph5,
                tc.tile_pool(name="ph5w", bufs=2) as ph5w,
                tc.tile_pool(name="ph5b", bufs=1) as ph5b,
                tc.tile_pool(name="ph5c", bufs=1) as ph5c,
            ):
                y3T_bf = ph5b.tile([P, NFC, Sq], BF16, tag="y3Tb")
                eps_t = ph5c.tile([P, 1], F32, tag="eps")
                nc.vector.memset(eps_t, EPS)
                lnfg_b = lnfb_b = None
                if ln_affine:
                    lnfg_b = ph5c.tile([P, H], F32, tag="lnfg")
                    lnfb_b = ph5c.tile([P, H], F32, tag="lnfb")
                    nc.gpsimd.dma_start(lnfg_b, _pbcast(lnfg_d[:], P))
                    nc.gpsimd.dma_start(lnfb_b, _pbcast(lnfb_d[:], P))
                for t in range(NTC):
                    yt = ph5w.tile([P, H], BF16, tag="yt", name="yt")
                    _layer_norm(ph5, yt, x_sb[:, t, :], lnfg_b, lnfb_b, eps_t)
                    _tq().dma_start_transpose(
                        y3T_bf[:, :, t * P:(t + 1) * P], yt[:, :]
                    )
                nc.vector.tensor_copy(y3T[:, :, 0, :], y3T_bf)
                nc.vector.tensor_tensor(
                    out=y3T[:, :, 1, :], in0=y3T_bf, in1=y3T[:, :, 0, :],
                    op=OP.subtract,
                )

            # FFN1 + gelu -> dual-plane hT, then FFN2 in ONE 8-bank pass
            with (
                tc.tile_pool(name="b1l", bufs=1) as b1pool,
                tc.tile_pool(name="w1s", bufs=2) as w1sp,
                tc.tile_pool(name="w2s", bufs=2) as w2sp,
                tc.tile_pool(name="h32", bufs=1) as h32p,
                tc.tile_pool(name="outp", bufs=1) as outp,
            ):
                b1_sb = b2_b = None
                if with_biases:
                    b1_sb = b1pool.tile([P, NFFC], F32, tag="b1")
                    nc.gpsimd.dma_start(b1_sb, b1_d[:, :])
                    b2_b = b1pool.tile([P, H], F32, tag="b2b")
                    nc.gpsimd.dma_start(b2_b, _pbcast(b2_d[:], P))

                # FFN1 streamed in 4 groups of 8 ffc chunks (group 0 was
                # prefetched into wlate during the early phases)
                GW = 1024  # ff columns per weight group
                f1ctx = tc.tile_pool(name="f1ps", bufs=2,
                                     space=bass.MemorySpace.PSUM)
                f1ps = f1ctx.__enter__()
                for gi in range(FF // GW):
                    if gi == 0:
                        w1h, w1l = w1h0_sb, w1l0_sb
                    else:
                        w1h = w1sp.tile([P, NFC, GW], FP8, tag="w1h",
                                        name="w1h")
                        w1l = w1sp.tile([P, NFC, GW], FP8, tag="w1l",
                                        name="w1l")
                        _dq().dma_start(
                            w1h, w1h_d[:, gi * GW:(gi + 1) * GW].rearrange(
                                "(kc p) f -> p kc f", p=P))
                        _dq().dma_start(
                            w1l, w1l_d[:, gi * GW:(gi + 1) * GW].rearrange(
                                "(kc p) f -> p kc f", p=P))
                    for fp_ in range(4):
                        ps = f1ps.tile([P, 2, Sq], F32, tag="mm", name="ps")
                        for i in range(2):
                            lo = (2 * fp_ + i) * P  # local 128-col block
                            for kc in range(NFC):
                                nc.tensor.matmul(
                                    ps[:, i, :],
                                    _srep(w1h[:, kc, lo:lo + P]),
                                    y3T[:, kc, :, :],
                                    start=(kc == 0), stop=False, perf_mode=DR,
                                )
                            for g in range(4):
                                nc.tensor.matmul(
                                    ps[:, i, :],
                                    w1l[:, 2 * g:2 * g + 2, lo:lo + P],
                                    y3T[:, 2 * g:2 * g + 2, 0, :],
                                    start=False, stop=(g == 3), perf_mode=DR,
                                )
                        h32 = h32p.tile([P, 2, Sq], F32, tag="h32",
                                        name="h32")
                        ffc0 = 4 * gi + 2 * fp_
                        if with_biases:
                            for i in range(2):
                                nc.scalar.activation(
                                    h32[:, i, :], ps[:, i, :], AF.Gelu,
                                    bias=b1_sb[:, ffc0 + i:ffc0 + i + 1],
                                    scale=IWS,
                                )
                        else:
                            nc.scalar.activation(h32, ps, AF.Gelu, scale=IWS)
                        nc.vector.tensor_copy(
                            hT[:, ffc0:ffc0 + 2, 0, :], h32)
                        nc.vector.tensor_tensor(
                            out=hT[:, ffc0:ffc0 + 2, 1, :], in0=h32,
                            in1=hT[:, ffc0:ffc0 + 2, 0, :], op=OP.subtract,
                        )

                # prefetch w2 group 0 while FFN1 still runs
                w2tiles = []
                for gi in range(4):
                    w2h = w2sp.tile([P, 8, H], FP8, tag="w2h", name="w2h")
                    w2l = w2sp.tile([P, 8, H], FP8, tag="w2l", name="w2l")
                    _dq().dma_start(
                        w2h, w2h_d[gi * GW:(gi + 1) * GW, :].rearrange(
                            "(c p) f -> p c f", p=P))
                    _dq().dma_start(
                        w2l, w2l_d[gi * GW:(gi + 1) * GW, :].rearrange(
                            "(c p) f -> p c f", p=P))
                    w2tiles.append((w2h, w2l))

                f1ctx.__exit__(None, None, None)

                # FFN2: single pass over all 4 token tiles (8 PSUM banks);
                # w2 planes streamed once in 4 groups of 8 ff chunks
                with tc.tile_pool(
                    name="f2ps", bufs=1, space=bass.MemorySpace.PSUM
                ) as f2ps:
                    accs = [
                        f2ps.tile([P, H], F32, tag=f"acc{t}",
                                  name=f"acc{t}")
                        for t in range(NTC)
                    ]
                    for gi in range(4):
                        w2h, w2l = w2tiles[gi]
                        for t in range(NTC):
                            for c in range(8):
                                ffc = 8 * gi + c
                                for nt in range(2):
                                    nc.tensor.matmul(
                                        accs[t][:, nt * 512:(nt + 1) * 512],
                                        hT[:, ffc, :, t * P:(t + 1) * P],
                                        _srep(w2h[:, c,
                                                  nt * 512:(nt + 1) * 512]),
                                        start=(ffc == 0), stop=False,
                                        perf_mode=DR,
                                    )
                            for c2 in range(4):
                                ffp = 8 * gi + 2 * c2
                                for nt in range(2):
                                    nc.tensor.matmul(
                                        accs[t][:, nt * 512:(nt + 1) * 512],
                                        hT[:, ffp:ffp + 2, 0,
                                           t * P:(t + 1) * P],
                                        w2l[:, 2 * c2:2 * c2 + 2,
                                            nt * 512:(nt + 1) * 512],
                                        start=False,
                                        stop=(gi == 3 and c2 == 3),
                                        perf_mode=DR,
                                    )
                    for t in range(NTC):
                        ot = outp.tile([P, H], F32, tag="ot", name="ot")
                        nc.vector.scalar_tensor_tensor(
                            out=ot, in0=accs[t], scalar=IWS,
                            in1=x_sb[:, t, :], op0=OP.mult, op1=OP.add,
                        )
                        if with_biases:
                            nc.vector.tensor_add(ot, ot, b2_b)
                        _dq().dma_start(out_d[t * P:(t + 1) * P, :], ot)
            hTctx.__exit__(None, None, None)

    nc.compile()
    return nc


_CACHE: dict = {}


def _get_program(ln_affine=True, with_biases=True):
    key = (ln_affine, with_biases)
    if key not in _CACHE:
        _CACHE[key] = build_program(
            ln_affine=ln_affine, with_biases=with_biases
        )
    return _CACHE[key]


def _detect_fast_flags(inputs):
    ones = lambda k: bool(np.all(np.asarray(inputs[k]) == 1.0))
    zeros = lambda k: bool(np.all(np.asarray(inputs[k]) == 0.0))
    ln_affine = not (
        ones("ln1_g") and ones("ln2_g") and ones("lnf_g")
        and zeros("ln1_b") and zeros("ln2_b") and zeros("lnf_b")
    )
    with_biases = not (
        zeros("bq") and zeros("bk") and zeros("bv") and zeros("bo")
        and zeros("b1") and zeros("b2")
    )
    return ln_affine, with_biases


def _make_in_maps(inputs: dict) -> list[dict]:
    import ml_dtypes

    fp8 = ml_dtypes.float8_e4m3
    bf16 = ml_dtypes.bfloat16
    f32 = lambda a: np.ascontiguousarray(np.asarray(a, dtype=np.float32))
    w8 = lambda a: np.ascontiguousarray(
        (np.asarray(a, dtype=np.float32) * WS).astype(fp8)
    )

    def w8planes(a):
        ws = np.asarray(a, dtype=np.float32) * WS
        hi = ws.astype(fp8)
        lo = (ws - hi.astype(np.float32)).astype(fp8)
        return np.ascontiguousarray(hi), np.ascontiguousarray(lo)

    x1 = np.asarray(inputs["x1"], dtype=np.float32)
    x2 = np.asarray(inputs["x2"], dtype=np.float32)
    attn_bias = np.asarray(inputs["attn_bias"], dtype=np.float32)
    w1h, w1l = w8planes(inputs["w1"])
    w2h, w2l = w8planes(inputs["w2"])
    shared = {
        "wq": w8(inputs["wq"]),
        "wk": w8(inputs["wk"]),
        "wv": w8(inputs["wv"]),
        "wo": w8(inputs["wo"]),
        "w1h": w1h, "w1l": w1l,
        "w2h": w2h, "w2l": w2l,
        "bq_pc": f32(np.asarray(inputs["bq"]).reshape(NFC, P).T),
        "bk_pc": f32(np.asarray(inputs["bk"]).reshape(NFC, P).T),
        "bv": f32(inputs["bv"]),
        "bo": f32(inputs["bo"]),
        "b1_pc": f32(np.asarray(inputs["b1"]).reshape(NFFC, P).T),
        "b2": f32(inputs["b2"]),
        "ln1_g": f32(inputs["ln1_g"]),
        "ln1_b": f32(inputs["ln1_b"]),
        "ln2_g": f32(inputs["ln2_g"]),
        "ln2_b": f32(inputs["ln2_b"]),
        "lnf_g": f32(inputs["lnf_g"]),
        "lnf_b": f32(inputs["lnf_b"]),
    }
    in_maps = []
    for c in range(8):
        b, half = c // 2, c % 2
        q0 = half * Sq
        in_maps.append(
            {
                "x1": np.ascontiguousarray(x1[b].astype(bf16)),
                "x2h": np.ascontiguousarray(x2[b, q0:q0 + Sq].astype(bf16)),
                "biasT": np.ascontiguousarray(
                    (attn_bias[b, :, q0:q0 + Sq, :].transpose(0, 2, 1)
                     * (1.0 / SCALE)).astype(fp8)
                ),
                **shared,
            }
        )
    return in_maps


def _assemble(results: list[dict]) -> np.ndarray:
    out = np.empty((B, S, H), np.float32)
    for c in range(8):
        b, half = c // 2, c % 2
        out[b, half * Sq:(half + 1) * Sq] = results[c]["out"]
    return out


def run(inputs: dict, **run_kwargs):
    from concourse.bass_utils import run_bass_kernel_spmd

    ln_affine, with_biases = _detect_fast_flags(inputs)
    nc = _get_program(ln_affine=ln_affine, with_biases=with_biases)
    in_maps = _make_in_maps(inputs)
    res = run_bass_kernel_spmd(nc, in_maps, core_ids=list(range(8)), **run_kwargs)
    return _assemble(res.results), res


def kernel(**inputs) -> np.ndarray:
    out, _ = run(inputs)
    return out


# revision 40
# speedup vs baseline: 1.4904x; 1.0003x over previous
"""CrossTransformerLayer on 8 TRN2 NeuronCores — fp8 DoubleRow edition.

Sharding: core c -> (batch b = c//2, q-half = c%2). Each core computes its
512 query rows of its batch end-to-end (k/v over the full 1024-token x1
sequence); no cross-core collectives.

Key device-side ideas (validated on-device in minitest.py):
  * Every large GEMM runs as fp8(e4m3) DoubleRow matmuls: 2x128 contraction
    per instruction at 0.5 cycles/row -> 4x the bf16 PE throughput. Weights
    are host-scaled by 32 (fp8 precision) and rescaled by 1/32 in the
    PSUM->SBUF drains.
  * Attention scores^T[k,q] contract only d=64 per head, too shallow for a
    DoubleRow pair. Instead: qT chunks are parity-padded with zeros (head h
    occupies partitions (h%2)*64..+64, the sibling half is zero), the packed
    kT chunk is slot-repeated with a stride-0 AP, and rhs slot 1 points at an
    all-zero qT plane -> one 256-cycle DR matmul per (head, kt) tile.
  * The attention bias lands in PSUM via fp8 DR "identity preload": lhsT
    [I|0] / [0|I], rhs = a pair of bias^T k-tiles -> 256 cycles per tile.
  * exp(scale*x - 3) on Act engine straight from 2-bank PSUM into fp8 e_t
    (the -3 shift keeps e^x inside e4m3 range; it cancels in the rowsum
    normalization). [v|1] rows are fp8, so the o-matmul is DR as well.
  * All y-transposes go through the DMA XBAR (dma_start_transpose, bf16,
    SP/Act queues) instead of PE+DVE; cheap SBUF->SBUF copies on the gpsimd
    engine convert bf16 y^T -> fp8 for the DR matmuls.
  * PSUM drains are DVE-only (gpsimd has no PSUM port); gpsimd takes the
    SBUF-side work (converts, memsets, rowsum broadcast); Act owns exp/gelu;
    bulk DMA alternates between the SP and Act queues (w2 on the gpsimd
    queue), which all transfer concurrently.

Numerics: x1/x2 in bf16; LN, softmax logits, residuals and the output stay
fp32; fp8 only on matmul operands (y^T, q^T, k^T, v, e^p, weights, bias^T).
"""

import sys

sys.path.insert(0, "/opt/trn_rl_repo")

from contextlib import ExitStack

import numpy as np

import concourse.bass as bass
import concourse.tile as tile
from concourse import bacc, mybir
from concourse.masks import make_identity

F32 = mybir.dt.float32
BF16 = mybir.dt.bfloat16
FP8 = mybir.dt.float8e4
DR = mybir.MatmulPerfMode.DoubleRow

B = 4
S = 1024   # full (k) sequence
Sq = 512   # query rows per core
H = 1024
NH = 16
Dh = 64    # head dim
FF = 4096
P = 128
NKT = S // P    # 8 k-token tiles
NFC = H // P    # 8 feature chunks
NTC = Sq // P   # 4 q-token tiles
NFFC = FF // P  # 32 ff chunks
EPS = 1e-5
SCALE = float(Dh) ** -0.5
WS = 32.0       # host-side fp8 weight scale
IWS = 1.0 / WS
ESHIFT = -3.0   # exp bias shift; cancels in the rowsum normalization
AF = mybir.ActivationFunctionType
OP = mybir.AluOpType


def _pbcast(ap: bass.AP, parts: int) -> bass.AP:
    """[.., N] access pattern -> [parts, .., N] with partition step 0."""
    return bass.AP(
        tensor=ap.tensor,
        offset=ap.offset,
        ap=[[0, parts]] + [list(d) for d in ap.ap],
    )


def _srep(ap: bass.AP, n: int = 2) -> bass.AP:
    """[p, F] AP -> [p, n, F] with slot stride 0 (repeat the same block)."""
    return bass.AP(
        tensor=ap.tensor,
        offset=ap.offset,
        ap=[list(ap.ap[0])] + [[0, n]] + [list(d) for d in ap.ap[1:]],
    )


def build_program(ln_affine=True, with_biases=True):
    nc = bacc.Bacc("TRN2", target_bir_lowering=False, debug=False)

    x1_d = nc.dram_tensor("x1", (S, H), BF16, kind="ExternalInput")
    x2h_d = nc.dram_tensor("x2h", (Sq, H), BF16, kind="ExternalInput")
    biasT_d = nc.dram_tensor("biasT", (NH, S, Sq), FP8, kind="ExternalInput")
    wq_d = nc.dram_tensor("wq", (H, H), FP8, kind="ExternalInput")
    wk_d = nc.dram_tensor("wk", (H, H), FP8, kind="ExternalInput")
    wv_d = nc.dram_tensor("wv", (H, H), FP8, kind="ExternalInput")
    wo_d = nc.dram_tensor("wo", (H, H), FP8, kind="ExternalInput")
    w1h_d = nc.dram_tensor("w1h", (H, FF), FP8, kind="ExternalInput")
    w1l_d = nc.dram_tensor("w1l", (H, FF), FP8, kind="ExternalInput")
    w2h_d = nc.dram_tensor("w2h", (FF, H), FP8, kind="ExternalInput")
    w2l_d = nc.dram_tensor("w2l", (FF, H), FP8, kind="ExternalInput")
    out_d = nc.dram_tensor("out", (Sq, H), F32, kind="ExternalOutput")
    bq_d = nc.dram_tensor("bq_pc", (P, NFC), F32, kind="ExternalInput")
    bk_d = nc.dram_tensor("bk_pc", (P, NFC), F32, kind="ExternalInput")
    bv_d = nc.dram_tensor("bv", (H,), F32, kind="ExternalInput")
    bo_d = nc.dram_tensor("bo", (H,), F32, kind="ExternalInput")
    b1_d = nc.dram_tensor("b1_pc", (P, NFFC), F32, kind="ExternalInput")
    b2_d = nc.dram_tensor("b2", (H,), F32, kind="ExternalInput")
    ln1g_d = nc.dram_tensor("ln1_g", (H,), F32, kind="ExternalInput")
    ln1b_d = nc.dram_tensor("ln1_b", (H,), F32, kind="ExternalInput")
    ln2g_d = nc.dram_tensor("ln2_g", (H,), F32, kind="ExternalInput")
    ln2b_d = nc.dram_tensor("ln2_b", (H,), F32, kind="ExternalInput")
    lnfg_d = nc.dram_tensor("lnf_g", (H,), F32, kind="ExternalInput")
    lnfb_d = nc.dram_tensor("lnf_b", (H,), F32, kind="ExternalInput")

    # Bulk DMA queues: SP (hwdge) and gpsimd (swdge). The Act queue is kept
    # free for compute dispatch: every hwdge DMA costs ~630ns of issuing-queue
    # SEQ time, which starves exp dispatch during attention.
    q_iter = {"i": 0}

    def _dq():
        q_iter["i"] += 1
        return nc.sync if q_iter["i"] % 2 else nc.gpsimd

    # XBAR transposes must use a hwdge queue (SP/Act); they are few.
    t_iter = {"i": 0}

    def _tq():
        t_iter["i"] += 1
        return nc.sync if t_iter["i"] % 2 else nc.scalar

    def _drain(out, ps, bias):
        """PSUM -> SBUF fp8/f32 with the 1/WS weight rescale (+ bias)."""
        if with_biases and bias is not None:
            nc.vector.tensor_scalar(out, ps, IWS, bias, op0=OP.mult,
                                    op1=OP.add)
        else:
            nc.vector.tensor_scalar(out, ps, IWS, None, op0=OP.mult)

    def _layer_norm(pool, y_out, x_in, g_b, b_b, eps_t):
        """y = (x - mean)/sqrt(var+eps) [* g + b] on a [128, H] tile."""
        stats = pool.tile([P, 2, 6], F32, tag="ln_stats", name="stats")
        nc.vector.bn_stats(stats[:, 0, :], x_in[:, 0:512])
        nc.vector.bn_stats(stats[:, 1, :], x_in[:, 512:1024])
        mv = pool.tile([P, 2], F32, tag="ln_mv", name="mv")
        nc.vector.bn_aggr(mv, stats)
        std = pool.tile([P, 1], F32, tag="ln_std", name="std")
        nc.scalar.activation(std, mv[:, 1:2], AF.Sqrt, bias=eps_t, scale=1.0)
        rstd = pool.tile([P, 1], F32, tag="ln_rstd", name="rstd")
        nc.vector.reciprocal(rstd, std)
        nc.vector.tensor_scalar(
            y_out, x_in, mv[:, 0:1], rstd, op0=OP.subtract, op1=OP.mult
        )
        if ln_affine:
            nc.vector.tensor_mul(y_out, y_out, g_b)
            nc.vector.tensor_add(y_out, y_out, b_b)

    with tile.TileContext(nc) as tc, ExitStack() as top:
        persist = top.enter_context(tc.tile_pool(name="persist", bufs=1))
        # [I|0] and [0|I] fp8 stationary tiles for the bias preloads
        ipadE = persist.tile([P, 2, P], FP8, tag="ipadE")
        ipadO = persist.tile([P, 2, P], FP8, tag="ipadO")
        # qT: plane 0 = parity-padded q chunks, plane 1 = zeros (DR slot 1)
        qT = persist.tile([P, 2, NH, Sq], FP8, tag="qT")
        oT = persist.tile([P, NFC, Sq], FP8, tag="oT")
        esh_t = persist.tile([P, 1], F32, tag="esh")
        nc.vector.memset(esh_t, ESHIFT)

        def _setup_consts():
            # Emitted AFTER the input/weight DMA issues: the gpsimd SEQ runs
            # its queue in order, and these memsets must not delay the DMAs.
            nc.gpsimd.memset(ipadE, 0.0)
            make_identity(nc, ipadE[:, 0, :], nomemset=True)
            nc.gpsimd.memset(ipadO, 0.0)
            make_identity(nc, ipadO[:, 1, :], nomemset=True)
            nc.gpsimd.memset(qT[:, 1, :, :], 0.0)
            qT_ev = qT[:, 0, :, :].rearrange("p (hh t) q -> p hh t q", t=2)
            nc.gpsimd.memset(qT_ev[64:128, :, 0, :], 0.0)
            nc.gpsimd.memset(qT_ev[0:64, :, 1, :], 0.0)

        xp = top.enter_context(tc.tile_pool(name="xp", bufs=1))
        x_sb = xp.tile([P, NTC, H], BF16, tag="x")      # attn residual out
        x2_sb = xp.tile([P, NTC, H], BF16, tag="x2")    # x2h kept resident

        # wo + the first w1 group preallocated up top so their DMAs overlap
        # the early phases / attention
        wlate = top.enter_context(tc.tile_pool(name="wlate", bufs=1))
        wo_sb = wlate.tile([P, NFC, H], FP8, tag="wo")
        w1h0_sb = wlate.tile([P, NFC, 1024], FP8, tag="w1h0")
        w1l0_sb = wlate.tile([P, NFC, 1024], FP8, tag="w1l0")

        with (
            tc.tile_pool(name="qkv", bufs=1) as qkvp,
            tc.tile_pool(name="bias_s", bufs=3) as bpool,
            tc.tile_pool(name="expp", bufs=4) as epool,
            tc.tile_pool(name="rin", bufs=2) as rpool,
        ):
            kT = qkvp.tile([P, NFC, S], FP8, tag="kT")
            v_aug = qkvp.tile([P, NKT, NH * 65], FP8, tag="vaug")

            # ---------- Phase 1+2: LN, XBAR transpose, QKV projections ------
            with (
                tc.tile_pool(name="xin", bufs=1) as xinp,
                tc.tile_pool(name="ybf", bufs=1) as ybf,
                tc.tile_pool(name="yT", bufs=1) as yTp,
                tc.tile_pool(name="ph1", bufs=4) as ph1,
                tc.tile_pool(name="ph1w", bufs=3) as ph1w,
                tc.tile_pool(name="ph1c", bufs=1) as ph1c,
                tc.tile_pool(name="wload", bufs=1) as wpool,
                tc.tile_pool(name="vecs", bufs=1) as vecs,
                tc.tile_pool(
                    name="ph2ps", bufs=4, space=bass.MemorySpace.PSUM
                ) as ps2,
            ):
                x1_sb = xinp.tile([P, NKT, H], BF16, tag="x1")
                # Inputs first, all on SP (they gate the LN ladders); weights
                # on the gpsimd queue so they transfer concurrently.
                for t in range(NTC):
                    nc.sync.dma_start(
                        x2_sb[:, t, :], x2h_d[t * P:(t + 1) * P, :]
                    )
                for t in range(NKT):
                    nc.sync.dma_start(
                        x1_sb[:, t, :], x1_d[t * P:(t + 1) * P, :]
                    )

                wq_sb = wpool.tile([P, NFC, H], FP8, tag="wq", name="wq_sb")
                nc.gpsimd.dma_start(
                    wq_sb, wq_d.rearrange("(kc p) f -> p kc f", p=P))
                wk_sb = wpool.tile([P, NFC, H], FP8, tag="wk", name="wk_sb")
                nc.sync.dma_start(
                    wk_sb, wk_d.rearrange("(kc p) f -> p kc f", p=P))
                wv_sb = wpool.tile([P, NFC, H], FP8, tag="wv", name="wv_sb")
                nc.gpsimd.dma_start(
                    wv_sb, wv_d.rearrange("(kc p) f -> p kc f", p=P))
                nc.sync.dma_start(
                    wo_sb, wo_d.rearrange("(kc p) f -> p kc f", p=P))
                nc.gpsimd.dma_start(
                    w1h0_sb, w1h_d[:, 0:1024].rearrange(
                        "(kc p) f -> p kc f", p=P))
                nc.sync.dma_start(
                    w1l0_sb, w1l_d[:, 0:1024].rearrange(
                        "(kc p) f -> p kc f", p=P))
                _setup_consts()
                ones_view = v_aug[:, :, :].rearrange(
                    "p t (h j) -> p t h j", j=65
                )[:, :, :, 64:65]
                nc.gpsimd.memset(ones_view, 1.0)

                y2T_bf = ybf.tile([P, NFC, Sq], BF16, tag="y2Tb")
                # y1T bf16 staging is halved and reused (tokens 0-511, then
                # 512-1023) to cut SBUF peak
                y1T_bf = ybf.tile([P, NFC, Sq], BF16, tag="y1Tb")
                y2T = yTp.tile([P, NFC, Sq], FP8, tag="y2T")
                y1T = yTp.tile([P, NFC, S], FP8, tag="y1T")

                eps_t = ph1c.tile([P, 1], F32, tag="eps")
                nc.vector.memset(eps_t, EPS)
                ln1g_b = ln1b_b = ln2g_b = ln2b_b = None
                if ln_affine:
                    ln1g_b = ph1c.tile([P, H], F32, tag="ln1g")
                    ln1b_b = ph1c.tile([P, H], F32, tag="ln1b")
                    ln2g_b = ph1c.tile([P, H], F32, tag="ln2g")
                    ln2b_b = ph1c.tile([P, H], F32, tag="ln2b")
                    nc.gpsimd.dma_start(ln1g_b, _pbcast(ln1g_d[:], P))
                    nc.gpsimd.dma_start(ln1b_b, _pbcast(ln1b_d[:], P))
                    nc.gpsimd.dma_start(ln2g_b, _pbcast(ln2g_d[:], P))
                    nc.gpsimd.dma_start(ln2b_b, _pbcast(ln2b_d[:], P))

                bq_sb = bk_sb = bv_b = None
                if with_biases:
                    bq_sb = vecs.tile([P, NFC], F32, tag="bq")
                    bk_sb = vecs.tile([P, NFC], F32, tag="bk")
                    bv_b = vecs.tile([P, H], F32, tag="bvb")
                    nc.gpsimd.dma_start(bq_sb, bq_d[:, :])
                    nc.gpsimd.dma_start(bk_sb, bk_d[:, :])
                    nc.gpsimd.dma_start(bv_b, _pbcast(bv_d[:], P))

                # x2h -> y2 -> y2T (XBAR) -> fp8
                for t in range(NTC):
                    yt = ph1w.tile([P, H], BF16, tag="yt", name="yt")
                    _layer_norm(ph1, yt, x2_sb[:, t, :], ln2g_b, ln2b_b, eps_t)
                    _tq().dma_start_transpose(
                        y2T_bf[:, :, t * P:(t + 1) * P], yt[:, :]
                    )
                nc.vector.tensor_copy(y2T, y2T_bf)

                # q^T per fo chunk; parity-split drains into padded qT
                for fo in range(NFC):
                    ps = ps2.tile([P, Sq], F32, tag="mm", name="ps")
                    for g in range(4):
                        nc.tensor.matmul(
                            ps,
                            wq_sb[:, 2 * g:2 * g + 2, fo * P:(fo + 1) * P],
                            y2T[:, 2 * g:2 * g + 2, :],
                            start=(g == 0), stop=(g == 3), perf_mode=DR,
                        )
                    h0, h1 = 2 * fo, 2 * fo + 1
                    nc.scalar.activation(
                        qT[0:64, 0, h0, :], ps[0:64, :], AF.Copy,
                        bias=(bq_sb[0:64, fo:fo + 1] if with_biases else 0.0),
                        scale=IWS,
                    )
                    nc.scalar.activation(
                        qT[64:128, 0, h1, :], ps[64:128, :], AF.Copy,
                        bias=(bq_sb[64:128, fo:fo + 1] if with_biases
                              else 0.0),
                        scale=IWS,
                    )

                # x1 -> y1 -> y1T (XBAR, halved staging) -> fp8
                for half in range(2):
                    for i in range(4):
                        t = 4 * half + i
                        yt = ph1w.tile([P, H], BF16, tag="yt", name="yt")
                        _layer_norm(ph1, yt, x1_sb[:, t, :],
                                    ln1g_b, ln1b_b, eps_t)
                        _tq().dma_start_transpose(
                            y1T_bf[:, :, i * P:(i + 1) * P], yt[:, :]
                        )
                    nc.vector.tensor_copy(
                        y1T[:, :, half * 512:(half + 1) * 512], y1T_bf
                    )

                # v and k projections, emitted in the order attention consumes
                # them (k fo=0,1 first so heads 0-3 can start, then v, then
                # the remaining k chunks). All drains on DVE: it idles during
                # the Act-bound attention phase and absorbs the stragglers.
                def _vproj(t, nt, act):
                    ps = ps2.tile([P, 512], F32, tag="mm", name="ps")
                    for g in range(4):
                        nc.tensor.matmul(
                            ps,
                            y1T[:, 2 * g:2 * g + 2, t * P:(t + 1) * P],
                            wv_sb[:, 2 * g:2 * g + 2,
                                  nt * 512:(nt + 1) * 512],
                            start=(g == 0), stop=(g == 3), perf_mode=DR,
                        )
                    dst = v_aug[
                        :, t, nt * 8 * 65:(nt * 8 + 8) * 65
                    ].rearrange("p (h j) -> p h j", j=65)[:, :, 0:64]
                    psr = ps.rearrange("p (h j) -> p h j", j=64)
                    if with_biases:
                        nc.vector.scalar_tensor_tensor(
                            out=dst, in0=psr, scalar=IWS,
                            in1=bv_b[
                                :, nt * 512:(nt + 1) * 512
                            ].rearrange("p (h j) -> p h j", j=64),
                            op0=OP.mult, op1=OP.add,
                        )
                    elif act:
                        nc.scalar.activation(dst, psr, AF.Copy, scale=IWS)
                    else:
                        nc.vector.tensor_scalar(
                            dst, psr, IWS, None, op0=OP.mult)

                def _kproj(fo, nt, act):
                    ps = ps2.tile([P, 512], F32, tag="mm", name="ps")
                    for g in range(4):
                        nc.tensor.matmul(
                            ps,
                            wk_sb[:, 2 * g:2 * g + 2, fo * P:(fo + 1) * P],
                            y1T[:, 2 * g:2 * g + 2,
                                nt * 512:(nt + 1) * 512],
                            start=(g == 0), stop=(g == 3), perf_mode=DR,
                        )
                    if act:
                        nc.scalar.activation(
                            kT[:, fo, nt * 512:(nt + 1) * 512], ps, AF.Copy,
                            bias=(bk_sb[:, fo:fo + 1] if with_biases
                                  else 0.0),
                            scale=IWS,
                        )
                    else:
                        _drain(kT[:, fo, nt * 512:(nt + 1) * 512], ps,
                               bk_sb[:, fo:fo + 1] if with_biases else None)

                # drains split by attention consumer order: the early chunks
                # (k fo0-3, v nt0) drain on Act right ahead of its exps; the
                # late chunks drain on DVE, overlapping the early exps.
                for fo in (0, 1):
                    for nt in range(2):
                        _kproj(fo, nt, act=True)
                for t in range(NKT):
                    _vproj(t, 0, act=True)
                for fo in (2, 3):
                    for nt in range(2):
                        _kproj(fo, nt, act=True)
                for t in range(NKT):
                    _vproj(t, 1, act=False)
                for fo in range(4, NFC):
                    for nt in range(2):
                        _kproj(fo, nt, act=False)

            # ---------------- Phase 3: attention ----------------
            # Per (head, kt): [I|.] preload puts bias^T in PSUM, one stride-0
            # slot-repeated DR matmul adds k_h.T @ q_h; exp from 2-bank PSUM
            # -> fp8 e_t; [v|1].T @ e_t accumulates o^T + rowsum.
            with (
                tc.tile_pool(
                    name="sc_ps", bufs=2, space=bass.MemorySpace.PSUM
                ) as scps,
                tc.tile_pool(
                    name="o_ps", bufs=2, space=bass.MemorySpace.PSUM
                ) as ops,
            ):
                for h in range(NH):
                    hp = (h % 2) * Dh
                    fc = h // 2
                    o_ps = ops.tile([65, Sq], F32, tag="o", name="o_ps")
                    bt = bpool.tile([P, NKT, Sq], FP8, tag="bt", name="bt")
                    _dq().dma_start(
                        bt, biasT_d[h].rearrange("(kt p) q -> p kt q", p=P)
                    )
                    for g in range(4):
                        scp = scps.tile([P, 2, Sq], F32, tag="sc", name="scp")
                        e_t = epool.tile([P, 2, Sq], FP8, tag="et", name="e_t")
                        for j in range(2):
                            kt = 2 * g + j
                            nc.tensor.matmul(
                                scp[:, j, :],
                                ipadE if j == 0 else ipadO,
                                bt[:, 2 * g:2 * g + 2, :],
                                start=True, stop=False, perf_mode=DR,
                            )
                            nc.tensor.matmul(
                                scp[:, j, :],
                                _srep(kT[:, fc, kt * P:(kt + 1) * P]),
                                qT[:, :, h, :],
                                start=False, stop=True, perf_mode=DR,
                            )
                        nc.scalar.activation(
                            e_t, scp, AF.Exp, bias=esh_t, scale=SCALE
                        )
                        nc.tensor.matmul(
                            o_ps,
                            v_aug[:, 2 * g:2 * g + 2, h * 65:(h + 1) * 65],
                            e_t,
                            start=(g == 0), stop=(g == 3), perf_mode=DR,
                        )
                    rinv = rpool.tile([1, Sq], F32, tag="rinv", name="rinv")
                    nc.vector.reciprocal(rinv, o_ps[64:65, :])
                    rb = rpool.tile([Dh, Sq], F32, tag="rb", name="rb")
                    nc.gpsimd.partition_broadcast(rb, rinv[0:1, :])
                    nc.vector.tensor_tensor(
                        out=oT[hp:hp + Dh, fc, :],
                        in0=o_ps[0:64, :], in1=rb,
                        op=OP.mult,
                    )

        # ---------------- Phase 4: output projection + residual -------------
        with (
            tc.tile_pool(name="ph4c", bufs=1) as ph4c,
            tc.tile_pool(name="ph4ps", bufs=3, space=bass.MemorySpace.PSUM) as ps4,
        ):
            bo_b = None
            if with_biases:
                bo_b = ph4c.tile([P, H], F32, tag="bob")
                nc.gpsimd.dma_start(bo_b, _pbcast(bo_d[:], P))
            for t in range(NTC):
                for half in range(2):
                    ps = ps4.tile([P, 512], F32, tag="mm", name="ps")
                    for g in range(4):
                        nc.tensor.matmul(
                            ps,
                            oT[:, 2 * g:2 * g + 2, t * P:(t + 1) * P],
                            wo_sb[:, 2 * g:2 * g + 2,
                                  half * 512:(half + 1) * 512],
                            start=(g == 0), stop=(g == 3), perf_mode=DR,
                        )
                    xs = x_sb[:, t, half * 512:(half + 1) * 512]
                    nc.vector.scalar_tensor_tensor(
                        out=xs, in0=ps, scalar=IWS,
                        in1=x2_sb[:, t, half * 512:(half + 1) * 512],
                        op0=OP.mult, op1=OP.add,
                    )
                    if with_biases:
                        nc.vector.tensor_add(
                            xs, xs, bo_b[:, half * 512:(half + 1) * 512]
                        )

        # ---------------- Phase 5+6+7: final LN + FFN ----------------
        # FFN precision: weights and activations both carried as fp8 hi+lo
        # planes; each matmul computes hi*hi + lo*hi + hi*lo (the lo*lo term
        # is negligible) -> bf16-class FFN accuracy at fp8-DR speed.
        with (
            tc.tile_pool(name="hTp", bufs=1) as hTp,
            tc.tile_pool(name="y3", bufs=1) as y3p,
        ):
            hT = hTp.tile([P, NFFC, 2, Sq], FP8, tag="hT")   # planes hi/lo
            y3T = y3p.tile([P, NFC, 2, Sq], FP8, tag="y3T")  # planes hi/lo

            with (
                tc.tile_pool(name="ph5", bufs=4) as ph5,
                tc.tile_pool(name="ph5w", bufs=2) as ph5w,
                tc.tile_pool(name="ph5b", bufs=1) as ph5b,
                tc.tile_pool(name="ph5c", bufs=1) as ph5c,
            ):
                y3T_bf = ph5b.tile([P, NFC, Sq], BF16, tag="y3Tb")
                eps_t = ph5c.tile([P, 1], F32, tag="eps")
                nc.vector.memset(eps_t, EPS)
                lnfg_b = lnfb_b = None
                if ln_affine:
                    lnfg_b = ph5c.tile([P, H], F32, tag="lnfg")
                    lnfb_b = ph5c.tile([P, H], F32, tag="lnfb")
                    nc.gpsimd.dma_start(lnfg_b, _pbcast(lnfg_d[:], P))
                    nc.gpsimd.dma_start(lnfb_b, _pbcast(lnfb_d[:], P))
                for t in range(NTC):
                    yt = ph5w.tile([P, H], BF16, tag="yt", name="yt")
                    _layer_norm(ph5, yt, x_sb[:, t, :], lnfg_b, lnfb_b, eps_t)
                    _tq().dma_start_transpose(
                        y3T_bf[:, :, t * P:(t + 1) * P], yt[:, :]
                    )
                nc.vector.tensor_copy(y3T[:, :, 0, :], y3T_bf)
                nc.vector.tensor_tensor(
                    out=y3T[:, :, 1, :], in0=y3T_bf, in1=y3T[:, :, 0, :],
                    op=OP.subtract,
                )

            # FFN1 + gelu -> dual-plane hT, then FFN2 in ONE 8-bank pass
            with (
                tc.tile_pool(name="b1l", bufs=1) as b1pool,
                tc.tile_pool(name="w1s", bufs=2) as w1sp,
                tc.tile_pool(name="w2s", bufs=2) as w2sp,
                tc.tile_pool(name="h32", bufs=1) as h32p,
                tc.tile_pool(name="outp", bufs=1) as outp,
            ):
                b1_sb = b2_b = None
                if with_biases:
                    b1_sb = b1pool.tile([P, NFFC], F32, tag="b1")
                    nc.gpsimd.dma_start(b1_sb, b1_d[:, :])
                    b2_b = b1pool.tile([P, H], F32, tag="b2b")
                    nc.gpsimd.dma_start(b2_b, _pbcast(b2_d[:], P))

                # FFN1 streamed in 4 groups of 8 ffc chunks (group 0 was
                # prefetched into wlate during the early phases)
                GW = 1024  # ff columns per weight group
                f1ctx = tc.tile_pool(name="f1ps", bufs=2,
                                     space=bass.MemorySpace.PSUM)
                f1ps = f1ctx.__enter__()
                for gi in range(FF // GW):
                    if gi == 0:
                        w1h, w1l = w1h0_sb, w1l0_sb
                    else:
                        w1h = w1sp.tile([P, NFC, GW], FP8, tag="w1h",
                                        name="w1h")
                        w1l = w1sp.tile([P, NFC, GW], FP8, tag="w1l",
                                        name="w1l")
                        _dq().dma_start(
                            w1h, w1h_d[:, gi * GW:(gi + 1) * GW].rearrange(
                                "(kc p) f -> p kc f", p=P))
                        _dq().dma_start(
                            w1l, w1l_d[:, gi * GW:(gi + 1) * GW].rearrange(
                                "(kc p) f -> p kc f", p=P))
                    for fp_ in range(4):
                        ps = f1ps.tile([P, 2, Sq], F32, tag="mm", name="ps")
                        for i in range(2):
                            lo = (2 * fp_ + i) * P  # local 128-col block
                            for kc in range(NFC):
                                nc.tensor.matmul(
                                    ps[:, i, :],
                                    _srep(w1h[:, kc, lo:lo + P]),
                                    y3T[:, kc, :, :],
                                    start=(kc == 0), stop=False, perf_mode=DR,
                                )
                            for g in range(4):
                                nc.tensor.matmul(
                                    ps[:, i, :],
                                    w1l[:, 2 * g:2 * g + 2, lo:lo + P],
                                    y3T[:, 2 * g:2 * g + 2, 0, :],
                                    start=False, stop=(g == 3), perf_mode=DR,
                                )
                        h32 = h32p.tile([P, 2, Sq], F32, tag="h32",
                                        name="h32")
                        ffc0 = 8 * gi + 2 * fp_
                        if with_biases:
                            for i in range(2):
                                nc.scalar.activation(
                                    h32[:, i, :], ps[:, i, :], AF.Gelu,
                                    bias=b1_sb[:, ffc0 + i:ffc0 + i + 1],
                                    scale=IWS,
                                )
                        else:
                            nc.scalar.activation(h32, ps, AF.Gelu, scale=IWS)
                        nc.vector.tensor_copy(
                            hT[:, ffc0:ffc0 + 2, 0, :], h32)
                        nc.vector.tensor_tensor(
                            out=hT[:, ffc0:ffc0 + 2, 1, :], in0=h32,
                            in1=hT[:, ffc0:ffc0 + 2, 0, :], op=OP.subtract,
                        )

                # prefetch w2 group 0 while FFN1 still runs
                w2tiles = []
                for gi in range(4):
                    w2h = w2sp.tile([P, 8, H], FP8, tag="w2h", name="w2h")
                    w2l = w2sp.tile([P, 8, H], FP8, tag="w2l", name="w2l")
                    _dq().dma_start(
                        w2h, w2h_d[gi * GW:(gi + 1) * GW, :].rearrange(
                            "(c p) f -> p c f", p=P))
                    _dq().dma_start(
                        w2l, w2l_d[gi * GW:(gi + 1) * GW, :].rearrange(
                            "(c p) f -> p c f", p=P))
                    w2tiles.append((w2h, w2l))

                f1ctx.__exit__(None, None, None)

                # FFN2: single pass over all 4 token tiles (8 PSUM banks);
                # w2 planes streamed once in 4 groups of 8 ff chunks
                with tc.tile_pool(
                    name="f2ps", bufs=1, space=bass.MemorySpace.PSUM
                ) as f2ps:
                    accs = [
                        f2ps.tile([P, H], F32, tag=f"acc{t}",
                                  name=f"acc{t}")
                        for t in range(NTC)
                    ]
                    for gi in range(4):
                        w2h, w2l = w2tiles[gi]
                        for t in range(NTC):
                            for c in range(8):
                                ffc = 8 * gi + c
                                for nt in range(2):
                                    nc.tensor.matmul(
                                        accs[t][:, nt * 512:(nt + 1) * 512],
                                        hT[:, ffc, :, t * P:(t + 1) * P],
                                        _srep(w2h[:, c,
                                                  nt * 512:(nt + 1) * 512]),
                                        start=(ffc == 0), stop=False,
                                        perf_mode=DR,
                                    )
                            for c2 in range(4):
                                ffp = 8 * gi + 2 * c2
                                for nt in range(2):
                                    nc.tensor.matmul(
                                        accs[t][:, nt * 512:(nt + 1) * 512],
                                        hT[:, ffp:ffp + 2, 0,
                                           t * P:(t + 1) * P],
                                        w2l[:, 2 * c2:2 * c2 + 2,
                                            nt * 512:(nt + 1) * 512],
                                        start=False,
                                        stop=(gi == 3 and c2 == 3),
                                        perf_mode=DR,
                                    )
                    for t in range(NTC):
                        ot = outp.tile([P, H], F32, tag="ot", name="ot")
                        nc.vector.scalar_tensor_tensor(
                            out=ot, in0=accs[t], scalar=IWS,
                            in1=x_sb[:, t, :], op0=OP.mult, op1=OP.add,
                        )
                        if with_biases:
                            nc.vector.tensor_add(ot, ot, b2_b)
                        _dq().dma_start(out_d[t * P:(t + 1) * P, :], ot)
            hTctx.__exit__(None, None, None)

    nc.compile()
    return nc


_CACHE: dict = {}


def _get_program(ln_affine=True, with_biases=True):
    key = (ln_affine, with_biases)
    if key not in _CACHE:
        _CACHE[key] = build_program(
            ln_affine=ln_affine, with_biases=with_biases
        )
    return _CACHE[key]


def _detect_fast_flags(inputs):
    ones = lambda k: bool(np.all(np.asarray(inputs[k]) == 1.0))
    zeros = lambda k: bool(np.all(np.asarray(inputs[k]) == 0.0))
    ln_affine = not (
        ones("ln1_g") and ones("ln2_g") and ones("lnf_g")
        and zeros("ln1_b") and zeros("ln2_b") and zeros("lnf_b")
    )
    with_biases = not (
        zeros("bq") and zeros("bk") and zeros("bv") and zeros("bo")
        and zeros("b1") and zeros("b2")
    )
    return ln_affine, with_biases


def _make_in_maps(inputs: dict) -> list[dict]:
    import ml_dtypes

    fp8 = ml_dtypes.float8_e4m3
    bf16 = ml_dtypes.bfloat16
    f32 = lambda a: np.ascontiguousarray(np.asarray(a, dtype=np.float32))
    w8 = lambda a: np.ascontiguousarray(
        (np.asarray(a, dtype=np.float32) * WS).astype(fp8)
    )

    def w8planes(a):
        ws = np.asarray(a, dtype=np.float32) * WS
        hi = ws.astype(fp8)
        lo = (ws - hi.astype(np.float32)).astype(fp8)
        return np.ascontiguousarray(hi), np.ascontiguousarray(lo)

    x1 = np.asarray(inputs["x1"], dtype=np.float32)
    x2 = np.asarray(inputs["x2"], dtype=np.float32)
    attn_bias = np.asarray(inputs["attn_bias"], dtype=np.float32)
    w1h, w1l = w8planes(inputs["w1"])
    w2h, w2l = w8planes(inputs["w2"])
    shared = {
        "wq": w8(inputs["wq"]),
        "wk": w8(inputs["wk"]),
        "wv": w8(inputs["wv"]),
        "wo": w8(inputs["wo"]),
        "w1h": w1h, "w1l": w1l,
        "w2h": w2h, "w2l": w2l,
        "bq_pc": f32(np.asarray(inputs["bq"]).reshape(NFC, P).T),
        "bk_pc": f32(np.asarray(inputs["bk"]).reshape(NFC, P).T),
        "bv": f32(inputs["bv"]),
        "bo": f32(inputs["bo"]),
        "b1_pc": f32(np.asarray(inputs["b1"]).reshape(NFFC, P).T),
        "b2": f32(inputs["b2"]),
        "ln1_g": f32(inputs["ln1_g"]),
        "ln1_b": f32(inputs["ln1_b"]),
        "ln2_g": f32(inputs["ln2_g"]),
        "ln2_b": f32(inputs["ln2_b"]),
        "lnf_g": f32(inputs["lnf_g"]),
        "lnf_b": f32(inputs["lnf_b"]),
    }
    in_maps = []
    for c in range(8):
        b, half = c // 2, c % 2
        q0 = half * Sq
        in_maps.append(
            {
                "x1": np.ascontiguousarray(x1[b].astype(bf16)),
                "x2h": np.ascontiguousarray(x2[b, q0:q0 + Sq].astype(bf16)),
                "biasT": np.ascontiguousarray(
                    (attn_bias[b, :, q0:q0 + Sq, :].transpose(0, 2, 1)
                     * (1.0 / SCALE)).astype(fp8)
                ),
                **shared,
            }
        )
    return in_maps


def _assemble(results: list[dict]) -> np.ndarray:
    out = np.empty((B, S, H), np.float32)
    for c in range(8):
        b, half = c // 2, c % 2
        out[b, half * Sq:(half + 1) * Sq] = results[c]["out"]
    return out


def run(inputs: dict, **run_kwargs):
    from concourse.bass_utils import run_bass_kernel_spmd

    ln_affine, with_biases = _detect_fast_flags(inputs)
    nc = _get_program(ln_affine=ln_affine, with_biases=with_biases)
    in_maps = _make_in_maps(inputs)
    res = run_bass_kernel_spmd(nc, in_maps, core_ids=list(range(8)), **run_kwargs)
    return _assemble(res.results), res


def kernel(**inputs) -> np.ndarray:
    out, _ = run(inputs)
    return out


# revision 41
# speedup vs baseline: 1.5069x; 1.0111x over previous
"""CrossTransformerLayer on 8 TRN2 NeuronCores — fp8 DoubleRow edition.

Sharding: core c -> (batch b = c//2, q-half = c%2). Each core computes its
512 query rows of its batch end-to-end (k/v over the full 1024-token x1
sequence); no cross-core collectives.

Key device-side ideas (validated on-device in minitest.py):
  * Every large GEMM runs as fp8(e4m3) DoubleRow matmuls: 2x128 contraction
    per instruction at 0.5 cycles/row -> 4x the bf16 PE throughput. Weights
    are host-scaled by 32 (fp8 precision) and rescaled by 1/32 in the
    PSUM->SBUF drains.
  * Attention scores^T[k,q] contract only d=64 per head, too shallow for a
    DoubleRow pair. Instead: qT chunks are parity-padded with zeros (head h
    occupies partitions (h%2)*64..+64, the sibling half is zero), the packed
    kT chunk is slot-repeated with a stride-0 AP, and rhs slot 1 points at an
    all-zero qT plane -> one 256-cycle DR matmul per (head, kt) tile.
  * The attention bias lands in PSUM via fp8 DR "identity preload": lhsT
    [I|0] / [0|I], rhs = a pair of bias^T k-tiles -> 256 cycles per tile.
  * exp(scale*x - 3) on Act engine straight from 2-bank PSUM into fp8 e_t
    (the -3 shift keeps e^x inside e4m3 range; it cancels in the rowsum
    normalization). [v|1] rows are fp8, so the o-matmul is DR as well.
  * All y-transposes go through the DMA XBAR (dma_start_transpose, bf16,
    SP/Act queues) instead of PE+DVE; cheap SBUF->SBUF copies on the gpsimd
    engine convert bf16 y^T -> fp8 for the DR matmuls.
  * PSUM drains are DVE-only (gpsimd has no PSUM port); gpsimd takes the
    SBUF-side work (converts, memsets, rowsum broadcast); Act owns exp/gelu;
    bulk DMA alternates between the SP and Act queues (w2 on the gpsimd
    queue), which all transfer concurrently.

Numerics: x1/x2 in bf16; LN, softmax logits, residuals and the output stay
fp32; fp8 only on matmul operands (y^T, q^T, k^T, v, e^p, weights, bias^T).
"""

import sys

sys.path.insert(0, "/opt/trn_rl_repo")

from contextlib import ExitStack

import numpy as np

import concourse.bass as bass
import concourse.tile as tile
from concourse import bacc, mybir
from concourse.masks import make_identity

F32 = mybir.dt.float32
BF16 = mybir.dt.bfloat16
FP8 = mybir.dt.float8e4
DR = mybir.MatmulPerfMode.DoubleRow

B = 4
S = 1024   # full (k) sequence
Sq = 512   # query rows per core
H = 1024
NH = 16
Dh = 64    # head dim
FF = 4096
P = 128
NKT = S // P    # 8 k-token tiles
NFC = H // P    # 8 feature chunks
NTC = Sq // P   # 4 q-token tiles
NFFC = FF // P  # 32 ff chunks
EPS = 1e-5
SCALE = float(Dh) ** -0.5
WS = 32.0       # host-side fp8 weight scale
IWS = 1.0 / WS
ESHIFT = -3.0   # exp bias shift; cancels in the rowsum normalization
AF = mybir.ActivationFunctionType
OP = mybir.AluOpType


def _pbcast(ap: bass.AP, parts: int) -> bass.AP:
    """[.., N] access pattern -> [parts, .., N] with partition step 0."""
    return bass.AP(
        tensor=ap.tensor,
        offset=ap.offset,
        ap=[[0, parts]] + [list(d) for d in ap.ap],
    )


def _srep(ap: bass.AP, n: int = 2) -> bass.AP:
    """[p, F] AP -> [p, n, F] with slot stride 0 (repeat the same block)."""
    return bass.AP(
        tensor=ap.tensor,
        offset=ap.offset,
        ap=[list(ap.ap[0])] + [[0, n]] + [list(d) for d in ap.ap[1:]],
    )


def build_program(ln_affine=True, with_biases=True):
    nc = bacc.Bacc("TRN2", target_bir_lowering=False, debug=False)

    x1_d = nc.dram_tensor("x1", (S, H), BF16, kind="ExternalInput")
    x2h_d = nc.dram_tensor("x2h", (Sq, H), BF16, kind="ExternalInput")
    biasT_d = nc.dram_tensor("biasT", (NH, S, Sq), FP8, kind="ExternalInput")
    wq_d = nc.dram_tensor("wq", (H, H), FP8, kind="ExternalInput")
    wk_d = nc.dram_tensor("wk", (H, H), FP8, kind="ExternalInput")
    wv_d = nc.dram_tensor("wv", (H, H), FP8, kind="ExternalInput")
    wo_d = nc.dram_tensor("wo", (H, H), FP8, kind="ExternalInput")
    w1h_d = nc.dram_tensor("w1h", (H, FF), FP8, kind="ExternalInput")
    w1l_d = nc.dram_tensor("w1l", (H, FF), FP8, kind="ExternalInput")
    w2h_d = nc.dram_tensor("w2h", (FF, H), FP8, kind="ExternalInput")
    w2l_d = nc.dram_tensor("w2l", (FF, H), FP8, kind="ExternalInput")
    out_d = nc.dram_tensor("out", (Sq, H), F32, kind="ExternalOutput")
    bq_d = nc.dram_tensor("bq_pc", (P, NFC), F32, kind="ExternalInput")
    bk_d = nc.dram_tensor("bk_pc", (P, NFC), F32, kind="ExternalInput")
    bv_d = nc.dram_tensor("bv", (H,), F32, kind="ExternalInput")
    bo_d = nc.dram_tensor("bo", (H,), F32, kind="ExternalInput")
    b1_d = nc.dram_tensor("b1_pc", (P, NFFC), F32, kind="ExternalInput")
    b2_d = nc.dram_tensor("b2", (H,), F32, kind="ExternalInput")
    ln1g_d = nc.dram_tensor("ln1_g", (H,), F32, kind="ExternalInput")
    ln1b_d = nc.dram_tensor("ln1_b", (H,), F32, kind="ExternalInput")
    ln2g_d = nc.dram_tensor("ln2_g", (H,), F32, kind="ExternalInput")
    ln2b_d = nc.dram_tensor("ln2_b", (H,), F32, kind="ExternalInput")
    lnfg_d = nc.dram_tensor("lnf_g", (H,), F32, kind="ExternalInput")
    lnfb_d = nc.dram_tensor("lnf_b", (H,), F32, kind="ExternalInput")

    # Bulk DMA queues: SP (hwdge) and gpsimd (swdge). The Act queue is kept
    # free for compute dispatch: every hwdge DMA costs ~630ns of issuing-queue
    # SEQ time, which starves exp dispatch during attention.
    q_iter = {"i": 0}

    def _dq():
        q_iter["i"] += 1
        return nc.sync if q_iter["i"] % 2 else nc.gpsimd

    # XBAR transposes must use a hwdge queue (SP/Act); they are few.
    t_iter = {"i": 0}

    def _tq():
        t_iter["i"] += 1
        return nc.sync if t_iter["i"] % 2 else nc.scalar

    def _drain(out, ps, bias):
        """PSUM -> SBUF fp8/f32 with the 1/WS weight rescale (+ bias)."""
        if with_biases and bias is not None:
            nc.vector.tensor_scalar(out, ps, IWS, bias, op0=OP.mult,
                                    op1=OP.add)
        else:
            nc.vector.tensor_scalar(out, ps, IWS, None, op0=OP.mult)

    def _layer_norm(pool, y_out, x_in, g_b, b_b, eps_t):
        """y = (x - mean)/sqrt(var+eps) [* g + b] on a [128, H] tile."""
        stats = pool.tile([P, 2, 6], F32, tag="ln_stats", name="stats")
        nc.vector.bn_stats(stats[:, 0, :], x_in[:, 0:512])
        nc.vector.bn_stats(stats[:, 1, :], x_in[:, 512:1024])
        mv = pool.tile([P, 2], F32, tag="ln_mv", name="mv")
        nc.vector.bn_aggr(mv, stats)
        # rstd = exp(-ln(var+eps)/2): Ln and Exp share an act table with the
        # attention exps, so the LN chain never forces a table reload.
        lnv = pool.tile([P, 1], F32, tag="ln_lnv", name="lnv")
        nc.scalar.activation(lnv, mv[:, 1:2], AF.Ln, bias=eps_t, scale=1.0)
        rstd = pool.tile([P, 1], F32, tag="ln_rstd", name="rstd")
        nc.scalar.activation(rstd, lnv, AF.Exp, scale=-0.5)
        nc.vector.tensor_scalar(
            y_out, x_in, mv[:, 0:1], rstd, op0=OP.subtract, op1=OP.mult
        )
        if ln_affine:
            nc.vector.tensor_mul(y_out, y_out, g_b)
            nc.vector.tensor_add(y_out, y_out, b_b)

    with tile.TileContext(nc) as tc, ExitStack() as top:
        persist = top.enter_context(tc.tile_pool(name="persist", bufs=1))
        # [I|0] and [0|I] fp8 stationary tiles for the bias preloads
        ipadE = persist.tile([P, 2, P], FP8, tag="ipadE")
        ipadO = persist.tile([P, 2, P], FP8, tag="ipadO")
        # qT: plane 0 = parity-padded q chunks, plane 1 = zeros (DR slot 1)
        qT = persist.tile([P, 2, NH, Sq], FP8, tag="qT")
        oT = persist.tile([P, NFC, Sq], FP8, tag="oT")
        esh_t = persist.tile([P, 1], F32, tag="esh")
        nc.vector.memset(esh_t, ESHIFT)

        def _setup_consts():
            # Emitted AFTER the input/weight DMA issues: the gpsimd SEQ runs
            # its queue in order, and these memsets must not delay the DMAs.
            nc.gpsimd.memset(ipadE, 0.0)
            make_identity(nc, ipadE[:, 0, :], nomemset=True)
            nc.gpsimd.memset(ipadO, 0.0)
            make_identity(nc, ipadO[:, 1, :], nomemset=True)
            nc.gpsimd.memset(qT[:, 1, :, :], 0.0)
            qT_ev = qT[:, 0, :, :].rearrange("p (hh t) q -> p hh t q", t=2)
            nc.gpsimd.memset(qT_ev[64:128, :, 0, :], 0.0)
            nc.gpsimd.memset(qT_ev[0:64, :, 1, :], 0.0)

        xp = top.enter_context(tc.tile_pool(name="xp", bufs=1))
        x_sb = xp.tile([P, NTC, H], BF16, tag="x")      # attn residual out
        x2_sb = xp.tile([P, NTC, H], BF16, tag="x2")    # x2h kept resident

        # wo + the first w1 group preallocated up top so their DMAs overlap
        # the early phases / attention
        wlate = top.enter_context(tc.tile_pool(name="wlate", bufs=1))
        wo_sb = wlate.tile([P, NFC, H], FP8, tag="wo")
        w1h0_sb = wlate.tile([P, NFC, 1024], FP8, tag="w1h0")
        w1l0_sb = wlate.tile([P, NFC, 1024], FP8, tag="w1l0")

        with (
            tc.tile_pool(name="qkv", bufs=1) as qkvp,
            tc.tile_pool(name="bias_s", bufs=3) as bpool,
            tc.tile_pool(name="expp", bufs=4) as epool,
            tc.tile_pool(name="rin", bufs=2) as rpool,
        ):
            kT = qkvp.tile([P, NFC, S], FP8, tag="kT")
            v_aug = qkvp.tile([P, NKT, NH * 65], FP8, tag="vaug")

            # ---------- Phase 1+2: LN, XBAR transpose, QKV projections ------
            with (
                tc.tile_pool(name="xin", bufs=1) as xinp,
                tc.tile_pool(name="ybf", bufs=1) as ybf,
                tc.tile_pool(name="yT", bufs=1) as yTp,
                tc.tile_pool(name="ph1", bufs=4) as ph1,
                tc.tile_pool(name="ph1w", bufs=3) as ph1w,
                tc.tile_pool(name="ph1c", bufs=1) as ph1c,
                tc.tile_pool(name="wload", bufs=1) as wpool,
                tc.tile_pool(name="vecs", bufs=1) as vecs,
                tc.tile_pool(
                    name="ph2ps", bufs=4, space=bass.MemorySpace.PSUM
                ) as ps2,
            ):
                x1_sb = xinp.tile([P, NKT, H], BF16, tag="x1")
                # Inputs first, all on SP (they gate the LN ladders); weights
                # on the gpsimd queue so they transfer concurrently.
                for t in range(NTC):
                    nc.sync.dma_start(
                        x2_sb[:, t, :], x2h_d[t * P:(t + 1) * P, :]
                    )
                for t in range(NKT):
                    nc.sync.dma_start(
                        x1_sb[:, t, :], x1_d[t * P:(t + 1) * P, :]
                    )

                wq_sb = wpool.tile([P, NFC, H], FP8, tag="wq", name="wq_sb")
                nc.gpsimd.dma_start(
                    wq_sb, wq_d.rearrange("(kc p) f -> p kc f", p=P))
                wk_sb = wpool.tile([P, NFC, H], FP8, tag="wk", name="wk_sb")
                nc.sync.dma_start(
                    wk_sb, wk_d.rearrange("(kc p) f -> p kc f", p=P))
                wv_sb = wpool.tile([P, NFC, H], FP8, tag="wv", name="wv_sb")
                nc.gpsimd.dma_start(
                    wv_sb, wv_d.rearrange("(kc p) f -> p kc f", p=P))
                nc.sync.dma_start(
                    wo_sb, wo_d.rearrange("(kc p) f -> p kc f", p=P))
                nc.gpsimd.dma_start(
                    w1h0_sb, w1h_d[:, 0:1024].rearrange(
                        "(kc p) f -> p kc f", p=P))
                nc.sync.dma_start(
                    w1l0_sb, w1l_d[:, 0:1024].rearrange(
                        "(kc p) f -> p kc f", p=P))
                _setup_consts()
                ones_view = v_aug[:, :, :].rearrange(
                    "p t (h j) -> p t h j", j=65
                )[:, :, :, 64:65]
                nc.gpsimd.memset(ones_view, 1.0)

                y2T_bf = ybf.tile([P, NFC, Sq], BF16, tag="y2Tb")
                # y1T bf16 staging is halved and reused (tokens 0-511, then
                # 512-1023) to cut SBUF peak
                y1T_bf = ybf.tile([P, NFC, Sq], BF16, tag="y1Tb")
                y2T = yTp.tile([P, NFC, Sq], FP8, tag="y2T")
                y1T = yTp.tile([P, NFC, S], FP8, tag="y1T")

                eps_t = ph1c.tile([P, 1], F32, tag="eps")
                nc.vector.memset(eps_t, EPS)
                ln1g_b = ln1b_b = ln2g_b = ln2b_b = None
                if ln_affine:
                    ln1g_b = ph1c.tile([P, H], F32, tag="ln1g")
                    ln1b_b = ph1c.tile([P, H], F32, tag="ln1b")
                    ln2g_b = ph1c.tile([P, H], F32, tag="ln2g")
                    ln2b_b = ph1c.tile([P, H], F32, tag="ln2b")
                    nc.gpsimd.dma_start(ln1g_b, _pbcast(ln1g_d[:], P))
                    nc.gpsimd.dma_start(ln1b_b, _pbcast(ln1b_d[:], P))
                    nc.gpsimd.dma_start(ln2g_b, _pbcast(ln2g_d[:], P))
                    nc.gpsimd.dma_start(ln2b_b, _pbcast(ln2b_d[:], P))

                bq_sb = bk_sb = bv_b = None
                if with_biases:
                    bq_sb = vecs.tile([P, NFC], F32, tag="bq")
                    bk_sb = vecs.tile([P, NFC], F32, tag="bk")
                    bv_b = vecs.tile([P, H], F32, tag="bvb")
                    nc.gpsimd.dma_start(bq_sb, bq_d[:, :])
                    nc.gpsimd.dma_start(bk_sb, bk_d[:, :])
                    nc.gpsimd.dma_start(bv_b, _pbcast(bv_d[:], P))

                # x2h -> y2 -> y2T (XBAR) -> fp8
                for t in range(NTC):
                    yt = ph1w.tile([P, H], BF16, tag="yt", name="yt")
                    _layer_norm(ph1, yt, x2_sb[:, t, :], ln2g_b, ln2b_b, eps_t)
                    _tq().dma_start_transpose(
                        y2T_bf[:, :, t * P:(t + 1) * P], yt[:, :]
                    )
                nc.vector.tensor_copy(y2T, y2T_bf)

                # q^T per fo chunk; parity-split drains into padded qT
                for fo in range(NFC):
                    ps = ps2.tile([P, Sq], F32, tag="mm", name="ps")
                    for g in range(4):
                        nc.tensor.matmul(
                            ps,
                            wq_sb[:, 2 * g:2 * g + 2, fo * P:(fo + 1) * P],
                            y2T[:, 2 * g:2 * g + 2, :],
                            start=(g == 0), stop=(g == 3), perf_mode=DR,
                        )
                    h0, h1 = 2 * fo, 2 * fo + 1
                    nc.scalar.activation(
                        qT[0:64, 0, h0, :], ps[0:64, :], AF.Copy,
                        bias=(bq_sb[0:64, fo:fo + 1] if with_biases else 0.0),
                        scale=IWS,
                    )
                    nc.scalar.activation(
                        qT[64:128, 0, h1, :], ps[64:128, :], AF.Copy,
                        bias=(bq_sb[64:128, fo:fo + 1] if with_biases
                              else 0.0),
                        scale=IWS,
                    )

                # x1 -> y1 -> y1T (XBAR, halved staging) -> fp8
                for half in range(2):
                    for i in range(4):
                        t = 4 * half + i
                        yt = ph1w.tile([P, H], BF16, tag="yt", name="yt")
                        _layer_norm(ph1, yt, x1_sb[:, t, :],
                                    ln1g_b, ln1b_b, eps_t)
                        _tq().dma_start_transpose(
                            y1T_bf[:, :, i * P:(i + 1) * P], yt[:, :]
                        )
                    nc.vector.tensor_copy(
                        y1T[:, :, half * 512:(half + 1) * 512], y1T_bf
                    )

                # v and k projections, emitted in the order attention consumes
                # them (k fo=0,1 first so heads 0-3 can start, then v, then
                # the remaining k chunks). All drains on DVE: it idles during
                # the Act-bound attention phase and absorbs the stragglers.
                def _vproj(t, nt, act):
                    ps = ps2.tile([P, 512], F32, tag="mm", name="ps")
                    for g in range(4):
                        nc.tensor.matmul(
                            ps,
                            y1T[:, 2 * g:2 * g + 2, t * P:(t + 1) * P],
                            wv_sb[:, 2 * g:2 * g + 2,
                                  nt * 512:(nt + 1) * 512],
                            start=(g == 0), stop=(g == 3), perf_mode=DR,
                        )
                    dst = v_aug[
                        :, t, nt * 8 * 65:(nt * 8 + 8) * 65
                    ].rearrange("p (h j) -> p h j", j=65)[:, :, 0:64]
                    psr = ps.rearrange("p (h j) -> p h j", j=64)
                    if with_biases:
                        nc.vector.scalar_tensor_tensor(
                            out=dst, in0=psr, scalar=IWS,
                            in1=bv_b[
                                :, nt * 512:(nt + 1) * 512
                            ].rearrange("p (h j) -> p h j", j=64),
                            op0=OP.mult, op1=OP.add,
                        )
                    elif act:
                        nc.scalar.activation(dst, psr, AF.Copy, scale=IWS)
                    else:
                        nc.vector.tensor_scalar(
                            dst, psr, IWS, None, op0=OP.mult)

                def _kproj(fo, nt, act):
                    ps = ps2.tile([P, 512], F32, tag="mm", name="ps")
                    for g in range(4):
                        nc.tensor.matmul(
                            ps,
                            wk_sb[:, 2 * g:2 * g + 2, fo * P:(fo + 1) * P],
                            y1T[:, 2 * g:2 * g + 2,
                                nt * 512:(nt + 1) * 512],
                            start=(g == 0), stop=(g == 3), perf_mode=DR,
                        )
                    if act:
                        nc.scalar.activation(
                            kT[:, fo, nt * 512:(nt + 1) * 512], ps, AF.Copy,
                            bias=(bk_sb[:, fo:fo + 1] if with_biases
                                  else 0.0),
                            scale=IWS,
                        )
                    else:
                        _drain(kT[:, fo, nt * 512:(nt + 1) * 512], ps,
                               bk_sb[:, fo:fo + 1] if with_biases else None)

                # drains split by attention consumer order: the early chunks
                # (k fo0-3, v nt0) drain on Act right ahead of its exps; the
                # late chunks drain on DVE, overlapping the early exps.
                for fo in (0, 1):
                    for nt in range(2):
                        _kproj(fo, nt, act=True)
                for t in range(NKT):
                    _vproj(t, 0, act=True)
                for fo in (2, 3):
                    for nt in range(2):
                        _kproj(fo, nt, act=True)
                for t in range(NKT):
                    _vproj(t, 1, act=False)
                for fo in range(4, NFC):
                    for nt in range(2):
                        _kproj(fo, nt, act=False)

            # ---------------- Phase 3: attention ----------------
            # Per (head, kt): [I|.] preload puts bias^T in PSUM, one stride-0
            # slot-repeated DR matmul adds k_h.T @ q_h; exp from 2-bank PSUM
            # -> fp8 e_t; [v|1].T @ e_t accumulates o^T + rowsum.
            with (
                tc.tile_pool(
                    name="sc_ps", bufs=2, space=bass.MemorySpace.PSUM
                ) as scps,
                tc.tile_pool(
                    name="o_ps", bufs=2, space=bass.MemorySpace.PSUM
                ) as ops,
            ):
                for h in range(NH):
                    hp = (h % 2) * Dh
                    fc = h // 2
                    o_ps = ops.tile([65, Sq], F32, tag="o", name="o_ps")
                    bt = bpool.tile([P, NKT, Sq], FP8, tag="bt", name="bt")
                    _dq().dma_start(
                        bt, biasT_d[h].rearrange("(kt p) q -> p kt q", p=P)
                    )
                    for g in range(4):
                        scp = scps.tile([P, 2, Sq], F32, tag="sc", name="scp")
                        e_t = epool.tile([P, 2, Sq], FP8, tag="et", name="e_t")
                        for j in range(2):
                            kt = 2 * g + j
                            nc.tensor.matmul(
                                scp[:, j, :],
                                ipadE if j == 0 else ipadO,
                                bt[:, 2 * g:2 * g + 2, :],
                                start=True, stop=False, perf_mode=DR,
                            )
                            nc.tensor.matmul(
                                scp[:, j, :],
                                _srep(kT[:, fc, kt * P:(kt + 1) * P]),
                                qT[:, :, h, :],
                                start=False, stop=True, perf_mode=DR,
                            )
                        nc.scalar.activation(
                            e_t, scp, AF.Exp, bias=esh_t, scale=SCALE
                        )
                        nc.tensor.matmul(
                            o_ps,
                            v_aug[:, 2 * g:2 * g + 2, h * 65:(h + 1) * 65],
                            e_t,
                            start=(g == 0), stop=(g == 3), perf_mode=DR,
                        )
                    rinv = rpool.tile([1, Sq], F32, tag="rinv", name="rinv")
                    nc.vector.reciprocal(rinv, o_ps[64:65, :])
                    rb = rpool.tile([Dh, Sq], F32, tag="rb", name="rb")
                    nc.gpsimd.partition_broadcast(rb, rinv[0:1, :])
                    nc.vector.tensor_tensor(
                        out=oT[hp:hp + Dh, fc, :],
                        in0=o_ps[0:64, :], in1=rb,
                        op=OP.mult,
                    )

        # ---------------- Phase 4: output projection + residual -------------
        with (
            tc.tile_pool(name="ph4c", bufs=1) as ph4c,
            tc.tile_pool(name="ph4ps", bufs=3, space=bass.MemorySpace.PSUM) as ps4,
        ):
            bo_b = None
            if with_biases:
                bo_b = ph4c.tile([P, H], F32, tag="bob")
                nc.gpsimd.dma_start(bo_b, _pbcast(bo_d[:], P))
            for t in range(NTC):
                for half in range(2):
                    ps = ps4.tile([P, 512], F32, tag="mm", name="ps")
                    for g in range(4):
                        nc.tensor.matmul(
                            ps,
                            oT[:, 2 * g:2 * g + 2, t * P:(t + 1) * P],
                            wo_sb[:, 2 * g:2 * g + 2,
                                  half * 512:(half + 1) * 512],
                            start=(g == 0), stop=(g == 3), perf_mode=DR,
                        )
                    xs = x_sb[:, t, half * 512:(half + 1) * 512]
                    nc.vector.scalar_tensor_tensor(
                        out=xs, in0=ps, scalar=IWS,
                        in1=x2_sb[:, t, half * 512:(half + 1) * 512],
                        op0=OP.mult, op1=OP.add,
                    )
                    if with_biases:
                        nc.vector.tensor_add(
                            xs, xs, bo_b[:, half * 512:(half + 1) * 512]
                        )

        # ---------------- Phase 5+6+7: final LN + FFN ----------------
        # FFN precision: weights and activations both carried as fp8 hi+lo
        # planes; each matmul computes hi*hi + lo*hi + hi*lo (the lo*lo term
        # is negligible) -> bf16-class FFN accuracy at fp8-DR speed.
        with (
            tc.tile_pool(name="hTp", bufs=1) as hTp,
            tc.tile_pool(name="y3", bufs=1) as y3p,
        ):
            hT = hTp.tile([P, NFFC, 2, Sq], FP8, tag="hT")   # planes hi/lo
            y3T = y3p.tile([P, NFC, 2, Sq], FP8, tag="y3T")  # planes hi/lo

            with (
                tc.tile_pool(name="ph5", bufs=4) as ph5,
                tc.tile_pool(name="ph5w", bufs=2) as ph5w,
                tc.tile_pool(name="ph5b", bufs=1) as ph5b,
                tc.tile_pool(name="ph5c", bufs=1) as ph5c,
            ):
                y3T_bf = ph5b.tile([P, NFC, Sq], BF16, tag="y3Tb")
                eps_t = ph5c.tile([P, 1], F32, tag="eps")
                nc.vector.memset(eps_t, EPS)
                lnfg_b = lnfb_b = None
                if ln_affine:
                    lnfg_b = ph5c.tile([P, H], F32, tag="lnfg")
                    lnfb_b = ph5c.tile([P, H], F32, tag="lnfb")
                    nc.gpsimd.dma_start(lnfg_b, _pbcast(lnfg_d[:], P))
                    nc.gpsimd.dma_start(lnfb_b, _pbcast(lnfb_d[:], P))
                for t in range(NTC):
                    yt = ph5w.tile([P, H], BF16, tag="yt", name="yt")
                    _layer_norm(ph5, yt, x_sb[:, t, :], lnfg_b, lnfb_b, eps_t)
                    _tq().dma_start_transpose(
                        y3T_bf[:, :, t * P:(t + 1) * P], yt[:, :]
                    )
                nc.vector.tensor_copy(y3T[:, :, 0, :], y3T_bf)
                nc.vector.tensor_tensor(
                    out=y3T[:, :, 1, :], in0=y3T_bf, in1=y3T[:, :, 0, :],
                    op=OP.subtract,
                )

            # FFN1 + gelu -> dual-plane hT, then FFN2 in ONE 8-bank pass
            with (
                tc.tile_pool(name="b1l", bufs=1) as b1pool,
                tc.tile_pool(name="w1s", bufs=2) as w1sp,
                tc.tile_pool(name="w2s", bufs=2) as w2sp,
                tc.tile_pool(name="h32", bufs=1) as h32p,
                tc.tile_pool(name="outp", bufs=1) as outp,
            ):
                b1_sb = b2_b = None
                if with_biases:
                    b1_sb = b1pool.tile([P, NFFC], F32, tag="b1")
                    nc.gpsimd.dma_start(b1_sb, b1_d[:, :])
                    b2_b = b1pool.tile([P, H], F32, tag="b2b")
                    nc.gpsimd.dma_start(b2_b, _pbcast(b2_d[:], P))

                # FFN1 streamed in 4 groups of 8 ffc chunks (group 0 was
                # prefetched into wlate during the early phases)
                GW = 1024  # ff columns per weight group
                f1ctx = tc.tile_pool(name="f1ps", bufs=2,
                                     space=bass.MemorySpace.PSUM)
                f1ps = f1ctx.__enter__()
                for gi in range(FF // GW):
                    if gi == 0:
                        w1h, w1l = w1h0_sb, w1l0_sb
                    else:
                        w1h = w1sp.tile([P, NFC, GW], FP8, tag="w1h",
                                        name="w1h")
                        w1l = w1sp.tile([P, NFC, GW], FP8, tag="w1l",
                                        name="w1l")
                        _dq().dma_start(
                            w1h, w1h_d[:, gi * GW:(gi + 1) * GW].rearrange(
                                "(kc p) f -> p kc f", p=P))
                        _dq().dma_start(
                            w1l, w1l_d[:, gi * GW:(gi + 1) * GW].rearrange(
                                "(kc p) f -> p kc f", p=P))
                    for fp_ in range(4):
                        ps = f1ps.tile([P, 2, Sq], F32, tag="mm", name="ps")
                        for i in range(2):
                            lo = (2 * fp_ + i) * P  # local 128-col block
                            for kc in range(NFC):
                                nc.tensor.matmul(
                                    ps[:, i, :],
                                    _srep(w1h[:, kc, lo:lo + P]),
                                    y3T[:, kc, :, :],
                                    start=(kc == 0), stop=False, perf_mode=DR,
                                )
                            for g in range(4):
                                nc.tensor.matmul(
                                    ps[:, i, :],
                                    w1l[:, 2 * g:2 * g + 2, lo:lo + P],
                                    y3T[:, 2 * g:2 * g + 2, 0, :],
                                    start=False, stop=(g == 3), perf_mode=DR,
                                )
                        h32 = h32p.tile([P, 2, Sq], F32, tag="h32",
                                        name="h32")
                        ffc0 = 8 * gi + 2 * fp_
                        if with_biases:
                            for i in range(2):
                                nc.scalar.activation(
                                    h32[:, i, :], ps[:, i, :], AF.Gelu,
                                    bias=b1_sb[:, ffc0 + i:ffc0 + i + 1],
                                    scale=IWS,
                                )
                        else:
                            nc.scalar.activation(h32, ps, AF.Gelu, scale=IWS)
                        nc.vector.tensor_copy(
                            hT[:, ffc0:ffc0 + 2, 0, :], h32)
                        nc.vector.tensor_tensor(
                            out=hT[:, ffc0:ffc0 + 2, 1, :], in0=h32,
                            in1=hT[:, ffc0:ffc0 + 2, 0, :], op=OP.subtract,
                        )

                # prefetch w2 group 0 while FFN1 still runs
                w2tiles = []
                for gi in range(4):
                    w2h = w2sp.tile([P, 8, H], FP8, tag="w2h", name="w2h")
                    w2l = w2sp.tile([P, 8, H], FP8, tag="w2l", name="w2l")
                    _dq().dma_start(
                        w2h, w2h_d[gi * GW:(gi + 1) * GW, :].rearrange(
                            "(c p) f -> p c f", p=P))
                    _dq().dma_start(
                        w2l, w2l_d[gi * GW:(gi + 1) * GW, :].rearrange(
                            "(c p) f -> p c f", p=P))
                    w2tiles.append((w2h, w2l))

                f1ctx.__exit__(None, None, None)

                # FFN2: single pass over all 4 token tiles (8 PSUM banks);
                # w2 planes streamed once in 4 groups of 8 ff chunks
                with tc.tile_pool(
                    name="f2ps", bufs=1, space=bass.MemorySpace.PSUM
                ) as f2ps:
                    accs = [
                        f2ps.tile([P, H], F32, tag=f"acc{t}",
                                  name=f"acc{t}")
                        for t in range(NTC)
                    ]
                    for gi in range(4):
                        w2h, w2l = w2tiles[gi]
                        for t in range(NTC):
                            for c in range(8):
                                ffc = 8 * gi + c
                                for nt in range(2):
                                    nc.tensor.matmul(
                                        accs[t][:, nt * 512:(nt + 1) * 512],
                                        hT[:, ffc, :, t * P:(t + 1) * P],
                                        _srep(w2h[:, c,
                                                  nt * 512:(nt + 1) * 512]),
                                        start=(ffc == 0), stop=False,
                                        perf_mode=DR,
                                    )
                            for c2 in range(4):
                                ffp = 8 * gi + 2 * c2
                                for nt in range(2):
                                    nc.tensor.matmul(
                                        accs[t][:, nt * 512:(nt + 1) * 512],
                                        hT[:, ffp:ffp + 2, 0,
                                           t * P:(t + 1) * P],
                                        w2l[:, 2 * c2:2 * c2 + 2,
                                            nt * 512:(nt + 1) * 512],
                                        start=False,
                                        stop=(gi == 3 and c2 == 3),
                                        perf_mode=DR,
                                    )
                    for t in range(NTC):
                        ot = outp.tile([P, H], F32, tag="ot", name="ot")
                        nc.vector.scalar_tensor_tensor(
                            out=ot, in0=accs[t], scalar=IWS,
                            in1=x_sb[:, t, :], op0=OP.mult, op1=OP.add,
                        )
                        if with_biases:
                            nc.vector.tensor_add(ot, ot, b2_b)
                        _dq().dma_start(out_d[t * P:(t + 1) * P, :], ot)
            hTctx.__exit__(None, None, None)

    nc.compile()
    return nc


_CACHE: dict = {}


def _get_program(ln_affine=True, with_biases=True):
    key = (ln_affine, with_biases)
    if key not in _CACHE:
        _CACHE[key] = build_program(
            ln_affine=ln_affine, with_biases=with_biases
        )
    return _CACHE[key]


def _detect_fast_flags(inputs):
    ones = lambda k: bool(np.all(np.asarray(inputs[k]) == 1.0))
    zeros = lambda k: bool(np.all(np.asarray(inputs[k]) == 0.0))
    ln_affine = not (
        ones("ln1_g") and ones("ln2_g") and ones("lnf_g")
        and zeros("ln1_b") and zeros("ln2_b") and zeros("lnf_b")
    )
    with_biases = not (
        zeros("bq") and zeros("bk") and zeros("bv") and zeros("bo")
        and zeros("b1") and zeros("b2")
    )
    return ln_affine, with_biases


def _make_in_maps(inputs: dict) -> list[dict]:
    import ml_dtypes

    fp8 = ml_dtypes.float8_e4m3
    bf16 = ml_dtypes.bfloat16
    f32 = lambda a: np.ascontiguousarray(np.asarray(a, dtype=np.float32))
    w8 = lambda a: np.ascontiguousarray(
        (np.asarray(a, dtype=np.float32) * WS).astype(fp8)
    )

    def w8planes(a):
        ws = np.asarray(a, dtype=np.float32) * WS
        hi = ws.astype(fp8)
        lo = (ws - hi.astype(np.float32)).astype(fp8)
        return np.ascontiguousarray(hi), np.ascontiguousarray(lo)

    x1 = np.asarray(inputs["x1"], dtype=np.float32)
    x2 = np.asarray(inputs["x2"], dtype=np.float32)
    attn_bias = np.asarray(inputs["attn_bias"], dtype=np.float32)
    w1h, w1l = w8planes(inputs["w1"])
    w2h, w2l = w8planes(inputs["w2"])
    shared = {
        "wq": w8(inputs["wq"]),
        "wk": w8(inputs["wk"]),
        "wv": w8(inputs["wv"]),
        "wo": w8(inputs["wo"]),
        "w1h": w1h, "w1l": w1l,
        "w2h": w2h, "w2l": w2l,
        "bq_pc": f32(np.asarray(inputs["bq"]).reshape(NFC, P).T),
        "bk_pc": f32(np.asarray(inputs["bk"]).reshape(NFC, P).T),
        "bv": f32(inputs["bv"]),
        "bo": f32(inputs["bo"]),
        "b1_pc": f32(np.asarray(inputs["b1"]).reshape(NFFC, P).T),
        "b2": f32(inputs["b2"]),
        "ln1_g": f32(inputs["ln1_g"]),
        "ln1_b": f32(inputs["ln1_b"]),
        "ln2_g": f32(inputs["ln2_g"]),
        "ln2_b": f32(inputs["ln2_b"]),
        "lnf_g": f32(inputs["lnf_g"]),
        "lnf_b": f32(inputs["lnf_b"]),
    }
    in_maps = []
    for c in range(8):
        b, half = c // 2, c % 2
        q0 = half * Sq
        in_maps.append(
            {
                "x1": np.ascontiguousarray(x1[b].astype(bf16)),
                "x2h": np.ascontiguousarray(x2[b, q0:q0 + Sq].astype(bf16)),
                "biasT": np.ascontiguousarray(
                    (attn_bias[b, :, q0:q0 + Sq, :].transpose(0, 2, 1)
                     * (1.0 / SCALE)).astype(fp8)
                ),
                **shared,
            }
        )
    return in_maps


def _assemble(results: list[dict]) -> np.ndarray:
    out = np.empty((B, S, H), np.float32)
    for c in range(8):
        b, half = c // 2, c % 2
        out[b, half * Sq:(half + 1) * Sq] = results[c]["out"]
    return out


def run(inputs: dict, **run_kwargs):
    from concourse.bass_utils import run_bass_kernel_spmd

    ln_affine, with_biases = _detect_fast_flags(inputs)
    nc = _get_program(ln_affine=ln_affine, with_biases=with_biases)
    in_maps = _make_in_maps(inputs)
    res = run_bass_kernel_spmd(nc, in_maps, core_ids=list(range(8)), **run_kwargs)
    return _assemble(res.results), res


def kernel(**inputs) -> np.ndarray:
    out, _ = run(inputs)
    return out
